# revision 27
# baseline (speedup 1.0000x reference)
"""Trainium2 Bass kernel for CustomLossWithCovariance.

loss = abs(logdet(sigma) + mean_b[(p_b - t_b)^T sigma^{-1} (p_b - t_b)])

Only the 3x3 Gram matrix G = sum_b d_b d_b^T (d = pred - targ) requires
touching the [B, 3] data; the device computes per-core partial pair-sums
of G, and the host finishes with the tiny 3x3 algebra:
    mean_mahalanobis = <sigma_inv, G> / B
    loss = |logdet(sigma) + mean_mahalanobis|

Sharding: data-parallel over the batch across 8 NeuronCores (each core
streams a contiguous [B/8, 3] shard; partial sums gathered on host).

Per-core device kernel (raw Bacc, manual semaphores — see
build_gram_kernel_raw; build_gram_kernel is the Tile-framework
baseline kept for reference). Per tile of [128, 2M]:
  - DMA both halves (pred | targ) flat-contiguous in one dma_start
  - DVE: d = pred - targ, in-place into the pred half (unit-stride fp32)
  - ACT: Square(d_i) with accum_out -> per-partition sums of d_i^2
  - DVE: scalar_tensor_tensor(d_i * d_j, accum_out) -> cross sums
  (component APs are stride-3 views of the flat tiles, grouped 4 tiles
  per reduce instruction to amortize fixed costs)
"""

import numpy as np

import concourse.bass as bass
import concourse.bacc as bacc
import concourse.mybir as mybir
from concourse import tile
from concourse.bass_utils import run_bass_kernel_spmd

N_CORES = 8
B_FULL = 8388608
P = 128

_PAIRS = [(0, 1), (0, 2), (1, 2)]


def build_gram_kernel(n_rows: int, n_tiles: int, use_act: bool = True):
    """Build the per-core Bass module.

    Input: pt [2, n_rows, 3] f32 (pred stacked with targ)
    Output: partials [128, 6 * n_tiles] f32
        col t*3+i            : sum over this tile/partition of d_i^2
        col 3*n_tiles + t*3+k: sum of d_i*d_j for pair k in _PAIRS
    """
    assert n_rows % (P * n_tiles) == 0
    r = n_rows // (P * n_tiles)  # rows per partition per tile
    m = 3 * r                    # flat f32 elements per partition per tile
    f32 = mybir.dt.float32

    # Bacc (not plain Bass): its compile() pass legalizes semaphore waits
    # (each TRN2 instruction holds at most one wait slot).
    nc = bacc.Bacc("TRN2", target_bir_lowering=False, debug=False)
    pt = nc.dram_tensor("pt", [2, n_rows, 3], f32, kind="ExternalInput")
    out = nc.dram_tensor("partials", [P, 6 * n_tiles], f32, kind="ExternalOutput")

    # [t][p][w(2), m] — per tile/partition: pred chunk and targ chunk, each
    # m contiguous f32 in DRAM.
    pt_v = pt[:].rearrange("w (t p r) c -> t p w (r c)", t=n_tiles, p=P)

    with tile.TileContext(nc) as tc:
        with (
            tc.tile_pool(name="io", bufs=3) as io_pool,
            tc.tile_pool(name="dve_scr", bufs=2) as dve_scr,
            tc.tile_pool(name="act_scr", bufs=2) as act_scr,
            tc.tile_pool(name="acc", bufs=1) as acc_pool,
        ):
            acc_sq = acc_pool.tile([P, 3 * n_tiles], f32)
            acc_cr = acc_pool.tile([P, 3 * n_tiles], f32)

            for t in range(n_tiles):
                buf = io_pool.tile([P, 2 * m], f32, tag="buf")
                nc.sync.dma_start(
                    out=buf[:].rearrange("p (w m) -> p w m", w=2),
                    in_=pt_v[t],
                )

                # In-place: d = pred - targ, overwriting the pred half.
                nc.vector.tensor_tensor(
                    out=buf[:, 0:m],
                    in0=buf[:, 0:m],
                    in1=buf[:, m : 2 * m],
                    op=mybir.AluOpType.subtract,
                )
                d3 = buf[:, 0:m].rearrange("p (r c) -> p c r", c=3)

                # Diagonal sums on the scalar engine (Square + accum_out),
                # overlapping with the DVE cross-products.
                if use_act:
                    for i in range(3):
                        sq = act_scr.tile([P, r], f32, tag="sq")
                        nc.scalar.activation(
                            out=sq[:],
                            in_=d3[:, i, :],
                            func=mybir.ActivationFunctionType.Square,
                            accum_out=acc_sq[:, t * 3 + i : t * 3 + i + 1],
                        )
                else:
                    for i in range(3):
                        sq = dve_scr.tile([P, r], f32, tag="pr")
                        nc.vector.scalar_tensor_tensor(
                            out=sq[:],
                            in0=d3[:, i, :],
                            scalar=1.0,
                            in1=d3[:, i, :],
                            op0=mybir.AluOpType.mult,
                            op1=mybir.AluOpType.mult,
                            accum_out=acc_sq[:, t * 3 + i : t * 3 + i + 1],
                        )
                # Cross sums: fused multiply+reduce on DVE
                # (scalar_tensor_tensor: out = (in0 * 1.0) * in1, accum = sum).
                for k, (i, j) in enumerate(_PAIRS):
                    pr = dve_scr.tile([P, r], f32, tag="pr")
                    nc.vector.scalar_tensor_tensor(
                        out=pr[:],
                        in0=d3[:, i, :],
                        scalar=1.0,
                        in1=d3[:, j, :],
                        op0=mybir.AluOpType.mult,
                        op1=mybir.AluOpType.mult,
                        accum_out=acc_cr[:, t * 3 + k : t * 3 + k + 1],
                    )

            nc.sync.dma_start(out=out[:, 0 : 3 * n_tiles], in_=acc_sq[:])
            nc.sync.dma_start(out=out[:, 3 * n_tiles : 6 * n_tiles], in_=acc_cr[:])

    nc.compile()
    return nc


def build_gram_kernel_raw(n_rows: int, n_tiles: int = 32, n_bufs: int = 24,
                          group: int = 4, skip_exit_barrier: bool = True):
    """Raw-Bacc variant: manual semaphores, no TileContext.

    Skips Tile's prologue/epilogue (drain + two all-engine EVSEM
    barriers, ~16 us) — the only sync needed is a three-semaphore chain:
    DMA loads (one HWDGE ring) -> DVE -> ACT.

    The ring of tile buffers lives in ONE SBUF tensor so the fused
    multiply-reduces can span `group` consecutive tiles with a single
    instruction (free-dim AP [group, r]) — amortizing the per-op fixed
    cost and the accumulator-drain, which keeps both compute engines
    well under the DMA pace.

    Input: pt [2, n_rows, 3] f32. Output: partials [128, 6 * n_groups]
    (same slot layout as build_gram_kernel, with n_groups slots).
    """
    assert n_tiles % group == 0 and n_bufs % group == 0
    assert n_rows % (P * n_tiles) == 0
    n_groups = n_tiles // group
    r = n_rows // (P * n_tiles)
    m = 3 * r
    f32 = mybir.dt.float32

    nc = bacc.Bacc("TRN2", target_bir_lowering=False, debug=False)
    pt = nc.dram_tensor("pt", [2, n_rows, 3], f32, kind="ExternalInput")
    out = nc.dram_tensor("partials", [P, 6 * n_groups], f32, kind="ExternalOutput")
    pt_v = pt[:].rearrange("w (t p r) c -> t p w (r c)", t=n_tiles, p=P)

    ring = nc.alloc_sbuf_tensor("ring", [P, n_bufs * 2 * m], f32).ap()

    def buf(t):
        s = t % n_bufs
        return ring[:, s * 2 * m : (s + 1) * 2 * m]

    def dgroup(g, i):
        # component i of the diff halves of tiles 4g..4g+3: [128, group, r]
        s0 = (g * group) % n_bufs
        w = ring[:, s0 * 2 * m : (s0 + group) * 2 * m]
        return w.rearrange("p (t w r c) -> p t w c r", t=group, w=2, c=3)[:, :, 0, i, :]

    acc_sq = nc.alloc_sbuf_tensor("acc_sq", [P, 3 * n_groups], f32).ap()
    acc_cr = nc.alloc_sbuf_tensor("acc_cr", [P, 3 * n_groups], f32).ap()
    # Rotated scratch (dead stores of the fused ops), 2 groups deep so each
    # group's single stale semaphore wait also covers the scratch WAW from
    # two groups back.
    pr_scrs = [
        nc.alloc_sbuf_tensor(f"pr_scr{k}", [P, group * r], f32).ap() for k in range(6)
    ]
    sq_scrs = [
        nc.alloc_sbuf_tensor(f"sq_scr{k}", [P, group * r], f32).ap() for k in range(6)
    ]

    # One DMA-completion semaphore per ring buffer: a single shared sem
    # would be unsound — each dma_start is split across 16 SDMA engines
    # whose sub-completions interleave across in-flight DMAs.
    dma_sems = [nc.alloc_semaphore(f"dma_sem{i}") for i in range(n_bufs)]
    out_sem = nc.alloc_semaphore("out_sem")
    dve_sem = nc.alloc_semaphore("dve_sem")
    act_sem = nc.alloc_semaphore("act_sem")

    # DVE emission order: subs run ahead; the grouped multiply-reduces for
    # group g are emitted after sub(4g+4) so their drain-wait on the last
    # sub of the group is already satisfied when it executes (DVE writes
    # drain asynchronously). Only the last group trails the final sub.
    dve_order = []
    for t in range(n_tiles):
        dve_order.append(("sub", t))
        if t % group == 0 and t >= group:
            # one sub of stagger after the group's last sub
            dve_order.append(("stt", t // group - 1))
    dve_order.append(("stt", n_groups - 1))
    sub_done, sttg_done = {}, {}
    v = 0
    for kind, x in dve_order:
        if kind == "sub":
            v += 1
            sub_done[x] = v
        else:
            v += 3
            sttg_done[x] = v

    # Output chunks: flush finished accumulator columns while later tiles
    # still stream, so the tail only waits on the last small chunk.
    chunk = max(1, n_groups // 2)
    chunks = [(c, min(c + chunk, n_groups)) for c in range(0, n_groups, chunk)]

    import contextlib

    @contextlib.contextmanager
    def _block():
        # no_gpsimd_drain=True emits per-engine drains explicitly and then a
        # sem-only all-engine butterfly. The butterfly only delays NEFF end
        # (outputs are already fenced by the sequencer's out_sem wait), so
        # optionally no-op it during Block.__exit__.
        with nc.Block(no_gpsimd_drain=True) as blk:
            try:
                yield blk
            finally:
                if skip_exit_barrier:
                    nc.all_engine_barrier = lambda **kw: None
        if skip_exit_barrier:
            del nc.all_engine_barrier  # restore class method

    with _block() as block:

        @block.sync
        def _(sync):
            for t in range(n_tiles):
                if t >= n_bufs:
                    # ring reuse: all consumers of the buffer's previous
                    # occupant (tile t - n_bufs) must be done
                    prev = t - n_bufs
                    sync.wait_ge(dve_sem, sttg_done[prev // group])
                    sync.wait_ge(act_sem, 3 * (prev // group + 1))
                sync.dma_start(
                    out=buf(t).rearrange("p (w m) -> p w m", w=2),
                    in_=pt_v[t],
                ).then_inc(dma_sems[t % n_bufs], 16)
            n_out = 0
            for lo, hi in chunks:
                sync.wait_ge(act_sem, 3 * hi)
                sync.dma_start(
                    out=out[:, 3 * lo : 3 * hi], in_=acc_sq[:, 3 * lo : 3 * hi]
                ).then_inc(out_sem, 16)
                sync.wait_ge(dve_sem, sttg_done[hi - 1])
                sync.dma_start(
                    out=out[:, 3 * (n_groups + lo) : 3 * (n_groups + hi)],
                    in_=acc_cr[:, 3 * lo : 3 * hi],
                ).then_inc(out_sem, 16)
                n_out += 32
            sync.wait_ge(out_sem, n_out)

        @block.vector
        def _(vector):
            for kind, x in dve_order:
                if kind == "sub":
                    b = buf(x)
                    vector.wait_ge(dma_sems[x % n_bufs], 16 * (x // n_bufs + 1))
                    vector.tensor_tensor(
                        out=b[:, 0:m],
                        in0=b[:, 0:m],
                        in1=b[:, m : 2 * m],
                        op=mybir.AluOpType.subtract,
                    ).then_inc(dve_sem, 1)
                else:
                    vector.wait_ge(dve_sem, sub_done[(x + 1) * group - 1])
                    for k, (i, j) in enumerate(_PAIRS):
                        vector.scalar_tensor_tensor(
                            out=pr_scrs[(x % 2) * 3 + k][:].rearrange(
                                "p (t r) -> p t r", t=group
                            ),
                            in0=dgroup(x, i),
                            scalar=1.0,
                            in1=dgroup(x, j),
                            op0=mybir.AluOpType.mult,
                            op1=mybir.AluOpType.mult,
                            accum_out=acc_cr[:, x * 3 + k : x * 3 + k + 1],
                        ).then_inc(dve_sem, 1)

        @block.scalar
        def _(scalar):
            for g in range(n_groups):
                scalar.wait_ge(dve_sem, sub_done[(g + 1) * group - 1])
                if g >= 2:
                    # scratch slot reuse from two groups back
                    scalar.wait_ge(act_sem, 3 * (g - 1))
                for i in range(3):
                    scalar.activation(
                        out=sq_scrs[(g % 2) * 3 + i][:].rearrange(
                            "p (t r) -> p t r", t=group
                        ),
                        in_=dgroup(g, i),
                        func=mybir.ActivationFunctionType.Square,
                        accum_out=acc_sq[:, g * 3 + i : g * 3 + i + 1],
                    ).then_inc(act_sem, 1)

    nc.compile()
    return nc

def build_gram_kernel_v2(n_rows: int, n_tiles: int = 32, n_bufs: int = 24,
                         group: int = 4, n_dbufs: int = 12,
                         n_scalar_dmas: int = 8, act_squares: int = 3,
                         skip_exit_barrier: bool = True):
    """bf16-deinterleaved variant of build_gram_kernel_raw.

    The fp32 per-tile subtract writes d = pred - targ as bf16 with the
    three vector components DEINTERLEAVED (each component a unit-stride
    block) into a small d-ring.  The multiply-reduces then read bf16 at
    step 1, which unlocks the DVE 2x packed perf mode (fp32/stride-3 in
    the baseline capped DVE at ~85-103 elem/ns and made compute lag the
    402 GB/s DMA stream by ~8 us).

    Other deltas vs the baseline:
      - input-ring reuse only waits on the SUB of the evicted tile (the
        crosses read the d-ring, not the input ring), so the DMA queue
        never stalls on the reduce tail;
      - the first `n_scalar_dmas` tile loads are issued from the scalar
        engine's HWDGE ring in parallel with the sync engine's, halving
        the issue-rate-limited ramp;
      - the accumulator is laid out group-major ([sq0..2 cr0..2] per
        group) so each output flush is ONE dma, and the final flush
        covers only the last group's 6 columns.

    Output: partials [128, 6 * n_groups], col 6g+i = sum d_i^2 of group
    g for i<3, col 6g+3+k = sum d_i*d_j for pair k.
    """
    assert n_tiles % group == 0 and n_bufs % group == 0
    assert n_dbufs % group == 0 and n_dbufs >= 2 * group
    assert n_rows % (P * n_tiles) == 0
    assert 0 <= act_squares <= 3
    n_groups = n_tiles // group
    r = n_rows // (P * n_tiles)
    m = 3 * r
    f32, bf16 = mybir.dt.float32, mybir.dt.bfloat16

    nc = bacc.Bacc("TRN2", target_bir_lowering=False, debug=False)
    pt = nc.dram_tensor("pt", [2, n_rows, 3], f32, kind="ExternalInput")
    out = nc.dram_tensor("partials", [P, 6 * n_groups], f32, kind="ExternalOutput")
    pt_v = pt[:].rearrange("w (t p r) c -> t p w (r c)", t=n_tiles, p=P)

    ring = nc.alloc_sbuf_tensor("ring", [P, n_bufs * 2 * m], f32).ap()
    dring = nc.alloc_sbuf_tensor("dring", [P, n_dbufs * m], bf16).ap()
    acc = nc.alloc_sbuf_tensor("acc", [P, 6 * n_groups], f32).ap()

    def buf(t):
        s = t % n_bufs
        return ring[:, s * 2 * m : (s + 1) * 2 * m]

    def sub_views(t):
        b = buf(t)
        p_v = b[:, 0:m].rearrange("p (r c) -> p r c", c=3)
        t_v = b[:, m : 2 * m].rearrange("p (r c) -> p r c", c=3)
        s = t % n_dbufs
        d_v = dring[:, s * m : (s + 1) * m].rearrange("p (c r) -> p r c", c=3)
        return p_v, t_v, d_v

    def dgroup(g, i):
        # component i of groups' d tiles: [128, group, r], unit inner stride
        s0 = (g * group) % n_dbufs
        w = dring[:, s0 * m : (s0 + group) * m]
        return w.rearrange("p (t c r) -> p t c r", t=group, c=3)[:, :, i, :]

    n_dve_sq = 3 - act_squares
    dve_ops_per_group = 3 + n_dve_sq
    # rotated dead-store scratch (2 groups deep per engine)
    dve_scrs = [
        nc.alloc_sbuf_tensor(f"dve_scr{k}", [P, group * r], bf16).ap()
        for k in range(2 * dve_ops_per_group)
    ]
    act_scrs = [
        nc.alloc_sbuf_tensor(f"act_scr{k}", [P, group * r], bf16).ap()
        for k in range(2 * act_squares)
    ]

    dma_sems = [nc.alloc_semaphore(f"dma_sem{i}") for i in range(n_bufs)]
    out_sem = nc.alloc_semaphore("out_sem")
    dve_sem = nc.alloc_semaphore("dve_sem")
    act_sem = nc.alloc_semaphore("act_sem") if act_squares else None

    # DVE emission order (as baseline): subs run ahead; group g's reduces
    # emitted after sub(4g+4) so their wait is stale when reached.
    dve_order = []
    for t in range(n_tiles):
        dve_order.append(("sub", t))
        if t % group == 0 and t >= group:
            dve_order.append(("red", t // group - 1))
    dve_order.append(("red", n_groups - 1))
    sub_done, redg_done = {}, {}
    v = 0
    for kind, x in dve_order:
        if kind == "sub":
            v += 1
            sub_done[x] = v
        else:
            v += dve_ops_per_group
            redg_done[x] = v

    # output flushes: all-but-last groups early, last group alone at the end
    chunks = [(0, n_groups - 1), (n_groups - 1, n_groups)]

    import contextlib

    @contextlib.contextmanager
    def _block():
        with nc.Block(no_gpsimd_drain=True) as blk:
            try:
                yield blk
            finally:
                if skip_exit_barrier:
                    nc.all_engine_barrier = lambda **kw: None
        if skip_exit_barrier:
            del nc.all_engine_barrier

    def emit_dma(eng, t):
        eng.dma_start(
            out=buf(t).rearrange("p (w m) -> p w m", w=2),
            in_=pt_v[t],
        ).then_inc(dma_sems[t % n_bufs], 16)

    with _block() as block:

        @block.sync
        def _(sync):
            for t in range(n_scalar_dmas, n_tiles):
                if t >= n_bufs:
                    # ring reuse: only the evicted tile's sub must be done
                    sync.wait_ge(dve_sem, sub_done[t - n_bufs])
                emit_dma(sync, t)
            n_out = 0
            for lo, hi in chunks:
                if act_squares:
                    sync.wait_ge(act_sem, act_squares * hi)
                sync.wait_ge(dve_sem, redg_done[hi - 1])
                sync.dma_start(
                    out=out[:, 6 * lo : 6 * hi], in_=acc[:, 6 * lo : 6 * hi]
                ).then_inc(out_sem, 16)
                n_out += 16
            sync.wait_ge(out_sem, n_out)

        @block.scalar
        def _(scalar):
            # early tile loads on the second HWDGE ring (ramp)
            for t in range(n_scalar_dmas):
                emit_dma(scalar, t)
            for g in range(n_groups):
                scalar.wait_ge(dve_sem, sub_done[(g + 1) * group - 1])
                if g >= 2:
                    scalar.wait_ge(act_sem, act_squares * (g - 1))
                for i in range(act_squares):
                    scalar.activation(
                        out=act_scrs[(g % 2) * act_squares + i][:].rearrange(
                            "p (t r) -> p t r", t=group
                        ),
                        in_=dgroup(g, i),
                        func=mybir.ActivationFunctionType.Square,
                        accum_out=acc[:, g * 6 + i : g * 6 + i + 1],
                    ).then_inc(act_sem, 1)

        @block.vector
        def _(vector):
            for kind, x in dve_order:
                if kind == "sub":
                    p_v, t_v, d_v = sub_views(x)
                    vector.wait_ge(dma_sems[x % n_bufs], 16 * (x // n_bufs + 1))
                    if x >= n_dbufs and act_squares:
                        # d-slot reuse: ACT squares of the evicted tile's
                        # group must be done (DVE's own reads are ordered
                        # by program order)
                        gp = (x - n_dbufs) // group
                        vector.wait_ge(act_sem, act_squares * (gp + 1))
                    vector.tensor_tensor(
                        out=d_v,
                        in0=p_v,
                        in1=t_v,
                        op=mybir.AluOpType.subtract,
                    ).then_inc(dve_sem, 1)
                else:
                    vector.wait_ge(dve_sem, sub_done[(x + 1) * group - 1])
                    ops = [(i, j, 3 + k) for k, (i, j) in enumerate(_PAIRS)]
                    ops += [(i, i, i) for i in range(act_squares, 3)]
                    for n, (i, j, col) in enumerate(ops):
                        vector.scalar_tensor_tensor(
                            out=dve_scrs[(x % 2) * dve_ops_per_group + n][:]
                            .rearrange("p (t r) -> p t r", t=group),
                            in0=dgroup(x, i),
                            scalar=1.0,
                            in1=dgroup(x, j),
                            op0=mybir.AluOpType.mult,
                            op1=mybir.AluOpType.mult,
                            accum_out=acc[:, x * 6 + col : x * 6 + col + 1],
                        ).then_inc(dve_sem, 1)

    nc.compile()
    return nc


def build_gram_kernel_v3(n_rows: int, n_tiles: int = 32,
                         group_sizes: tuple = (4, 4, 4, 4, 4, 4, 4, 2, 1, 1),
                         skip_exit_barrier: bool = True):
    """Planar bf16 variant: host supplies component-planar tiles, the DMA
    casts fp32->bf16 in flight (SWDGE), and every on-chip operand is
    unit-stride bf16.

    Input pt [n_tiles, 128, 6r] f32, per (tile, partition) one contiguous
    chunk [p0 r | p1 r | p2 r | t0 r | t1 r | t2 r] (6 KB for r=256 — the
    descriptor sweet spot, ~413 GB/s vs 402 for the baseline's split
    chunks).  SWDGE (gpsimd-issued) DMA casts to bf16 on the fly — probe-
    measured at full read rate.  Per tile the DVE subtract then runs in
    2x packed mode (bf16, step 1): d = pred - targ IN-PLACE over the pred
    half.  Cross/square reduces read d unit-stride (no stride-3 penalty).

    Every tile has its own SBUF slot (32 x 3 KB bf16) and semaphore — no
    ring reuse, so the DMA stream never waits on compute.  Group sizes
    taper at the end so the after-last-DMA tail is only the final tile's
    sub + three N=256 reduces.

    Output: partials [128, 6 * n_groups]; col 6g+i = sum d_i^2, col
    6g+3+k = sum d_i d_j over group g's tiles.
    """
    assert sum(group_sizes) == n_tiles
    assert n_rows % (P * n_tiles) == 0
    n_groups = len(group_sizes)
    r = n_rows // (P * n_tiles)
    h = 3 * r  # bf16 elems per half-tile per partition
    f32, bf16 = mybir.dt.float32, mybir.dt.bfloat16
    max_g = max(group_sizes)

    ends = []
    e = -1
    for sz in group_sizes:
        e += sz
        ends.append(e)
    starts = [e - sz + 1 for e, sz in zip(ends, group_sizes)]

    nc = bacc.Bacc("TRN2", target_bir_lowering=False, debug=False)
    pt = nc.dram_tensor("pt", [n_tiles, P, 2 * h], f32, kind="ExternalInput")
    out = nc.dram_tensor("partials", [P, 6 * n_groups], f32, kind="ExternalOutput")
    pt_v = pt[:]

    ring = nc.alloc_sbuf_tensor("ring", [P, n_tiles * 2 * h], bf16).ap()
    acc = nc.alloc_sbuf_tensor("acc", [P, 6 * n_groups], f32).ap()

    def half(t, w):
        return ring[:, (2 * t + w) * h : (2 * t + w + 1) * h]

    def dgroup(g, i):
        # component i of group g's d (pred) halves: [128, size, r] step-1
        s = starts[g]
        w = ring[:, 2 * s * h : 2 * (s + group_sizes[g]) * h]
        return w.rearrange(
            "p (t w c r) -> p t w c r", t=group_sizes[g], w=2, c=3
        )[:, :, 0, i, :]

    dve_scrs = [
        nc.alloc_sbuf_tensor(f"dve_scr{k}", [P, max_g * r], bf16).ap()
        for k in range(6)
    ]
    act_scrs = [
        nc.alloc_sbuf_tensor(f"act_scr{k}", [P, max_g * r], bf16).ap()
        for k in range(6)
    ]

    dma_sems = [nc.alloc_semaphore(f"dma_sem{t}") for t in range(n_tiles)]
    out_sem = nc.alloc_semaphore("out_sem")
    dve_sem = nc.alloc_semaphore("dve_sem")
    act_sem = nc.alloc_semaphore("act_sem")

    # DVE order: subs run ahead, group reduces staggered one sub late.
    dve_order = []
    for t in range(n_tiles):
        dve_order.append(("sub", t))
        dve_order.extend(("red", g) for g in range(n_groups) if ends[g] == t - 1)
    dve_order.extend(("red", g) for g in range(n_groups) if ends[g] >= n_tiles - 1)
    sub_done, redg_done = {}, {}
    v = 0
    for kind, x in dve_order:
        if kind == "sub":
            v += 1
            sub_done[x] = v
        else:
            v += 3
            redg_done[x] = v

    chunks = [(0, n_groups - 1), (n_groups - 1, n_groups)]

    import contextlib

    @contextlib.contextmanager
    def _block():
        with nc.Block() as blk:
            try:
                yield blk
            finally:
                if skip_exit_barrier:
                    nc.all_engine_barrier = lambda **kw: None
        if skip_exit_barrier:
            del nc.all_engine_barrier

    with _block() as block:

        @block.gpsimd
        def _(gp):
            for t in range(n_tiles):
                gp.dma_start(
                    out=ring[:, 2 * t * h : 2 * (t + 1) * h],
                    in_=pt_v[t],
                ).then_inc(dma_sems[t], 16)

        @block.sync
        def _(sync):
            n_out = 0
            for lo, hi in chunks:
                sync.wait_ge(act_sem, 3 * hi)
                sync.wait_ge(dve_sem, redg_done[hi - 1])
                sync.dma_start(
                    out=out[:, 6 * lo : 6 * hi], in_=acc[:, 6 * lo : 6 * hi]
                ).then_inc(out_sem, 16)
                n_out += 16
            sync.wait_ge(out_sem, n_out)

        @block.scalar
        def _(scalar):
            for g in range(n_groups):
                scalar.wait_ge(dve_sem, sub_done[ends[g]])
                if g >= 2:
                    scalar.wait_ge(act_sem, 3 * (g - 1))
                for i in range(3):
                    scalar.activation(
                        out=act_scrs[(g % 2) * 3 + i][
                            :, : group_sizes[g] * r
                        ].rearrange("p (t r) -> p t r", t=group_sizes[g]),
                        in_=dgroup(g, i),
                        func=mybir.ActivationFunctionType.Square,
                        accum_out=acc[:, g * 6 + i : g * 6 + i + 1],
                    ).then_inc(act_sem, 1)

        @block.vector
        def _(vector):
            for kind, x in dve_order:
                if kind == "sub":
                    vector.wait_ge(dma_sems[x], 16)
                    vector.tensor_tensor(
                        out=half(x, 0),
                        in0=half(x, 0),
                        in1=half(x, 1),
                        op=mybir.AluOpType.subtract,
                    ).then_inc(dve_sem, 1)
                else:
                    vector.wait_ge(dve_sem, sub_done[ends[x]])
                    for k, (i, j) in enumerate(_PAIRS):
                        vector.scalar_tensor_tensor(
                            out=dve_scrs[(x % 2) * 3 + k][
                                :, : group_sizes[x] * r
                            ].rearrange("p (t r) -> p t r", t=group_sizes[x]),
                            in0=dgroup(x, i),
                            scalar=1.0,
                            in1=dgroup(x, j),
                            op0=mybir.AluOpType.mult,
                            op1=mybir.AluOpType.mult,
                            accum_out=acc[:, x * 6 + 3 + k : x * 6 + 4 + k],
                        ).then_inc(dve_sem, 1)

    nc.compile()
    return nc


def build_gram_kernel_v4(n_rows: int, n_tiles: int = 32, n_bufs: int = 16,
                         n_dbufs: int = 32,
                         sub_sizes: tuple = (1,) * 12 + (4, 4, 4, 4, 2, 1, 1),
                         cr_sizes: tuple = (4, 4, 4, 8, 8, 2, 1, 1),
                         sq_sizes: tuple = (4, 4, 4, 4, 4, 4, 4, 2, 1, 1),
                         n_scalar_dmas: int = 0, use_ttr: bool = False,
                         gp_batches: tuple = (13, 15),
                         skip_exit_barrier: bool = True):
    """Planar fp32 over HWDGE + bf16 d-ring; fused sub batches.

    Input pt [n_tiles, 128, 6r] f32 planar (one contiguous 6 KB chunk
    per tile/partition -> ~413 GB/s vs 402 for split chunks; HWDGE so no
    SWDGE descriptor-ring contention, which made SDMA engine 15 a 17%
    straggler that paced the whole SWDGE-cast variant).

    DVE subtracts in fused batches (one op across a batch's tiles, 3D AP
    [t, h] over the fp32 ring) writing unit-stride bf16 into the d-ring;
    cross/square reduces then run on step-1 bf16 (no stride-3 penalty).
    Crosses (DVE) use 8-tile groups, squares (ACT) 4-tile groups — the
    tilings are independent since the host sums all partial columns.
    All sizes taper to 1 tile at the end so the post-stream tail is just
    the last tile's sub + three N=r reduces.

    acc column layout (out [128, 3*(n_sq+n_cr)]): all-but-last sq groups,
    all-but-last cr groups, last sq group, last cr group — so each of the
    two output flushes is ONE contiguous dma and the final flush covers
    only the last-tile groups.
    """
    assert sum(sub_sizes) == n_tiles and sum(cr_sizes) == n_tiles
    assert sum(sq_sizes) == n_tiles
    assert n_rows % (P * n_tiles) == 0
    r = n_rows // (P * n_tiles)
    h = 3 * r
    f32, bf16 = mybir.dt.float32, mybir.dt.bfloat16

    def bounds(sizes):
        ends, e = [], -1
        for sz in sizes:
            e += sz
            ends.append(e)
        return [e - sz + 1 for e, sz in zip(ends, sizes)], ends

    sub_starts, sub_ends = bounds(sub_sizes)
    cr_starts, cr_ends = bounds(cr_sizes)
    sq_starts, sq_ends = bounds(sq_sizes)
    n_sq, n_cr = len(sq_sizes), len(cr_sizes)
    # fused subs and cross groups need their tiles contiguous in the rings
    for s, e in zip(sub_starts, sub_ends):
        assert (s % n_bufs) + (e - s) < n_bufs and (s % n_dbufs) + (e - s) < n_dbufs
    for s, e in zip(cr_starts, cr_ends):
        assert (s % n_dbufs) + (e - s) < n_dbufs
    for s, e in zip(sq_starts, sq_ends):
        assert (s % n_dbufs) + (e - s) < n_dbufs

    # acc columns ordered so the early flush [0, chunk_split) covers all
    # groups finishing by tile E1, and the final flush only the rest
    E1 = n_tiles - 5
    sq_early = [g for g in range(n_sq) if sq_ends[g] <= E1]
    sq_late = [g for g in range(n_sq) if sq_ends[g] > E1]
    cr_early = [g for g in range(n_cr) if cr_ends[g] <= E1]
    cr_late = [g for g in range(n_cr) if cr_ends[g] > E1]
    # groups complete in index order on each engine, so early must be a prefix
    assert sq_early == list(range(len(sq_early)))
    assert cr_early == list(range(len(cr_early)))
    sq_col, cr_col = {}, {}
    c = 0
    for g in sq_early:
        sq_col[g] = c; c += 3
    for g in cr_early:
        cr_col[g] = c; c += 3
    chunk_split = c
    for g in sq_late:
        sq_col[g] = c; c += 3
    for g in cr_late:
        cr_col[g] = c; c += 3
    n_cols = c

    nc = bacc.Bacc("TRN2", target_bir_lowering=False, debug=False)
    pt = nc.dram_tensor("pt", [n_tiles, P, 2 * h], f32, kind="ExternalInput")
    out = nc.dram_tensor("partials", [P, n_cols], f32, kind="ExternalOutput")
    pt_v = pt[:]

    ring = nc.alloc_sbuf_tensor("ring", [P, n_bufs * 2 * h], f32).ap()
    dring = nc.alloc_sbuf_tensor("dring", [P, n_dbufs * h], bf16).ap()
    acc = nc.alloc_sbuf_tensor("acc", [P, n_cols], f32).ap()
    max_cr = max(cr_sizes)
    max_sq = max(sq_sizes)
    dve_scrs = [
        nc.alloc_sbuf_tensor(f"dve_scr{k}", [P, max_cr * r], bf16).ap()
        for k in range(3)
    ]
    act_scrs = [
        nc.alloc_sbuf_tensor(f"act_scr{k}", [P, max_sq * r], bf16).ap()
        for k in range(3)
    ]

    dma_sems = [nc.alloc_semaphore(f"dma_sem{i}") for i in range(n_bufs)]
    out_sem = nc.alloc_semaphore("out_sem")
    dve_sem = nc.alloc_semaphore("dve_sem")
    act_sem = nc.alloc_semaphore("act_sem")
    gp_sem = nc.alloc_semaphore("gp_sem")

    def sub_views(b):
        s, sz = sub_starts[b], sub_sizes[b]
        rs = s % n_bufs
        w = ring[:, rs * 2 * h : (rs + sz) * 2 * h].rearrange(
            "p (t w h) -> p t w h", t=sz, w=2)
        ds = s % n_dbufs
        d = dring[:, ds * h : (ds + sz) * h].rearrange(
            "p (t h) -> p t h", t=sz)
        return w[:, :, 0, :], w[:, :, 1, :], d

    def dgroup(starts, sizes, g, i):
        s, sz = starts[g], sizes[g]
        ds = s % n_dbufs
        w = dring[:, ds * h : (ds + sz) * h]
        return w.rearrange("p (t c r) -> p t c r", t=sz, c=3)[:, :, i, :]

    # Sub batches listed in gp_batches run on the (otherwise idle) GpSimd
    # engine — DVE sub work shrinks below the DMA-arrival window so
    # schedule jitter can't cascade.  GpSimd elementwise is ~2.2 ns/elem,
    # under the 4-tile arrival period.
    n_batches = len(sub_sizes)
    gp_batches = tuple(sorted(gp_batches))
    assert all(0 <= b < n_batches for b in gp_batches)
    gp_done = {b: i + 1 for i, b in enumerate(gp_batches)}

    # DVE emission: cross group g emitted as soon as every DVE sub batch
    # overlapping it has been emitted (no stagger — a stagger delays
    # crosses a whole batch and starves the DVE when batches wait on DMA
    # arrivals; gp-run batches are covered by gp_sem waits instead)
    def overlapping_dve_batches(g):
        return [b for b in range(n_batches)
                if b not in gp_done
                and sub_ends[b] >= cr_starts[g] and sub_starts[b] <= cr_ends[g]]

    dve_order = []
    emitted_cr = set()
    for b in range(n_batches):
        if b in gp_done:
            continue
        dve_order.append(("sub", b))
        for g in range(n_cr):
            if g not in emitted_cr and all(
                bb <= b for bb in overlapping_dve_batches(g)
            ):
                emitted_cr.add(g)
                dve_order.append(("cr", g))
    assert len(emitted_cr) == n_cr
    sub_done, crg_done = {}, {}
    v = 0
    for kind, x in dve_order:
        v += 1 if kind == "sub" else 3
        (sub_done if kind == "sub" else crg_done)[x] = v

    def sub_targets(last_tile):
        """(dve_sem, gp_sem) targets ensuring every sub batch touching
        tiles <= last_tile has completed."""
        dve_t = gp_t = 0
        for b in range(n_batches):
            if sub_starts[b] <= last_tile:
                if b in gp_done:
                    gp_t = max(gp_t, gp_done[b])
                else:
                    dve_t = max(dve_t, sub_done[b])
        return dve_t, gp_t

    import contextlib

    @contextlib.contextmanager
    def _block():
        with nc.Block(no_gpsimd_drain=True) as blk:
            try:
                yield blk
            finally:
                if skip_exit_barrier:
                    nc.all_engine_barrier = lambda **kw: None
        if skip_exit_barrier:
            del nc.all_engine_barrier

    def emit_dma(eng, t):
        # split each partition's 6 KB chunk into two 3 KB descriptors:
        # >4 KB packets degrade SDMA round-robin under concurrent engine
        # load (SDMA 15 became a 20% straggler and paced every tile sem)
        eng.dma_start(
            out=ring[:, (t % n_bufs) * 2 * h : (t % n_bufs + 1) * 2 * h]
            .rearrange("p (x k) -> p x k", x=2),
            in_=pt_v[t].rearrange("p (x k) -> p x k", x=2),
        ).then_inc(dma_sems[t % n_bufs], 16)

    with _block() as block:

        if gp_batches:

            @block.gpsimd
            def _(gp):
                for b in gp_batches:
                    e = sub_ends[b]
                    gp.wait_ge(dma_sems[e % n_bufs], 16 * (e // n_bufs + 1))
                    p_v, t_v, d_v = sub_views(b)
                    gp.tensor_tensor(
                        out=d_v, in0=p_v, in1=t_v,
                        op=mybir.AluOpType.subtract,
                    ).then_inc(gp_sem, 1)

        @block.sync
        def _(sync):
            for t in range(n_scalar_dmas, n_tiles):
                if t >= n_bufs:
                    dve_t, gp_t = sub_targets(t - n_bufs)
                    if dve_t:
                        sync.wait_ge(dve_sem, dve_t)
                    if gp_t:
                        sync.wait_ge(gp_sem, gp_t)
                emit_dma(sync, t)
            n_out = 0
            for lo, hi, n_sq_done, n_cr_done in (
                (0, chunk_split, len(sq_early), len(cr_early)),
                (chunk_split, n_cols, n_sq, n_cr),
            ):
                sync.wait_ge(act_sem, 3 * n_sq_done)
                sync.wait_ge(dve_sem, crg_done[n_cr_done - 1])
                sync.dma_start(
                    out=out[:, lo:hi], in_=acc[:, lo:hi]
                ).then_inc(out_sem, 16)
                n_out += 16
            sync.wait_ge(out_sem, n_out)

        @block.scalar
        def _(scalar):
            for t in range(n_scalar_dmas):
                emit_dma(scalar, t)
            for g in range(n_sq):
                dve_t, gp_t = sub_targets(sq_ends[g])
                if dve_t:
                    scalar.wait_ge(dve_sem, dve_t)
                if gp_t:
                    scalar.wait_ge(gp_sem, gp_t)
                for i in range(3):
                    scalar.activation(
                        out=act_scrs[i][:, : sq_sizes[g] * r].rearrange(
                            "p (t r) -> p t r", t=sq_sizes[g]),
                        in_=dgroup(sq_starts, sq_sizes, g, i),
                        func=mybir.ActivationFunctionType.Square,
                        accum_out=acc[:, sq_col[g] + i : sq_col[g] + i + 1],
                    ).then_inc(act_sem, 1)

        @block.vector
        def _(vector):
            for kind, x in dve_order:
                if kind == "sub":
                    p_v, t_v, d_v = sub_views(x)
                    s, e = sub_starts[x], sub_ends[x]
                    vector.wait_ge(dma_sems[e % n_bufs], 16 * (e // n_bufs + 1))
                    if s >= n_dbufs:
                        # d-slot reuse: ACT squares over the evicted tiles
                        # must be done (DVE's own reads are program-ordered)
                        gp = next(g for g in range(n_sq)
                                  if sq_ends[g] >= e - n_dbufs)
                        vector.wait_ge(act_sem, 3 * (gp + 1))
                    vector.tensor_tensor(
                        out=d_v, in0=p_v, in1=t_v,
                        op=mybir.AluOpType.subtract,
                    ).then_inc(dve_sem, 1)
                else:
                    dve_t, gp_t = sub_targets(cr_ends[x])
                    if dve_t:
                        vector.wait_ge(dve_sem, dve_t)
                    if gp_t:
                        vector.wait_ge(gp_sem, gp_t)
                    for k, (i, j) in enumerate(_PAIRS):
                        scr = dve_scrs[k][:, : cr_sizes[x] * r].rearrange(
                            "p (t r) -> p t r", t=cr_sizes[x])
                        if use_ttr:
                            # fused multiply + sum-reduce, accumulator
                            # written directly by the instruction
                            vector.tensor_tensor_reduce(
                                out=scr,
                                in0=dgroup(cr_starts, cr_sizes, x, i),
                                in1=dgroup(cr_starts, cr_sizes, x, j),
                                scale=1.0,
                                scalar=0.0,
                                op0=mybir.AluOpType.mult,
                                op1=mybir.AluOpType.add,
                                accum_out=acc[:, cr_col[x] + k : cr_col[x] + k + 1],
                            ).then_inc(dve_sem, 1)
                        else:
                            vector.scalar_tensor_tensor(
                                out=scr,
                                in0=dgroup(cr_starts, cr_sizes, x, i),
                                scalar=1.0,
                                in1=dgroup(cr_starts, cr_sizes, x, j),
                                op0=mybir.AluOpType.mult,
                                op1=mybir.AluOpType.mult,
                                accum_out=acc[:, cr_col[x] + k : cr_col[x] + k + 1],
                            ).then_inc(dve_sem, 1)

    # record triple-row roles for the host-side unpack
    _V4_SQ_ROWS.clear()
    _V4_SQ_ROWS.extend(sq_col[g] // 3 for g in range(n_sq))
    _V4_CR_ROWS.clear()
    _V4_CR_ROWS.extend(cr_col[g] // 3 for g in range(n_cr))

    nc.compile()
    return nc


_V4_SQ_ROWS: list = []
_V4_CR_ROWS: list = []


def gram_from_partials_v4(partials: np.ndarray) -> np.ndarray:
    """v4 partials [..., 128, 3*(n_sq+n_cr)] -> 3x3 Gram (float64).

    Column triples ordered [sq_early, cr_early, sq_late, cr_late]; the
    builder records which triple rows are squares vs crosses in
    _V4_SQ_ROWS/_V4_CR_ROWS.
    """
    s = partials.astype(np.float64).reshape(-1, partials.shape[-1]).sum(axis=0)
    tri = s.reshape(-1, 3)
    sq = tri[_V4_SQ_ROWS].sum(axis=0)
    cr = tri[_V4_CR_ROWS].sum(axis=0)
    g = np.empty((3, 3), dtype=np.float64)
    g[0, 0], g[1, 1], g[2, 2] = sq
    for k, (i, j) in enumerate(_PAIRS):
        g[i, j] = g[j, i] = cr[k]
    return g


def planarize(predictions: np.ndarray, targets: np.ndarray,
              n_tiles: int = 32) -> np.ndarray:
    """[B,3] pred/targ -> per-core planar tiles [cores, n_tiles, P, 6r] f32."""
    b = predictions.shape[0]
    n_rows = b // N_CORES
    r = n_rows // (P * n_tiles)
    out = np.empty((N_CORES, n_tiles, P, 6 * r), dtype=np.float32)
    pv = out[..., : 3 * r].reshape(N_CORES, n_tiles, P, 3, r)
    tv = out[..., 3 * r :].reshape(N_CORES, n_tiles, P, 3, r)
    pv[:] = np.asarray(predictions, dtype=np.float32).reshape(
        N_CORES, n_tiles, P, r, 3).transpose(0, 1, 2, 4, 3)
    tv[:] = np.asarray(targets, dtype=np.float32).reshape(
        N_CORES, n_tiles, P, r, 3).transpose(0, 1, 2, 4, 3)
    return out


_NC_CACHE: dict[tuple, object] = {}


def _get_nc(n_rows: int, n_tiles: int, use_act: bool, raw: bool = False,
            group: int = 4, **kw):
    key = (n_rows, n_tiles, use_act, raw, group, tuple(sorted(kw.items())))
    if key not in _NC_CACHE:
        if raw:
            _NC_CACHE[key] = build_gram_kernel_v4(n_rows, n_tiles, **kw)
        else:
            _NC_CACHE[key] = build_gram_kernel(n_rows, n_tiles, use_act)
    return _NC_CACHE[key]


def gram_from_partials(partials: np.ndarray, n_tiles: int | None = None) -> np.ndarray:
    """Device partials -> full 3x3 Gram matrix (float64).

    Dispatches on column count: 51 -> v4 layout, else v2/v3 group-major
    layout (col 6g+i = sq_i, col 6g+3+k = cross pair k).
    """
    if partials.shape[-1] == 3 * (len(_V4_SQ_ROWS) + len(_V4_CR_ROWS)):
        return gram_from_partials_v4(partials)
    slots = partials.shape[-1] // 6
    s = partials.astype(np.float64).reshape(-1, slots, 6).sum(axis=0).sum(axis=0)
    g = np.empty((3, 3), dtype=np.float64)
    g[0, 0], g[1, 1], g[2, 2] = s[0:3]
    for k, (i, j) in enumerate(_PAIRS):
        g[i, j] = g[j, i] = s[3 + k]
    return g


def run_device_partials(predictions: np.ndarray, targets: np.ndarray,
                        n_tiles: int = 32, use_act: bool = True,
                        raw: bool = True, group: int = 4, **run_kwargs):
    """Shard over N_CORES, run on device, return per-core partials + results."""
    b = predictions.shape[0]
    assert b % N_CORES == 0
    n_rows = b // N_CORES
    nc = _get_nc(n_rows, n_tiles, use_act, raw, group)
    planar = planarize(predictions, targets, n_tiles)
    in_maps = [{"pt": planar[c]} for c in range(N_CORES)]
    res = run_bass_kernel_spmd(nc, in_maps, list(range(N_CORES)), **run_kwargs)
    partials = np.stack([r["partials"] for r in res.results])
    return partials, res


def kernel(predictions: np.ndarray, targets: np.ndarray, sigma: np.ndarray) -> np.ndarray:
    predictions = np.asarray(predictions, dtype=np.float32)
    targets = np.asarray(targets, dtype=np.float32)
    sigma64 = np.asarray(sigma, dtype=np.float64)

    partials, _ = run_device_partials(predictions, targets, n_tiles=32, raw=True)
    g = gram_from_partials(partials)

    sigma_inv = np.linalg.inv(sigma64)
    _, logdet = np.linalg.slogdet(sigma64)
    mean_mahal = float((sigma_inv * g).sum()) / predictions.shape[0]
    loss = abs(logdet + mean_mahal)
    return np.float32(loss)



# revision 28
# speedup vs baseline: 1.1102x; 1.1102x over previous
"""Trainium2 Bass kernel for CustomLossWithCovariance.

loss = abs(logdet(sigma) + mean_b[(p_b - t_b)^T sigma^{-1} (p_b - t_b)])

Only the 3x3 Gram matrix G = sum_b d_b d_b^T (d = pred - targ) requires
touching the [B, 3] data; the device computes per-core partial pair-sums
of G, and the host finishes with the tiny 3x3 algebra:
    mean_mahalanobis = <sigma_inv, G> / B
    loss = |logdet(sigma) + mean_mahalanobis|

Sharding: data-parallel over the batch across 8 NeuronCores (each core
streams a contiguous [B/8, 3] shard; partial sums gathered on host).

Per-core device kernel (raw Bacc, manual semaphores — see
build_gram_kernel_raw; build_gram_kernel is the Tile-framework
baseline kept for reference). Per tile of [128, 2M]:
  - DMA both halves (pred | targ) flat-contiguous in one dma_start
  - DVE: d = pred - targ, in-place into the pred half (unit-stride fp32)
  - ACT: Square(d_i) with accum_out -> per-partition sums of d_i^2
  - DVE: scalar_tensor_tensor(d_i * d_j, accum_out) -> cross sums
  (component APs are stride-3 views of the flat tiles, grouped 4 tiles
  per reduce instruction to amortize fixed costs)
"""

import numpy as np

import concourse.bass as bass
import concourse.bacc as bacc
import concourse.mybir as mybir
from concourse import tile
from concourse.bass_utils import run_bass_kernel_spmd

N_CORES = 8
B_FULL = 8388608
P = 128

_PAIRS = [(0, 1), (0, 2), (1, 2)]


def build_gram_kernel(n_rows: int, n_tiles: int, use_act: bool = True):
    """Build the per-core Bass module.

    Input: pt [2, n_rows, 3] f32 (pred stacked with targ)
    Output: partials [128, 6 * n_tiles] f32
        col t*3+i            : sum over this tile/partition of d_i^2
        col 3*n_tiles + t*3+k: sum of d_i*d_j for pair k in _PAIRS
    """
    assert n_rows % (P * n_tiles) == 0
    r = n_rows // (P * n_tiles)  # rows per partition per tile
    m = 3 * r                    # flat f32 elements per partition per tile
    f32 = mybir.dt.float32

    # Bacc (not plain Bass): its compile() pass legalizes semaphore waits
    # (each TRN2 instruction holds at most one wait slot).
    nc = bacc.Bacc("TRN2", target_bir_lowering=False, debug=False)
    pt = nc.dram_tensor("pt", [2, n_rows, 3], f32, kind="ExternalInput")
    out = nc.dram_tensor("partials", [P, 6 * n_tiles], f32, kind="ExternalOutput")

    # [t][p][w(2), m] — per tile/partition: pred chunk and targ chunk, each
    # m contiguous f32 in DRAM.
    pt_v = pt[:].rearrange("w (t p r) c -> t p w (r c)", t=n_tiles, p=P)

    with tile.TileContext(nc) as tc:
        with (
            tc.tile_pool(name="io", bufs=3) as io_pool,
            tc.tile_pool(name="dve_scr", bufs=2) as dve_scr,
            tc.tile_pool(name="act_scr", bufs=2) as act_scr,
            tc.tile_pool(name="acc", bufs=1) as acc_pool,
        ):
            acc_sq = acc_pool.tile([P, 3 * n_tiles], f32)
            acc_cr = acc_pool.tile([P, 3 * n_tiles], f32)

            for t in range(n_tiles):
                buf = io_pool.tile([P, 2 * m], f32, tag="buf")
                nc.sync.dma_start(
                    out=buf[:].rearrange("p (w m) -> p w m", w=2),
                    in_=pt_v[t],
                )

                # In-place: d = pred - targ, overwriting the pred half.
                nc.vector.tensor_tensor(
                    out=buf[:, 0:m],
                    in0=buf[:, 0:m],
                    in1=buf[:, m : 2 * m],
                    op=mybir.AluOpType.subtract,
                )
                d3 = buf[:, 0:m].rearrange("p (r c) -> p c r", c=3)

                # Diagonal sums on the scalar engine (Square + accum_out),
                # overlapping with the DVE cross-products.
                if use_act:
                    for i in range(3):
                        sq = act_scr.tile([P, r], f32, tag="sq")
                        nc.scalar.activation(
                            out=sq[:],
                            in_=d3[:, i, :],
                            func=mybir.ActivationFunctionType.Square,
                            accum_out=acc_sq[:, t * 3 + i : t * 3 + i + 1],
                        )
                else:
                    for i in range(3):
                        sq = dve_scr.tile([P, r], f32, tag="pr")
                        nc.vector.scalar_tensor_tensor(
                            out=sq[:],
                            in0=d3[:, i, :],
                            scalar=1.0,
                            in1=d3[:, i, :],
                            op0=mybir.AluOpType.mult,
                            op1=mybir.AluOpType.mult,
                            accum_out=acc_sq[:, t * 3 + i : t * 3 + i + 1],
                        )
                # Cross sums: fused multiply+reduce on DVE
                # (scalar_tensor_tensor: out = (in0 * 1.0) * in1, accum = sum).
                for k, (i, j) in enumerate(_PAIRS):
                    pr = dve_scr.tile([P, r], f32, tag="pr")
                    nc.vector.scalar_tensor_tensor(
                        out=pr[:],
                        in0=d3[:, i, :],
                        scalar=1.0,
                        in1=d3[:, j, :],
                        op0=mybir.AluOpType.mult,
                        op1=mybir.AluOpType.mult,
                        accum_out=acc_cr[:, t * 3 + k : t * 3 + k + 1],
                    )

            nc.sync.dma_start(out=out[:, 0 : 3 * n_tiles], in_=acc_sq[:])
            nc.sync.dma_start(out=out[:, 3 * n_tiles : 6 * n_tiles], in_=acc_cr[:])

    nc.compile()
    return nc


def build_gram_kernel_raw(n_rows: int, n_tiles: int = 32, n_bufs: int = 24,
                          group: int = 4, skip_exit_barrier: bool = True):
    """Raw-Bacc variant: manual semaphores, no TileContext.

    Skips Tile's prologue/epilogue (drain + two all-engine EVSEM
    barriers, ~16 us) — the only sync needed is a three-semaphore chain:
    DMA loads (one HWDGE ring) -> DVE -> ACT.

    The ring of tile buffers lives in ONE SBUF tensor so the fused
    multiply-reduces can span `group` consecutive tiles with a single
    instruction (free-dim AP [group, r]) — amortizing the per-op fixed
    cost and the accumulator-drain, which keeps both compute engines
    well under the DMA pace.

    Input: pt [2, n_rows, 3] f32. Output: partials [128, 6 * n_groups]
    (same slot layout as build_gram_kernel, with n_groups slots).
    """
    assert n_tiles % group == 0 and n_bufs % group == 0
    assert n_rows % (P * n_tiles) == 0
    n_groups = n_tiles // group
    r = n_rows // (P * n_tiles)
    m = 3 * r
    f32 = mybir.dt.float32

    nc = bacc.Bacc("TRN2", target_bir_lowering=False, debug=False)
    pt = nc.dram_tensor("pt", [2, n_rows, 3], f32, kind="ExternalInput")
    out = nc.dram_tensor("partials", [P, 6 * n_groups], f32, kind="ExternalOutput")
    pt_v = pt[:].rearrange("w (t p r) c -> t p w (r c)", t=n_tiles, p=P)

    ring = nc.alloc_sbuf_tensor("ring", [P, n_bufs * 2 * m], f32).ap()

    def buf(t):
        s = t % n_bufs
        return ring[:, s * 2 * m : (s + 1) * 2 * m]

    def dgroup(g, i):
        # component i of the diff halves of tiles 4g..4g+3: [128, group, r]
        s0 = (g * group) % n_bufs
        w = ring[:, s0 * 2 * m : (s0 + group) * 2 * m]
        return w.rearrange("p (t w r c) -> p t w c r", t=group, w=2, c=3)[:, :, 0, i, :]

    acc_sq = nc.alloc_sbuf_tensor("acc_sq", [P, 3 * n_groups], f32).ap()
    acc_cr = nc.alloc_sbuf_tensor("acc_cr", [P, 3 * n_groups], f32).ap()
    # Rotated scratch (dead stores of the fused ops), 2 groups deep so each
    # group's single stale semaphore wait also covers the scratch WAW from
    # two groups back.
    pr_scrs = [
        nc.alloc_sbuf_tensor(f"pr_scr{k}", [P, group * r], f32).ap() for k in range(6)
    ]
    sq_scrs = [
        nc.alloc_sbuf_tensor(f"sq_scr{k}", [P, group * r], f32).ap() for k in range(6)
    ]

    # One DMA-completion semaphore per ring buffer: a single shared sem
    # would be unsound — each dma_start is split across 16 SDMA engines
    # whose sub-completions interleave across in-flight DMAs.
    dma_sems = [nc.alloc_semaphore(f"dma_sem{i}") for i in range(n_bufs)]
    out_sem = nc.alloc_semaphore("out_sem")
    dve_sem = nc.alloc_semaphore("dve_sem")
    act_sem = nc.alloc_semaphore("act_sem")

    # DVE emission order: subs run ahead; the grouped multiply-reduces for
    # group g are emitted after sub(4g+4) so their drain-wait on the last
    # sub of the group is already satisfied when it executes (DVE writes
    # drain asynchronously). Only the last group trails the final sub.
    dve_order = []
    for t in range(n_tiles):
        dve_order.append(("sub", t))
        if t % group == 0 and t >= group:
            # one sub of stagger after the group's last sub
            dve_order.append(("stt", t // group - 1))
    dve_order.append(("stt", n_groups - 1))
    sub_done, sttg_done = {}, {}
    v = 0
    for kind, x in dve_order:
        if kind == "sub":
            v += 1
            sub_done[x] = v
        else:
            v += 3
            sttg_done[x] = v

    # Output chunks: flush finished accumulator columns while later tiles
    # still stream, so the tail only waits on the last small chunk.
    chunk = max(1, n_groups // 2)
    chunks = [(c, min(c + chunk, n_groups)) for c in range(0, n_groups, chunk)]

    import contextlib

    @contextlib.contextmanager
    def _block():
        # no_gpsimd_drain=True emits per-engine drains explicitly and then a
        # sem-only all-engine butterfly. The butterfly only delays NEFF end
        # (outputs are already fenced by the sequencer's out_sem wait), so
        # optionally no-op it during Block.__exit__.
        with nc.Block(no_gpsimd_drain=True) as blk:
            try:
                yield blk
            finally:
                if skip_exit_barrier:
                    nc.all_engine_barrier = lambda **kw: None
        if skip_exit_barrier:
            del nc.all_engine_barrier  # restore class method

    with _block() as block:

        @block.sync
        def _(sync):
            for t in range(n_tiles):
                if t >= n_bufs:
                    # ring reuse: all consumers of the buffer's previous
                    # occupant (tile t - n_bufs) must be done
                    prev = t - n_bufs
                    sync.wait_ge(dve_sem, sttg_done[prev // group])
                    sync.wait_ge(act_sem, 3 * (prev // group + 1))
                sync.dma_start(
                    out=buf(t).rearrange("p (w m) -> p w m", w=2),
                    in_=pt_v[t],
                ).then_inc(dma_sems[t % n_bufs], 16)
            n_out = 0
            for lo, hi in chunks:
                sync.wait_ge(act_sem, 3 * hi)
                sync.dma_start(
                    out=out[:, 3 * lo : 3 * hi], in_=acc_sq[:, 3 * lo : 3 * hi]
                ).then_inc(out_sem, 16)
                sync.wait_ge(dve_sem, sttg_done[hi - 1])
                sync.dma_start(
                    out=out[:, 3 * (n_groups + lo) : 3 * (n_groups + hi)],
                    in_=acc_cr[:, 3 * lo : 3 * hi],
                ).then_inc(out_sem, 16)
                n_out += 32
            sync.wait_ge(out_sem, n_out)

        @block.vector
        def _(vector):
            for kind, x in dve_order:
                if kind == "sub":
                    b = buf(x)
                    vector.wait_ge(dma_sems[x % n_bufs], 16 * (x // n_bufs + 1))
                    vector.tensor_tensor(
                        out=b[:, 0:m],
                        in0=b[:, 0:m],
                        in1=b[:, m : 2 * m],
                        op=mybir.AluOpType.subtract,
                    ).then_inc(dve_sem, 1)
                else:
                    vector.wait_ge(dve_sem, sub_done[(x + 1) * group - 1])
                    for k, (i, j) in enumerate(_PAIRS):
                        vector.scalar_tensor_tensor(
                            out=pr_scrs[(x % 2) * 3 + k][:].rearrange(
                                "p (t r) -> p t r", t=group
                            ),
                            in0=dgroup(x, i),
                            scalar=1.0,
                            in1=dgroup(x, j),
                            op0=mybir.AluOpType.mult,
                            op1=mybir.AluOpType.mult,
                            accum_out=acc_cr[:, x * 3 + k : x * 3 + k + 1],
                        ).then_inc(dve_sem, 1)

        @block.scalar
        def _(scalar):
            for g in range(n_groups):
                scalar.wait_ge(dve_sem, sub_done[(g + 1) * group - 1])
                if g >= 2:
                    # scratch slot reuse from two groups back
                    scalar.wait_ge(act_sem, 3 * (g - 1))
                for i in range(3):
                    scalar.activation(
                        out=sq_scrs[(g % 2) * 3 + i][:].rearrange(
                            "p (t r) -> p t r", t=group
                        ),
                        in_=dgroup(g, i),
                        func=mybir.ActivationFunctionType.Square,
                        accum_out=acc_sq[:, g * 3 + i : g * 3 + i + 1],
                    ).then_inc(act_sem, 1)

    nc.compile()
    return nc

def build_gram_kernel_v2(n_rows: int, n_tiles: int = 32, n_bufs: int = 24,
                         group: int = 4, n_dbufs: int = 12,
                         n_scalar_dmas: int = 8, act_squares: int = 3,
                         skip_exit_barrier: bool = True):
    """bf16-deinterleaved variant of build_gram_kernel_raw.

    The fp32 per-tile subtract writes d = pred - targ as bf16 with the
    three vector components DEINTERLEAVED (each component a unit-stride
    block) into a small d-ring.  The multiply-reduces then read bf16 at
    step 1, which unlocks the DVE 2x packed perf mode (fp32/stride-3 in
    the baseline capped DVE at ~85-103 elem/ns and made compute lag the
    402 GB/s DMA stream by ~8 us).

    Other deltas vs the baseline:
      - input-ring reuse only waits on the SUB of the evicted tile (the
        crosses read the d-ring, not the input ring), so the DMA queue
        never stalls on the reduce tail;
      - the first `n_scalar_dmas` tile loads are issued from the scalar
        engine's HWDGE ring in parallel with the sync engine's, halving
        the issue-rate-limited ramp;
      - the accumulator is laid out group-major ([sq0..2 cr0..2] per
        group) so each output flush is ONE dma, and the final flush
        covers only the last group's 6 columns.

    Output: partials [128, 6 * n_groups], col 6g+i = sum d_i^2 of group
    g for i<3, col 6g+3+k = sum d_i*d_j for pair k.
    """
    assert n_tiles % group == 0 and n_bufs % group == 0
    assert n_dbufs % group == 0 and n_dbufs >= 2 * group
    assert n_rows % (P * n_tiles) == 0
    assert 0 <= act_squares <= 3
    n_groups = n_tiles // group
    r = n_rows // (P * n_tiles)
    m = 3 * r
    f32, bf16 = mybir.dt.float32, mybir.dt.bfloat16

    nc = bacc.Bacc("TRN2", target_bir_lowering=False, debug=False)
    pt = nc.dram_tensor("pt", [2, n_rows, 3], f32, kind="ExternalInput")
    out = nc.dram_tensor("partials", [P, 6 * n_groups], f32, kind="ExternalOutput")
    pt_v = pt[:].rearrange("w (t p r) c -> t p w (r c)", t=n_tiles, p=P)

    ring = nc.alloc_sbuf_tensor("ring", [P, n_bufs * 2 * m], f32).ap()
    dring = nc.alloc_sbuf_tensor("dring", [P, n_dbufs * m], bf16).ap()
    acc = nc.alloc_sbuf_tensor("acc", [P, 6 * n_groups], f32).ap()

    def buf(t):
        s = t % n_bufs
        return ring[:, s * 2 * m : (s + 1) * 2 * m]

    def sub_views(t):
        b = buf(t)
        p_v = b[:, 0:m].rearrange("p (r c) -> p r c", c=3)
        t_v = b[:, m : 2 * m].rearrange("p (r c) -> p r c", c=3)
        s = t % n_dbufs
        d_v = dring[:, s * m : (s + 1) * m].rearrange("p (c r) -> p r c", c=3)
        return p_v, t_v, d_v

    def dgroup(g, i):
        # component i of groups' d tiles: [128, group, r], unit inner stride
        s0 = (g * group) % n_dbufs
        w = dring[:, s0 * m : (s0 + group) * m]
        return w.rearrange("p (t c r) -> p t c r", t=group, c=3)[:, :, i, :]

    n_dve_sq = 3 - act_squares
    dve_ops_per_group = 3 + n_dve_sq
    # rotated dead-store scratch (2 groups deep per engine)
    dve_scrs = [
        nc.alloc_sbuf_tensor(f"dve_scr{k}", [P, group * r], bf16).ap()
        for k in range(2 * dve_ops_per_group)
    ]
    act_scrs = [
        nc.alloc_sbuf_tensor(f"act_scr{k}", [P, group * r], bf16).ap()
        for k in range(2 * act_squares)
    ]

    dma_sems = [nc.alloc_semaphore(f"dma_sem{i}") for i in range(n_bufs)]
    out_sem = nc.alloc_semaphore("out_sem")
    dve_sem = nc.alloc_semaphore("dve_sem")
    act_sem = nc.alloc_semaphore("act_sem") if act_squares else None

    # DVE emission order (as baseline): subs run ahead; group g's reduces
    # emitted after sub(4g+4) so their wait is stale when reached.
    dve_order = []
    for t in range(n_tiles):
        dve_order.append(("sub", t))
        if t % group == 0 and t >= group:
            dve_order.append(("red", t // group - 1))
    dve_order.append(("red", n_groups - 1))
    sub_done, redg_done = {}, {}
    v = 0
    for kind, x in dve_order:
        if kind == "sub":
            v += 1
            sub_done[x] = v
        else:
            v += dve_ops_per_group
            redg_done[x] = v

    # output flushes: all-but-last groups early, last group alone at the end
    chunks = [(0, n_groups - 1), (n_groups - 1, n_groups)]

    import contextlib

    @contextlib.contextmanager
    def _block():
        with nc.Block(no_gpsimd_drain=True) as blk:
            try:
                yield blk
            finally:
                if skip_exit_barrier:
                    nc.all_engine_barrier = lambda **kw: None
        if skip_exit_barrier:
            del nc.all_engine_barrier

    def emit_dma(eng, t):
        eng.dma_start(
            out=buf(t).rearrange("p (w m) -> p w m", w=2),
            in_=pt_v[t],
        ).then_inc(dma_sems[t % n_bufs], 16)

    with _block() as block:

        @block.sync
        def _(sync):
            for t in range(n_scalar_dmas, n_tiles):
                if t >= n_bufs:
                    # ring reuse: only the evicted tile's sub must be done
                    sync.wait_ge(dve_sem, sub_done[t - n_bufs])
                emit_dma(sync, t)
            n_out = 0
            for lo, hi in chunks:
                if act_squares:
                    sync.wait_ge(act_sem, act_squares * hi)
                sync.wait_ge(dve_sem, redg_done[hi - 1])
                sync.dma_start(
                    out=out[:, 6 * lo : 6 * hi], in_=acc[:, 6 * lo : 6 * hi]
                ).then_inc(out_sem, 16)
                n_out += 16
            sync.wait_ge(out_sem, n_out)

        @block.scalar
        def _(scalar):
            # early tile loads on the second HWDGE ring (ramp)
            for t in range(n_scalar_dmas):
                emit_dma(scalar, t)
            for g in range(n_groups):
                scalar.wait_ge(dve_sem, sub_done[(g + 1) * group - 1])
                if g >= 2:
                    scalar.wait_ge(act_sem, act_squares * (g - 1))
                for i in range(act_squares):
                    scalar.activation(
                        out=act_scrs[(g % 2) * act_squares + i][:].rearrange(
                            "p (t r) -> p t r", t=group
                        ),
                        in_=dgroup(g, i),
                        func=mybir.ActivationFunctionType.Square,
                        accum_out=acc[:, g * 6 + i : g * 6 + i + 1],
                    ).then_inc(act_sem, 1)

        @block.vector
        def _(vector):
            for kind, x in dve_order:
                if kind == "sub":
                    p_v, t_v, d_v = sub_views(x)
                    vector.wait_ge(dma_sems[x % n_bufs], 16 * (x // n_bufs + 1))
                    if x >= n_dbufs and act_squares:
                        # d-slot reuse: ACT squares of the evicted tile's
                        # group must be done (DVE's own reads are ordered
                        # by program order)
                        gp = (x - n_dbufs) // group
                        vector.wait_ge(act_sem, act_squares * (gp + 1))
                    vector.tensor_tensor(
                        out=d_v,
                        in0=p_v,
                        in1=t_v,
                        op=mybir.AluOpType.subtract,
                    ).then_inc(dve_sem, 1)
                else:
                    vector.wait_ge(dve_sem, sub_done[(x + 1) * group - 1])
                    ops = [(i, j, 3 + k) for k, (i, j) in enumerate(_PAIRS)]
                    ops += [(i, i, i) for i in range(act_squares, 3)]
                    for n, (i, j, col) in enumerate(ops):
                        vector.scalar_tensor_tensor(
                            out=dve_scrs[(x % 2) * dve_ops_per_group + n][:]
                            .rearrange("p (t r) -> p t r", t=group),
                            in0=dgroup(x, i),
                            scalar=1.0,
                            in1=dgroup(x, j),
                            op0=mybir.AluOpType.mult,
                            op1=mybir.AluOpType.mult,
                            accum_out=acc[:, x * 6 + col : x * 6 + col + 1],
                        ).then_inc(dve_sem, 1)

    nc.compile()
    return nc


def build_gram_kernel_v3(n_rows: int, n_tiles: int = 32,
                         group_sizes: tuple = (4, 4, 4, 4, 4, 4, 4, 2, 1, 1),
                         skip_exit_barrier: bool = True):
    """Planar bf16 variant: host supplies component-planar tiles, the DMA
    casts fp32->bf16 in flight (SWDGE), and every on-chip operand is
    unit-stride bf16.

    Input pt [n_tiles, 128, 6r] f32, per (tile, partition) one contiguous
    chunk [p0 r | p1 r | p2 r | t0 r | t1 r | t2 r] (6 KB for r=256 — the
    descriptor sweet spot, ~413 GB/s vs 402 for the baseline's split
    chunks).  SWDGE (gpsimd-issued) DMA casts to bf16 on the fly — probe-
    measured at full read rate.  Per tile the DVE subtract then runs in
    2x packed mode (bf16, step 1): d = pred - targ IN-PLACE over the pred
    half.  Cross/square reduces read d unit-stride (no stride-3 penalty).

    Every tile has its own SBUF slot (32 x 3 KB bf16) and semaphore — no
    ring reuse, so the DMA stream never waits on compute.  Group sizes
    taper at the end so the after-last-DMA tail is only the final tile's
    sub + three N=256 reduces.

    Output: partials [128, 6 * n_groups]; col 6g+i = sum d_i^2, col
    6g+3+k = sum d_i d_j over group g's tiles.
    """
    assert sum(group_sizes) == n_tiles
    assert n_rows % (P * n_tiles) == 0
    n_groups = len(group_sizes)
    r = n_rows // (P * n_tiles)
    h = 3 * r  # bf16 elems per half-tile per partition
    f32, bf16 = mybir.dt.float32, mybir.dt.bfloat16
    max_g = max(group_sizes)

    ends = []
    e = -1
    for sz in group_sizes:
        e += sz
        ends.append(e)
    starts = [e - sz + 1 for e, sz in zip(ends, group_sizes)]

    nc = bacc.Bacc("TRN2", target_bir_lowering=False, debug=False)
    pt = nc.dram_tensor("pt", [n_tiles, P, 2 * h], f32, kind="ExternalInput")
    out = nc.dram_tensor("partials", [P, 6 * n_groups], f32, kind="ExternalOutput")
    pt_v = pt[:]

    ring = nc.alloc_sbuf_tensor("ring", [P, n_tiles * 2 * h], bf16).ap()
    acc = nc.alloc_sbuf_tensor("acc", [P, 6 * n_groups], f32).ap()

    def half(t, w):
        return ring[:, (2 * t + w) * h : (2 * t + w + 1) * h]

    def dgroup(g, i):
        # component i of group g's d (pred) halves: [128, size, r] step-1
        s = starts[g]
        w = ring[:, 2 * s * h : 2 * (s + group_sizes[g]) * h]
        return w.rearrange(
            "p (t w c r) -> p t w c r", t=group_sizes[g], w=2, c=3
        )[:, :, 0, i, :]

    dve_scrs = [
        nc.alloc_sbuf_tensor(f"dve_scr{k}", [P, max_g * r], bf16).ap()
        for k in range(6)
    ]
    act_scrs = [
        nc.alloc_sbuf_tensor(f"act_scr{k}", [P, max_g * r], bf16).ap()
        for k in range(6)
    ]

    dma_sems = [nc.alloc_semaphore(f"dma_sem{t}") for t in range(n_tiles)]
    out_sem = nc.alloc_semaphore("out_sem")
    dve_sem = nc.alloc_semaphore("dve_sem")
    act_sem = nc.alloc_semaphore("act_sem")

    # DVE order: subs run ahead, group reduces staggered one sub late.
    dve_order = []
    for t in range(n_tiles):
        dve_order.append(("sub", t))
        dve_order.extend(("red", g) for g in range(n_groups) if ends[g] == t - 1)
    dve_order.extend(("red", g) for g in range(n_groups) if ends[g] >= n_tiles - 1)
    sub_done, redg_done = {}, {}
    v = 0
    for kind, x in dve_order:
        if kind == "sub":
            v += 1
            sub_done[x] = v
        else:
            v += 3
            redg_done[x] = v

    chunks = [(0, n_groups - 1), (n_groups - 1, n_groups)]

    import contextlib

    @contextlib.contextmanager
    def _block():
        with nc.Block() as blk:
            try:
                yield blk
            finally:
                if skip_exit_barrier:
                    nc.all_engine_barrier = lambda **kw: None
        if skip_exit_barrier:
            del nc.all_engine_barrier

    with _block() as block:

        @block.gpsimd
        def _(gp):
            for t in range(n_tiles):
                gp.dma_start(
                    out=ring[:, 2 * t * h : 2 * (t + 1) * h],
                    in_=pt_v[t],
                ).then_inc(dma_sems[t], 16)

        @block.sync
        def _(sync):
            n_out = 0
            for lo, hi in chunks:
                sync.wait_ge(act_sem, 3 * hi)
                sync.wait_ge(dve_sem, redg_done[hi - 1])
                sync.dma_start(
                    out=out[:, 6 * lo : 6 * hi], in_=acc[:, 6 * lo : 6 * hi]
                ).then_inc(out_sem, 16)
                n_out += 16
            sync.wait_ge(out_sem, n_out)

        @block.scalar
        def _(scalar):
            for g in range(n_groups):
                scalar.wait_ge(dve_sem, sub_done[ends[g]])
                if g >= 2:
                    scalar.wait_ge(act_sem, 3 * (g - 1))
                for i in range(3):
                    scalar.activation(
                        out=act_scrs[(g % 2) * 3 + i][
                            :, : group_sizes[g] * r
                        ].rearrange("p (t r) -> p t r", t=group_sizes[g]),
                        in_=dgroup(g, i),
                        func=mybir.ActivationFunctionType.Square,
                        accum_out=acc[:, g * 6 + i : g * 6 + i + 1],
                    ).then_inc(act_sem, 1)

        @block.vector
        def _(vector):
            for kind, x in dve_order:
                if kind == "sub":
                    vector.wait_ge(dma_sems[x], 16)
                    vector.tensor_tensor(
                        out=half(x, 0),
                        in0=half(x, 0),
                        in1=half(x, 1),
                        op=mybir.AluOpType.subtract,
                    ).then_inc(dve_sem, 1)
                else:
                    vector.wait_ge(dve_sem, sub_done[ends[x]])
                    for k, (i, j) in enumerate(_PAIRS):
                        vector.scalar_tensor_tensor(
                            out=dve_scrs[(x % 2) * 3 + k][
                                :, : group_sizes[x] * r
                            ].rearrange("p (t r) -> p t r", t=group_sizes[x]),
                            in0=dgroup(x, i),
                            scalar=1.0,
                            in1=dgroup(x, j),
                            op0=mybir.AluOpType.mult,
                            op1=mybir.AluOpType.mult,
                            accum_out=acc[:, x * 6 + 3 + k : x * 6 + 4 + k],
                        ).then_inc(dve_sem, 1)

    nc.compile()
    return nc


def build_gram_kernel_v4(n_rows: int, n_tiles: int = 32, n_bufs: int = 16,
                         n_dbufs: int = 32,
                         sub_sizes: tuple = (4,) * 8,
                         cr_sizes: tuple = (4,) * 8,
                         sq_sizes: tuple = (4,) * 8,
                         n_scalar_dmas: int = 0, use_ttr: bool = False,
                         gp_batches: tuple = (),
                         skip_exit_barrier: bool = True):
    """Planar fp32 over HWDGE + bf16 d-ring; fused sub batches.

    Input pt [n_tiles, 128, 6r] f32 planar (one contiguous 6 KB chunk
    per tile/partition -> ~413 GB/s vs 402 for split chunks; HWDGE so no
    SWDGE descriptor-ring contention, which made SDMA engine 15 a 17%
    straggler that paced the whole SWDGE-cast variant).

    DVE subtracts in fused batches (one op across a batch's tiles, 3D AP
    [t, h] over the fp32 ring) writing unit-stride bf16 into the d-ring;
    cross/square reduces then run on step-1 bf16 (no stride-3 penalty).
    Crosses (DVE) use 8-tile groups, squares (ACT) 4-tile groups — the
    tilings are independent since the host sums all partial columns.
    All sizes taper to 1 tile at the end so the post-stream tail is just
    the last tile's sub + three N=r reduces.

    acc column layout (out [128, 3*(n_sq+n_cr)]): all-but-last sq groups,
    all-but-last cr groups, last sq group, last cr group — so each of the
    two output flushes is ONE contiguous dma and the final flush covers
    only the last-tile groups.
    """
    assert sum(sub_sizes) == n_tiles and sum(cr_sizes) == n_tiles
    assert sum(sq_sizes) == n_tiles
    assert n_rows % (P * n_tiles) == 0
    r = n_rows // (P * n_tiles)
    h = 3 * r
    f32, bf16 = mybir.dt.float32, mybir.dt.bfloat16

    def bounds(sizes):
        ends, e = [], -1
        for sz in sizes:
            e += sz
            ends.append(e)
        return [e - sz + 1 for e, sz in zip(ends, sizes)], ends

    sub_starts, sub_ends = bounds(sub_sizes)
    cr_starts, cr_ends = bounds(cr_sizes)
    sq_starts, sq_ends = bounds(sq_sizes)
    n_sq, n_cr = len(sq_sizes), len(cr_sizes)
    # fused subs and cross groups need their tiles contiguous in the rings
    for s, e in zip(sub_starts, sub_ends):
        assert (s % n_bufs) + (e - s) < n_bufs and (s % n_dbufs) + (e - s) < n_dbufs
    for s, e in zip(cr_starts, cr_ends):
        assert (s % n_dbufs) + (e - s) < n_dbufs
    for s, e in zip(sq_starts, sq_ends):
        assert (s % n_dbufs) + (e - s) < n_dbufs

    # acc columns ordered so the early flush [0, chunk_split) covers all
    # groups finishing by tile E1, and the final flush only the rest
    E1 = n_tiles - 5
    sq_early = [g for g in range(n_sq) if sq_ends[g] <= E1]
    sq_late = [g for g in range(n_sq) if sq_ends[g] > E1]
    cr_early = [g for g in range(n_cr) if cr_ends[g] <= E1]
    cr_late = [g for g in range(n_cr) if cr_ends[g] > E1]
    # groups complete in index order on each engine, so early must be a prefix
    assert sq_early == list(range(len(sq_early)))
    assert cr_early == list(range(len(cr_early)))
    sq_col, cr_col = {}, {}
    c = 0
    for g in sq_early:
        sq_col[g] = c; c += 3
    for g in cr_early:
        cr_col[g] = c; c += 3
    chunk_split = c
    for g in sq_late:
        sq_col[g] = c; c += 3
    for g in cr_late:
        cr_col[g] = c; c += 3
    n_cols = c

    nc = bacc.Bacc("TRN2", target_bir_lowering=False, debug=False)
    pt = nc.dram_tensor("pt", [n_tiles, P, 2 * h], f32, kind="ExternalInput")
    out = nc.dram_tensor("partials", [P, n_cols], f32, kind="ExternalOutput")
    pt_v = pt[:]

    ring = nc.alloc_sbuf_tensor("ring", [P, n_bufs * 2 * h], f32).ap()
    dring = nc.alloc_sbuf_tensor("dring", [P, n_dbufs * h], bf16).ap()
    acc = nc.alloc_sbuf_tensor("acc", [P, n_cols], f32).ap()
    max_cr = max(cr_sizes)
    max_sq = max(sq_sizes)
    dve_scrs = [
        nc.alloc_sbuf_tensor(f"dve_scr{k}", [P, max_cr * r], bf16).ap()
        for k in range(3)
    ]
    act_scrs = [
        nc.alloc_sbuf_tensor(f"act_scr{k}", [P, max_sq * r], bf16).ap()
        for k in range(3)
    ]

    dma_sems = [nc.alloc_semaphore(f"dma_sem{i}") for i in range(n_bufs)]
    out_sem = nc.alloc_semaphore("out_sem")
    dve_sem = nc.alloc_semaphore("dve_sem")
    act_sem = nc.alloc_semaphore("act_sem")
    gp_sem = nc.alloc_semaphore("gp_sem")

    def sub_views(b):
        s, sz = sub_starts[b], sub_sizes[b]
        rs = s % n_bufs
        w = ring[:, rs * 2 * h : (rs + sz) * 2 * h].rearrange(
            "p (t w h) -> p t w h", t=sz, w=2)
        ds = s % n_dbufs
        d = dring[:, ds * h : (ds + sz) * h].rearrange(
            "p (t h) -> p t h", t=sz)
        return w[:, :, 0, :], w[:, :, 1, :], d

    def dgroup(starts, sizes, g, i):
        s, sz = starts[g], sizes[g]
        ds = s % n_dbufs
        w = dring[:, ds * h : (ds + sz) * h]
        return w.rearrange("p (t c r) -> p t c r", t=sz, c=3)[:, :, i, :]

    # Sub batches listed in gp_batches run on the (otherwise idle) GpSimd
    # engine — DVE sub work shrinks below the DMA-arrival window so
    # schedule jitter can't cascade.  GpSimd elementwise is ~2.2 ns/elem,
    # under the 4-tile arrival period.
    n_batches = len(sub_sizes)
    gp_batches = tuple(sorted(gp_batches))
    assert all(0 <= b < n_batches for b in gp_batches)
    gp_done = {b: i + 1 for i, b in enumerate(gp_batches)}

    # DVE emission: cross group g emitted as soon as every DVE sub batch
    # overlapping it has been emitted (no stagger — a stagger delays
    # crosses a whole batch and starves the DVE when batches wait on DMA
    # arrivals; gp-run batches are covered by gp_sem waits instead)
    def overlapping_dve_batches(g):
        return [b for b in range(n_batches)
                if b not in gp_done
                and sub_ends[b] >= cr_starts[g] and sub_starts[b] <= cr_ends[g]]

    dve_order = []
    emitted_cr = set()
    for b in range(n_batches):
        if b in gp_done:
            continue
        dve_order.append(("sub", b))
        for g in range(n_cr):
            if g not in emitted_cr and all(
                bb <= b for bb in overlapping_dve_batches(g)
            ):
                emitted_cr.add(g)
                dve_order.append(("cr", g))
    assert len(emitted_cr) == n_cr
    sub_done, crg_done = {}, {}
    v = 0
    for kind, x in dve_order:
        v += 1 if kind == "sub" else 3
        (sub_done if kind == "sub" else crg_done)[x] = v

    def sub_targets(last_tile):
        """(dve_sem, gp_sem) targets ensuring every sub batch touching
        tiles <= last_tile has completed."""
        dve_t = gp_t = 0
        for b in range(n_batches):
            if sub_starts[b] <= last_tile:
                if b in gp_done:
                    gp_t = max(gp_t, gp_done[b])
                else:
                    dve_t = max(dve_t, sub_done[b])
        return dve_t, gp_t

    import contextlib

    @contextlib.contextmanager
    def _block():
        with nc.Block(no_gpsimd_drain=True) as blk:
            try:
                yield blk
            finally:
                if skip_exit_barrier:
                    nc.all_engine_barrier = lambda **kw: None
        if skip_exit_barrier:
            del nc.all_engine_barrier

    def emit_dma(eng, t):
        # split each partition's 6 KB chunk into two 3 KB descriptors:
        # >4 KB packets degrade SDMA round-robin under concurrent engine
        # load (SDMA 15 became a 20% straggler and paced every tile sem)
        eng.dma_start(
            out=ring[:, (t % n_bufs) * 2 * h : (t % n_bufs + 1) * 2 * h]
            .rearrange("p (x k) -> p x k", x=2),
            in_=pt_v[t].rearrange("p (x k) -> p x k", x=2),
        ).then_inc(dma_sems[t % n_bufs], 16)

    with _block() as block:

        if gp_batches:

            @block.gpsimd
            def _(gp):
                for b in gp_batches:
                    e = sub_ends[b]
                    gp.wait_ge(dma_sems[e % n_bufs], 16 * (e // n_bufs + 1))
                    p_v, t_v, d_v = sub_views(b)
                    gp.tensor_tensor(
                        out=d_v, in0=p_v, in1=t_v,
                        op=mybir.AluOpType.subtract,
                    ).then_inc(gp_sem, 1)

        @block.sync
        def _(sync):
            for t in range(n_scalar_dmas, n_tiles):
                if t >= n_bufs:
                    dve_t, gp_t = sub_targets(t - n_bufs)
                    if dve_t:
                        sync.wait_ge(dve_sem, dve_t)
                    if gp_t:
                        sync.wait_ge(gp_sem, gp_t)
                emit_dma(sync, t)
            n_out = 0
            for lo, hi, n_sq_done, n_cr_done in (
                (0, chunk_split, len(sq_early), len(cr_early)),
                (chunk_split, n_cols, n_sq, n_cr),
            ):
                sync.wait_ge(act_sem, 3 * n_sq_done)
                sync.wait_ge(dve_sem, crg_done[n_cr_done - 1])
                sync.dma_start(
                    out=out[:, lo:hi], in_=acc[:, lo:hi]
                ).then_inc(out_sem, 16)
                n_out += 16
            sync.wait_ge(out_sem, n_out)

        @block.scalar
        def _(scalar):
            for t in range(n_scalar_dmas):
                emit_dma(scalar, t)
            for g in range(n_sq):
                dve_t, gp_t = sub_targets(sq_ends[g])
                if dve_t:
                    scalar.wait_ge(dve_sem, dve_t)
                if gp_t:
                    scalar.wait_ge(gp_sem, gp_t)
                for i in range(3):
                    scalar.activation(
                        out=act_scrs[i][:, : sq_sizes[g] * r].rearrange(
                            "p (t r) -> p t r", t=sq_sizes[g]),
                        in_=dgroup(sq_starts, sq_sizes, g, i),
                        func=mybir.ActivationFunctionType.Square,
                        accum_out=acc[:, sq_col[g] + i : sq_col[g] + i + 1],
                    ).then_inc(act_sem, 1)

        @block.vector
        def _(vector):
            for kind, x in dve_order:
                if kind == "sub":
                    p_v, t_v, d_v = sub_views(x)
                    s, e = sub_starts[x], sub_ends[x]
                    vector.wait_ge(dma_sems[e % n_bufs], 16 * (e // n_bufs + 1))
                    if s >= n_dbufs:
                        # d-slot reuse: ACT squares over the evicted tiles
                        # must be done (DVE's own reads are program-ordered)
                        gp = next(g for g in range(n_sq)
                                  if sq_ends[g] >= e - n_dbufs)
                        vector.wait_ge(act_sem, 3 * (gp + 1))
                    vector.tensor_tensor(
                        out=d_v, in0=p_v, in1=t_v,
                        op=mybir.AluOpType.subtract,
                    ).then_inc(dve_sem, 1)
                else:
                    dve_t, gp_t = sub_targets(cr_ends[x])
                    if dve_t:
                        vector.wait_ge(dve_sem, dve_t)
                    if gp_t:
                        vector.wait_ge(gp_sem, gp_t)
                    for k, (i, j) in enumerate(_PAIRS):
                        scr = dve_scrs[k][:, : cr_sizes[x] * r].rearrange(
                            "p (t r) -> p t r", t=cr_sizes[x])
                        if use_ttr:
                            # fused multiply + sum-reduce, accumulator
                            # written directly by the instruction
                            vector.tensor_tensor_reduce(
                                out=scr,
                                in0=dgroup(cr_starts, cr_sizes, x, i),
                                in1=dgroup(cr_starts, cr_sizes, x, j),
                                scale=1.0,
                                scalar=0.0,
                                op0=mybir.AluOpType.mult,
                                op1=mybir.AluOpType.add,
                                accum_out=acc[:, cr_col[x] + k : cr_col[x] + k + 1],
                            ).then_inc(dve_sem, 1)
                        else:
                            vector.scalar_tensor_tensor(
                                out=scr,
                                in0=dgroup(cr_starts, cr_sizes, x, i),
                                scalar=1.0,
                                in1=dgroup(cr_starts, cr_sizes, x, j),
                                op0=mybir.AluOpType.mult,
                                op1=mybir.AluOpType.mult,
                                accum_out=acc[:, cr_col[x] + k : cr_col[x] + k + 1],
                            ).then_inc(dve_sem, 1)

    # record triple-row roles for the host-side unpack
    _V4_SQ_ROWS.clear()
    _V4_SQ_ROWS.extend(sq_col[g] // 3 for g in range(n_sq))
    _V4_CR_ROWS.clear()
    _V4_CR_ROWS.extend(cr_col[g] // 3 for g in range(n_cr))

    nc.compile()
    return nc


_V4_SQ_ROWS: list = []
_V4_CR_ROWS: list = []


def gram_from_partials_v4(partials: np.ndarray) -> np.ndarray:
    """v4 partials [..., 128, 3*(n_sq+n_cr)] -> 3x3 Gram (float64).

    Column triples ordered [sq_early, cr_early, sq_late, cr_late]; the
    builder records which triple rows are squares vs crosses in
    _V4_SQ_ROWS/_V4_CR_ROWS.
    """
    s = partials.astype(np.float64).reshape(-1, partials.shape[-1]).sum(axis=0)
    tri = s.reshape(-1, 3)
    sq = tri[_V4_SQ_ROWS].sum(axis=0)
    cr = tri[_V4_CR_ROWS].sum(axis=0)
    g = np.empty((3, 3), dtype=np.float64)
    g[0, 0], g[1, 1], g[2, 2] = sq
    for k, (i, j) in enumerate(_PAIRS):
        g[i, j] = g[j, i] = cr[k]
    return g


def planarize(predictions: np.ndarray, targets: np.ndarray,
              n_tiles: int = 32) -> np.ndarray:
    """[B,3] pred/targ -> per-core planar tiles [cores, n_tiles, P, 6r] f32."""
    b = predictions.shape[0]
    n_rows = b // N_CORES
    r = n_rows // (P * n_tiles)
    out = np.empty((N_CORES, n_tiles, P, 6 * r), dtype=np.float32)
    pv = out[..., : 3 * r].reshape(N_CORES, n_tiles, P, 3, r)
    tv = out[..., 3 * r :].reshape(N_CORES, n_tiles, P, 3, r)
    pv[:] = np.asarray(predictions, dtype=np.float32).reshape(
        N_CORES, n_tiles, P, r, 3).transpose(0, 1, 2, 4, 3)
    tv[:] = np.asarray(targets, dtype=np.float32).reshape(
        N_CORES, n_tiles, P, r, 3).transpose(0, 1, 2, 4, 3)
    return out


_NC_CACHE: dict[tuple, object] = {}


def _get_nc(n_rows: int, n_tiles: int, use_act: bool, raw: bool = False,
            group: int = 4, **kw):
    key = (n_rows, n_tiles, use_act, raw, group, tuple(sorted(kw.items())))
    if key not in _NC_CACHE:
        if raw:
            _NC_CACHE[key] = build_gram_kernel_v4(n_rows, n_tiles, **kw)
        else:
            _NC_CACHE[key] = build_gram_kernel(n_rows, n_tiles, use_act)
    return _NC_CACHE[key]


def gram_from_partials(partials: np.ndarray, n_tiles: int | None = None) -> np.ndarray:
    """Device partials -> full 3x3 Gram matrix (float64).

    Dispatches on column count: 51 -> v4 layout, else v2/v3 group-major
    layout (col 6g+i = sq_i, col 6g+3+k = cross pair k).
    """
    if partials.shape[-1] == 3 * (len(_V4_SQ_ROWS) + len(_V4_CR_ROWS)):
        return gram_from_partials_v4(partials)
    slots = partials.shape[-1] // 6
    s = partials.astype(np.float64).reshape(-1, slots, 6).sum(axis=0).sum(axis=0)
    g = np.empty((3, 3), dtype=np.float64)
    g[0, 0], g[1, 1], g[2, 2] = s[0:3]
    for k, (i, j) in enumerate(_PAIRS):
        g[i, j] = g[j, i] = s[3 + k]
    return g


def run_device_partials(predictions: np.ndarray, targets: np.ndarray,
                        n_tiles: int = 32, use_act: bool = True,
                        raw: bool = True, group: int = 4, **run_kwargs):
    """Shard over N_CORES, run on device, return per-core partials + results."""
    b = predictions.shape[0]
    assert b % N_CORES == 0
    n_rows = b // N_CORES
    nc = _get_nc(n_rows, n_tiles, use_act, raw, group)
    planar = planarize(predictions, targets, n_tiles)
    in_maps = [{"pt": planar[c]} for c in range(N_CORES)]
    res = run_bass_kernel_spmd(nc, in_maps, list(range(N_CORES)), **run_kwargs)
    partials = np.stack([r["partials"] for r in res.results])
    return partials, res


def kernel(predictions: np.ndarray, targets: np.ndarray, sigma: np.ndarray) -> np.ndarray:
    predictions = np.asarray(predictions, dtype=np.float32)
    targets = np.asarray(targets, dtype=np.float32)
    sigma64 = np.asarray(sigma, dtype=np.float64)

    partials, _ = run_device_partials(predictions, targets, n_tiles=32, raw=True)
    g = gram_from_partials(partials)

    sigma_inv = np.linalg.inv(sigma64)
    _, logdet = np.linalg.slogdet(sigma64)
    mean_mahal = float((sigma_inv * g).sum()) / predictions.shape[0]
    loss = abs(logdet + mean_mahal)
    return np.float32(loss)



# revision 29
# speedup vs baseline: 1.1135x; 1.0030x over previous
"""Trainium2 Bass kernel for CustomLossWithCovariance.

loss = abs(logdet(sigma) + mean_b[(p_b - t_b)^T sigma^{-1} (p_b - t_b)])

Only the 3x3 Gram matrix G = sum_b d_b d_b^T (d = pred - targ) requires
touching the [B, 3] data; the device computes per-core partial pair-sums
of G, and the host finishes with the tiny 3x3 algebra:
    mean_mahalanobis = <sigma_inv, G> / B
    loss = |logdet(sigma) + mean_mahalanobis|

Sharding: data-parallel over the batch across 8 NeuronCores (each core
streams a contiguous [B/8, 3] shard; partial sums gathered on host).

Per-core device kernel (raw Bacc, manual semaphores — see
build_gram_kernel_raw; build_gram_kernel is the Tile-framework
baseline kept for reference). Per tile of [128, 2M]:
  - DMA both halves (pred | targ) flat-contiguous in one dma_start
  - DVE: d = pred - targ, in-place into the pred half (unit-stride fp32)
  - ACT: Square(d_i) with accum_out -> per-partition sums of d_i^2
  - DVE: scalar_tensor_tensor(d_i * d_j, accum_out) -> cross sums
  (component APs are stride-3 views of the flat tiles, grouped 4 tiles
  per reduce instruction to amortize fixed costs)
"""

import numpy as np

import concourse.bass as bass
import concourse.bacc as bacc
import concourse.mybir as mybir
from concourse import tile
from concourse.bass_utils import run_bass_kernel_spmd

N_CORES = 8
B_FULL = 8388608
P = 128

_PAIRS = [(0, 1), (0, 2), (1, 2)]


def build_gram_kernel(n_rows: int, n_tiles: int, use_act: bool = True):
    """Build the per-core Bass module.

    Input: pt [2, n_rows, 3] f32 (pred stacked with targ)
    Output: partials [128, 6 * n_tiles] f32
        col t*3+i            : sum over this tile/partition of d_i^2
        col 3*n_tiles + t*3+k: sum of d_i*d_j for pair k in _PAIRS
    """
    assert n_rows % (P * n_tiles) == 0
    r = n_rows // (P * n_tiles)  # rows per partition per tile
    m = 3 * r                    # flat f32 elements per partition per tile
    f32 = mybir.dt.float32

    # Bacc (not plain Bass): its compile() pass legalizes semaphore waits
    # (each TRN2 instruction holds at most one wait slot).
    nc = bacc.Bacc("TRN2", target_bir_lowering=False, debug=False)
    pt = nc.dram_tensor("pt", [2, n_rows, 3], f32, kind="ExternalInput")
    out = nc.dram_tensor("partials", [P, 6 * n_tiles], f32, kind="ExternalOutput")

    # [t][p][w(2), m] — per tile/partition: pred chunk and targ chunk, each
    # m contiguous f32 in DRAM.
    pt_v = pt[:].rearrange("w (t p r) c -> t p w (r c)", t=n_tiles, p=P)

    with tile.TileContext(nc) as tc:
        with (
            tc.tile_pool(name="io", bufs=3) as io_pool,
            tc.tile_pool(name="dve_scr", bufs=2) as dve_scr,
            tc.tile_pool(name="act_scr", bufs=2) as act_scr,
            tc.tile_pool(name="acc", bufs=1) as acc_pool,
        ):
            acc_sq = acc_pool.tile([P, 3 * n_tiles], f32)
            acc_cr = acc_pool.tile([P, 3 * n_tiles], f32)

            for t in range(n_tiles):
                buf = io_pool.tile([P, 2 * m], f32, tag="buf")
                nc.sync.dma_start(
                    out=buf[:].rearrange("p (w m) -> p w m", w=2),
                    in_=pt_v[t],
                )

                # In-place: d = pred - targ, overwriting the pred half.
                nc.vector.tensor_tensor(
                    out=buf[:, 0:m],
                    in0=buf[:, 0:m],
                    in1=buf[:, m : 2 * m],
                    op=mybir.AluOpType.subtract,
                )
                d3 = buf[:, 0:m].rearrange("p (r c) -> p c r", c=3)

                # Diagonal sums on the scalar engine (Square + accum_out),
                # overlapping with the DVE cross-products.
                if use_act:
                    for i in range(3):
                        sq = act_scr.tile([P, r], f32, tag="sq")
                        nc.scalar.activation(
                            out=sq[:],
                            in_=d3[:, i, :],
                            func=mybir.ActivationFunctionType.Square,
                            accum_out=acc_sq[:, t * 3 + i : t * 3 + i + 1],
                        )
                else:
                    for i in range(3):
                        sq = dve_scr.tile([P, r], f32, tag="pr")
                        nc.vector.scalar_tensor_tensor(
                            out=sq[:],
                            in0=d3[:, i, :],
                            scalar=1.0,
                            in1=d3[:, i, :],
                            op0=mybir.AluOpType.mult,
                            op1=mybir.AluOpType.mult,
                            accum_out=acc_sq[:, t * 3 + i : t * 3 + i + 1],
                        )
                # Cross sums: fused multiply+reduce on DVE
                # (scalar_tensor_tensor: out = (in0 * 1.0) * in1, accum = sum).
                for k, (i, j) in enumerate(_PAIRS):
                    pr = dve_scr.tile([P, r], f32, tag="pr")
                    nc.vector.scalar_tensor_tensor(
                        out=pr[:],
                        in0=d3[:, i, :],
                        scalar=1.0,
                        in1=d3[:, j, :],
                        op0=mybir.AluOpType.mult,
                        op1=mybir.AluOpType.mult,
                        accum_out=acc_cr[:, t * 3 + k : t * 3 + k + 1],
                    )

            nc.sync.dma_start(out=out[:, 0 : 3 * n_tiles], in_=acc_sq[:])
            nc.sync.dma_start(out=out[:, 3 * n_tiles : 6 * n_tiles], in_=acc_cr[:])

    nc.compile()
    return nc


def build_gram_kernel_raw(n_rows: int, n_tiles: int = 32, n_bufs: int = 24,
                          group: int = 4, skip_exit_barrier: bool = True):
    """Raw-Bacc variant: manual semaphores, no TileContext.

    Skips Tile's prologue/epilogue (drain + two all-engine EVSEM
    barriers, ~16 us) — the only sync needed is a three-semaphore chain:
    DMA loads (one HWDGE ring) -> DVE -> ACT.

    The ring of tile buffers lives in ONE SBUF tensor so the fused
    multiply-reduces can span `group` consecutive tiles with a single
    instruction (free-dim AP [group, r]) — amortizing the per-op fixed
    cost and the accumulator-drain, which keeps both compute engines
    well under the DMA pace.

    Input: pt [2, n_rows, 3] f32. Output: partials [128, 6 * n_groups]
    (same slot layout as build_gram_kernel, with n_groups slots).
    """
    assert n_tiles % group == 0 and n_bufs % group == 0
    assert n_rows % (P * n_tiles) == 0
    n_groups = n_tiles // group
    r = n_rows // (P * n_tiles)
    m = 3 * r
    f32 = mybir.dt.float32

    nc = bacc.Bacc("TRN2", target_bir_lowering=False, debug=False)
    pt = nc.dram_tensor("pt", [2, n_rows, 3], f32, kind="ExternalInput")
    out = nc.dram_tensor("partials", [P, 6 * n_groups], f32, kind="ExternalOutput")
    pt_v = pt[:].rearrange("w (t p r) c -> t p w (r c)", t=n_tiles, p=P)

    ring = nc.alloc_sbuf_tensor("ring", [P, n_bufs * 2 * m], f32).ap()

    def buf(t):
        s = t % n_bufs
        return ring[:, s * 2 * m : (s + 1) * 2 * m]

    def dgroup(g, i):
        # component i of the diff halves of tiles 4g..4g+3: [128, group, r]
        s0 = (g * group) % n_bufs
        w = ring[:, s0 * 2 * m : (s0 + group) * 2 * m]
        return w.rearrange("p (t w r c) -> p t w c r", t=group, w=2, c=3)[:, :, 0, i, :]

    acc_sq = nc.alloc_sbuf_tensor("acc_sq", [P, 3 * n_groups], f32).ap()
    acc_cr = nc.alloc_sbuf_tensor("acc_cr", [P, 3 * n_groups], f32).ap()
    # Rotated scratch (dead stores of the fused ops), 2 groups deep so each
    # group's single stale semaphore wait also covers the scratch WAW from
    # two groups back.
    pr_scrs = [
        nc.alloc_sbuf_tensor(f"pr_scr{k}", [P, group * r], f32).ap() for k in range(6)
    ]
    sq_scrs = [
        nc.alloc_sbuf_tensor(f"sq_scr{k}", [P, group * r], f32).ap() for k in range(6)
    ]

    # One DMA-completion semaphore per ring buffer: a single shared sem
    # would be unsound — each dma_start is split across 16 SDMA engines
    # whose sub-completions interleave across in-flight DMAs.
    dma_sems = [nc.alloc_semaphore(f"dma_sem{i}") for i in range(n_bufs)]
    out_sem = nc.alloc_semaphore("out_sem")
    dve_sem = nc.alloc_semaphore("dve_sem")
    act_sem = nc.alloc_semaphore("act_sem")

    # DVE emission order: subs run ahead; the grouped multiply-reduces for
    # group g are emitted after sub(4g+4) so their drain-wait on the last
    # sub of the group is already satisfied when it executes (DVE writes
    # drain asynchronously). Only the last group trails the final sub.
    dve_order = []
    for t in range(n_tiles):
        dve_order.append(("sub", t))
        if t % group == 0 and t >= group:
            # one sub of stagger after the group's last sub
            dve_order.append(("stt", t // group - 1))
    dve_order.append(("stt", n_groups - 1))
    sub_done, sttg_done = {}, {}
    v = 0
    for kind, x in dve_order:
        if kind == "sub":
            v += 1
            sub_done[x] = v
        else:
            v += 3
            sttg_done[x] = v

    # Output chunks: flush finished accumulator columns while later tiles
    # still stream, so the tail only waits on the last small chunk.
    chunk = max(1, n_groups // 2)
    chunks = [(c, min(c + chunk, n_groups)) for c in range(0, n_groups, chunk)]

    import contextlib

    @contextlib.contextmanager
    def _block():
        # no_gpsimd_drain=True emits per-engine drains explicitly and then a
        # sem-only all-engine butterfly. The butterfly only delays NEFF end
        # (outputs are already fenced by the sequencer's out_sem wait), so
        # optionally no-op it during Block.__exit__.
        with nc.Block(no_gpsimd_drain=True) as blk:
            try:
                yield blk
            finally:
                if skip_exit_barrier:
                    nc.all_engine_barrier = lambda **kw: None
        if skip_exit_barrier:
            del nc.all_engine_barrier  # restore class method

    with _block() as block:

        @block.sync
        def _(sync):
            for t in range(n_tiles):
                if t >= n_bufs:
                    # ring reuse: all consumers of the buffer's previous
                    # occupant (tile t - n_bufs) must be done
                    prev = t - n_bufs
                    sync.wait_ge(dve_sem, sttg_done[prev // group])
                    sync.wait_ge(act_sem, 3 * (prev // group + 1))
                sync.dma_start(
                    out=buf(t).rearrange("p (w m) -> p w m", w=2),
                    in_=pt_v[t],
                ).then_inc(dma_sems[t % n_bufs], 16)
            n_out = 0
            for lo, hi in chunks:
                sync.wait_ge(act_sem, 3 * hi)
                sync.dma_start(
                    out=out[:, 3 * lo : 3 * hi], in_=acc_sq[:, 3 * lo : 3 * hi]
                ).then_inc(out_sem, 16)
                sync.wait_ge(dve_sem, sttg_done[hi - 1])
                sync.dma_start(
                    out=out[:, 3 * (n_groups + lo) : 3 * (n_groups + hi)],
                    in_=acc_cr[:, 3 * lo : 3 * hi],
                ).then_inc(out_sem, 16)
                n_out += 32
            sync.wait_ge(out_sem, n_out)

        @block.vector
        def _(vector):
            for kind, x in dve_order:
                if kind == "sub":
                    b = buf(x)
                    vector.wait_ge(dma_sems[x % n_bufs], 16 * (x // n_bufs + 1))
                    vector.tensor_tensor(
                        out=b[:, 0:m],
                        in0=b[:, 0:m],
                        in1=b[:, m : 2 * m],
                        op=mybir.AluOpType.subtract,
                    ).then_inc(dve_sem, 1)
                else:
                    vector.wait_ge(dve_sem, sub_done[(x + 1) * group - 1])
                    for k, (i, j) in enumerate(_PAIRS):
                        vector.scalar_tensor_tensor(
                            out=pr_scrs[(x % 2) * 3 + k][:].rearrange(
                                "p (t r) -> p t r", t=group
                            ),
                            in0=dgroup(x, i),
                            scalar=1.0,
                            in1=dgroup(x, j),
                            op0=mybir.AluOpType.mult,
                            op1=mybir.AluOpType.mult,
                            accum_out=acc_cr[:, x * 3 + k : x * 3 + k + 1],
                        ).then_inc(dve_sem, 1)

        @block.scalar
        def _(scalar):
            for g in range(n_groups):
                scalar.wait_ge(dve_sem, sub_done[(g + 1) * group - 1])
                if g >= 2:
                    # scratch slot reuse from two groups back
                    scalar.wait_ge(act_sem, 3 * (g - 1))
                for i in range(3):
                    scalar.activation(
                        out=sq_scrs[(g % 2) * 3 + i][:].rearrange(
                            "p (t r) -> p t r", t=group
                        ),
                        in_=dgroup(g, i),
                        func=mybir.ActivationFunctionType.Square,
                        accum_out=acc_sq[:, g * 3 + i : g * 3 + i + 1],
                    ).then_inc(act_sem, 1)

    nc.compile()
    return nc

def build_gram_kernel_v2(n_rows: int, n_tiles: int = 32, n_bufs: int = 24,
                         group: int = 4, n_dbufs: int = 12,
                         n_scalar_dmas: int = 8, act_squares: int = 3,
                         skip_exit_barrier: bool = True):
    """bf16-deinterleaved variant of build_gram_kernel_raw.

    The fp32 per-tile subtract writes d = pred - targ as bf16 with the
    three vector components DEINTERLEAVED (each component a unit-stride
    block) into a small d-ring.  The multiply-reduces then read bf16 at
    step 1, which unlocks the DVE 2x packed perf mode (fp32/stride-3 in
    the baseline capped DVE at ~85-103 elem/ns and made compute lag the
    402 GB/s DMA stream by ~8 us).

    Other deltas vs the baseline:
      - input-ring reuse only waits on the SUB of the evicted tile (the
        crosses read the d-ring, not the input ring), so the DMA queue
        never stalls on the reduce tail;
      - the first `n_scalar_dmas` tile loads are issued from the scalar
        engine's HWDGE ring in parallel with the sync engine's, halving
        the issue-rate-limited ramp;
      - the accumulator is laid out group-major ([sq0..2 cr0..2] per
        group) so each output flush is ONE dma, and the final flush
        covers only the last group's 6 columns.

    Output: partials [128, 6 * n_groups], col 6g+i = sum d_i^2 of group
    g for i<3, col 6g+3+k = sum d_i*d_j for pair k.
    """
    assert n_tiles % group == 0 and n_bufs % group == 0
    assert n_dbufs % group == 0 and n_dbufs >= 2 * group
    assert n_rows % (P * n_tiles) == 0
    assert 0 <= act_squares <= 3
    n_groups = n_tiles // group
    r = n_rows // (P * n_tiles)
    m = 3 * r
    f32, bf16 = mybir.dt.float32, mybir.dt.bfloat16

    nc = bacc.Bacc("TRN2", target_bir_lowering=False, debug=False)
    pt = nc.dram_tensor("pt", [2, n_rows, 3], f32, kind="ExternalInput")
    out = nc.dram_tensor("partials", [P, 6 * n_groups], f32, kind="ExternalOutput")
    pt_v = pt[:].rearrange("w (t p r) c -> t p w (r c)", t=n_tiles, p=P)

    ring = nc.alloc_sbuf_tensor("ring", [P, n_bufs * 2 * m], f32).ap()
    dring = nc.alloc_sbuf_tensor("dring", [P, n_dbufs * m], bf16).ap()
    acc = nc.alloc_sbuf_tensor("acc", [P, 6 * n_groups], f32).ap()

    def buf(t):
        s = t % n_bufs
        return ring[:, s * 2 * m : (s + 1) * 2 * m]

    def sub_views(t):
        b = buf(t)
        p_v = b[:, 0:m].rearrange("p (r c) -> p r c", c=3)
        t_v = b[:, m : 2 * m].rearrange("p (r c) -> p r c", c=3)
        s = t % n_dbufs
        d_v = dring[:, s * m : (s + 1) * m].rearrange("p (c r) -> p r c", c=3)
        return p_v, t_v, d_v

    def dgroup(g, i):
        # component i of groups' d tiles: [128, group, r], unit inner stride
        s0 = (g * group) % n_dbufs
        w = dring[:, s0 * m : (s0 + group) * m]
        return w.rearrange("p (t c r) -> p t c r", t=group, c=3)[:, :, i, :]

    n_dve_sq = 3 - act_squares
    dve_ops_per_group = 3 + n_dve_sq
    # rotated dead-store scratch (2 groups deep per engine)
    dve_scrs = [
        nc.alloc_sbuf_tensor(f"dve_scr{k}", [P, group * r], bf16).ap()
        for k in range(2 * dve_ops_per_group)
    ]
    act_scrs = [
        nc.alloc_sbuf_tensor(f"act_scr{k}", [P, group * r], bf16).ap()
        for k in range(2 * act_squares)
    ]

    dma_sems = [nc.alloc_semaphore(f"dma_sem{i}") for i in range(n_bufs)]
    out_sem = nc.alloc_semaphore("out_sem")
    dve_sem = nc.alloc_semaphore("dve_sem")
    act_sem = nc.alloc_semaphore("act_sem") if act_squares else None

    # DVE emission order (as baseline): subs run ahead; group g's reduces
    # emitted after sub(4g+4) so their wait is stale when reached.
    dve_order = []
    for t in range(n_tiles):
        dve_order.append(("sub", t))
        if t % group == 0 and t >= group:
            dve_order.append(("red", t // group - 1))
    dve_order.append(("red", n_groups - 1))
    sub_done, redg_done = {}, {}
    v = 0
    for kind, x in dve_order:
        if kind == "sub":
            v += 1
            sub_done[x] = v
        else:
            v += dve_ops_per_group
            redg_done[x] = v

    # output flushes: all-but-last groups early, last group alone at the end
    chunks = [(0, n_groups - 1), (n_groups - 1, n_groups)]

    import contextlib

    @contextlib.contextmanager
    def _block():
        with nc.Block(no_gpsimd_drain=True) as blk:
            try:
                yield blk
            finally:
                if skip_exit_barrier:
                    nc.all_engine_barrier = lambda **kw: None
        if skip_exit_barrier:
            del nc.all_engine_barrier

    def emit_dma(eng, t):
        eng.dma_start(
            out=buf(t).rearrange("p (w m) -> p w m", w=2),
            in_=pt_v[t],
        ).then_inc(dma_sems[t % n_bufs], 16)

    with _block() as block:

        @block.sync
        def _(sync):
            for t in range(n_scalar_dmas, n_tiles):
                if t >= n_bufs:
                    # ring reuse: only the evicted tile's sub must be done
                    sync.wait_ge(dve_sem, sub_done[t - n_bufs])
                emit_dma(sync, t)
            n_out = 0
            for lo, hi in chunks:
                if act_squares:
                    sync.wait_ge(act_sem, act_squares * hi)
                sync.wait_ge(dve_sem, redg_done[hi - 1])
                sync.dma_start(
                    out=out[:, 6 * lo : 6 * hi], in_=acc[:, 6 * lo : 6 * hi]
                ).then_inc(out_sem, 16)
                n_out += 16
            sync.wait_ge(out_sem, n_out)

        @block.scalar
        def _(scalar):
            # early tile loads on the second HWDGE ring (ramp)
            for t in range(n_scalar_dmas):
                emit_dma(scalar, t)
            for g in range(n_groups):
                scalar.wait_ge(dve_sem, sub_done[(g + 1) * group - 1])
                if g >= 2:
                    scalar.wait_ge(act_sem, act_squares * (g - 1))
                for i in range(act_squares):
                    scalar.activation(
                        out=act_scrs[(g % 2) * act_squares + i][:].rearrange(
                            "p (t r) -> p t r", t=group
                        ),
                        in_=dgroup(g, i),
                        func=mybir.ActivationFunctionType.Square,
                        accum_out=acc[:, g * 6 + i : g * 6 + i + 1],
                    ).then_inc(act_sem, 1)

        @block.vector
        def _(vector):
            for kind, x in dve_order:
                if kind == "sub":
                    p_v, t_v, d_v = sub_views(x)
                    vector.wait_ge(dma_sems[x % n_bufs], 16 * (x // n_bufs + 1))
                    if x >= n_dbufs and act_squares:
                        # d-slot reuse: ACT squares of the evicted tile's
                        # group must be done (DVE's own reads are ordered
                        # by program order)
                        gp = (x - n_dbufs) // group
                        vector.wait_ge(act_sem, act_squares * (gp + 1))
                    vector.tensor_tensor(
                        out=d_v,
                        in0=p_v,
                        in1=t_v,
                        op=mybir.AluOpType.subtract,
                    ).then_inc(dve_sem, 1)
                else:
                    vector.wait_ge(dve_sem, sub_done[(x + 1) * group - 1])
                    ops = [(i, j, 3 + k) for k, (i, j) in enumerate(_PAIRS)]
                    ops += [(i, i, i) for i in range(act_squares, 3)]
                    for n, (i, j, col) in enumerate(ops):
                        vector.scalar_tensor_tensor(
                            out=dve_scrs[(x % 2) * dve_ops_per_group + n][:]
                            .rearrange("p (t r) -> p t r", t=group),
                            in0=dgroup(x, i),
                            scalar=1.0,
                            in1=dgroup(x, j),
                            op0=mybir.AluOpType.mult,
                            op1=mybir.AluOpType.mult,
                            accum_out=acc[:, x * 6 + col : x * 6 + col + 1],
                        ).then_inc(dve_sem, 1)

    nc.compile()
    return nc


def build_gram_kernel_v3(n_rows: int, n_tiles: int = 32,
                         group_sizes: tuple = (4, 4, 4, 4, 4, 4, 4, 2, 1, 1),
                         skip_exit_barrier: bool = True):
    """Planar bf16 variant: host supplies component-planar tiles, the DMA
    casts fp32->bf16 in flight (SWDGE), and every on-chip operand is
    unit-stride bf16.

    Input pt [n_tiles, 128, 6r] f32, per (tile, partition) one contiguous
    chunk [p0 r | p1 r | p2 r | t0 r | t1 r | t2 r] (6 KB for r=256 — the
    descriptor sweet spot, ~413 GB/s vs 402 for the baseline's split
    chunks).  SWDGE (gpsimd-issued) DMA casts to bf16 on the fly — probe-
    measured at full read rate.  Per tile the DVE subtract then runs in
    2x packed mode (bf16, step 1): d = pred - targ IN-PLACE over the pred
    half.  Cross/square reduces read d unit-stride (no stride-3 penalty).

    Every tile has its own SBUF slot (32 x 3 KB bf16) and semaphore — no
    ring reuse, so the DMA stream never waits on compute.  Group sizes
    taper at the end so the after-last-DMA tail is only the final tile's
    sub + three N=256 reduces.

    Output: partials [128, 6 * n_groups]; col 6g+i = sum d_i^2, col
    6g+3+k = sum d_i d_j over group g's tiles.
    """
    assert sum(group_sizes) == n_tiles
    assert n_rows % (P * n_tiles) == 0
    n_groups = len(group_sizes)
    r = n_rows // (P * n_tiles)
    h = 3 * r  # bf16 elems per half-tile per partition
    f32, bf16 = mybir.dt.float32, mybir.dt.bfloat16
    max_g = max(group_sizes)

    ends = []
    e = -1
    for sz in group_sizes:
        e += sz
        ends.append(e)
    starts = [e - sz + 1 for e, sz in zip(ends, group_sizes)]

    nc = bacc.Bacc("TRN2", target_bir_lowering=False, debug=False)
    pt = nc.dram_tensor("pt", [n_tiles, P, 2 * h], f32, kind="ExternalInput")
    out = nc.dram_tensor("partials", [P, 6 * n_groups], f32, kind="ExternalOutput")
    pt_v = pt[:]

    ring = nc.alloc_sbuf_tensor("ring", [P, n_tiles * 2 * h], bf16).ap()
    acc = nc.alloc_sbuf_tensor("acc", [P, 6 * n_groups], f32).ap()

    def half(t, w):
        return ring[:, (2 * t + w) * h : (2 * t + w + 1) * h]

    def dgroup(g, i):
        # component i of group g's d (pred) halves: [128, size, r] step-1
        s = starts[g]
        w = ring[:, 2 * s * h : 2 * (s + group_sizes[g]) * h]
        return w.rearrange(
            "p (t w c r) -> p t w c r", t=group_sizes[g], w=2, c=3
        )[:, :, 0, i, :]

    dve_scrs = [
        nc.alloc_sbuf_tensor(f"dve_scr{k}", [P, max_g * r], bf16).ap()
        for k in range(6)
    ]
    act_scrs = [
        nc.alloc_sbuf_tensor(f"act_scr{k}", [P, max_g * r], bf16).ap()
        for k in range(6)
    ]

    dma_sems = [nc.alloc_semaphore(f"dma_sem{t}") for t in range(n_tiles)]
    out_sem = nc.alloc_semaphore("out_sem")
    dve_sem = nc.alloc_semaphore("dve_sem")
    act_sem = nc.alloc_semaphore("act_sem")

    # DVE order: subs run ahead, group reduces staggered one sub late.
    dve_order = []
    for t in range(n_tiles):
        dve_order.append(("sub", t))
        dve_order.extend(("red", g) for g in range(n_groups) if ends[g] == t - 1)
    dve_order.extend(("red", g) for g in range(n_groups) if ends[g] >= n_tiles - 1)
    sub_done, redg_done = {}, {}
    v = 0
    for kind, x in dve_order:
        if kind == "sub":
            v += 1
            sub_done[x] = v
        else:
            v += 3
            redg_done[x] = v

    chunks = [(0, n_groups - 1), (n_groups - 1, n_groups)]

    import contextlib

    @contextlib.contextmanager
    def _block():
        with nc.Block() as blk:
            try:
                yield blk
            finally:
                if skip_exit_barrier:
                    nc.all_engine_barrier = lambda **kw: None
        if skip_exit_barrier:
            del nc.all_engine_barrier

    with _block() as block:

        @block.gpsimd
        def _(gp):
            for t in range(n_tiles):
                gp.dma_start(
                    out=ring[:, 2 * t * h : 2 * (t + 1) * h],
                    in_=pt_v[t],
                ).then_inc(dma_sems[t], 16)

        @block.sync
        def _(sync):
            n_out = 0
            for lo, hi in chunks:
                sync.wait_ge(act_sem, 3 * hi)
                sync.wait_ge(dve_sem, redg_done[hi - 1])
                sync.dma_start(
                    out=out[:, 6 * lo : 6 * hi], in_=acc[:, 6 * lo : 6 * hi]
                ).then_inc(out_sem, 16)
                n_out += 16
            sync.wait_ge(out_sem, n_out)

        @block.scalar
        def _(scalar):
            for g in range(n_groups):
                scalar.wait_ge(dve_sem, sub_done[ends[g]])
                if g >= 2:
                    scalar.wait_ge(act_sem, 3 * (g - 1))
                for i in range(3):
                    scalar.activation(
                        out=act_scrs[(g % 2) * 3 + i][
                            :, : group_sizes[g] * r
                        ].rearrange("p (t r) -> p t r", t=group_sizes[g]),
                        in_=dgroup(g, i),
                        func=mybir.ActivationFunctionType.Square,
                        accum_out=acc[:, g * 6 + i : g * 6 + i + 1],
                    ).then_inc(act_sem, 1)

        @block.vector
        def _(vector):
            for kind, x in dve_order:
                if kind == "sub":
                    vector.wait_ge(dma_sems[x], 16)
                    vector.tensor_tensor(
                        out=half(x, 0),
                        in0=half(x, 0),
                        in1=half(x, 1),
                        op=mybir.AluOpType.subtract,
                    ).then_inc(dve_sem, 1)
                else:
                    vector.wait_ge(dve_sem, sub_done[ends[x]])
                    for k, (i, j) in enumerate(_PAIRS):
                        vector.scalar_tensor_tensor(
                            out=dve_scrs[(x % 2) * 3 + k][
                                :, : group_sizes[x] * r
                            ].rearrange("p (t r) -> p t r", t=group_sizes[x]),
                            in0=dgroup(x, i),
                            scalar=1.0,
                            in1=dgroup(x, j),
                            op0=mybir.AluOpType.mult,
                            op1=mybir.AluOpType.mult,
                            accum_out=acc[:, x * 6 + 3 + k : x * 6 + 4 + k],
                        ).then_inc(dve_sem, 1)

    nc.compile()
    return nc


def build_gram_kernel_v4(n_rows: int, n_tiles: int = 32, n_bufs: int = 16,
                         n_dbufs: int = 32,
                         sub_sizes: tuple = (1, 1, 1, 1) + (4,) * 6 + (2, 1, 1),
                         cr_sizes: tuple = (4,) * 8,
                         sq_sizes: tuple = (4,) * 8,
                         n_scalar_dmas: int = 0, use_ttr: bool = False,
                         gp_batches: tuple = (),
                         skip_exit_barrier: bool = True):
    """Planar fp32 over HWDGE + bf16 d-ring; fused sub batches.

    Input pt [n_tiles, 128, 6r] f32 planar (one contiguous 6 KB chunk
    per tile/partition -> ~413 GB/s vs 402 for split chunks; HWDGE so no
    SWDGE descriptor-ring contention, which made SDMA engine 15 a 17%
    straggler that paced the whole SWDGE-cast variant).

    DVE subtracts in fused batches (one op across a batch's tiles, 3D AP
    [t, h] over the fp32 ring) writing unit-stride bf16 into the d-ring;
    cross/square reduces then run on step-1 bf16 (no stride-3 penalty).
    Crosses (DVE) use 8-tile groups, squares (ACT) 4-tile groups — the
    tilings are independent since the host sums all partial columns.
    All sizes taper to 1 tile at the end so the post-stream tail is just
    the last tile's sub + three N=r reduces.

    acc column layout (out [128, 3*(n_sq+n_cr)]): all-but-last sq groups,
    all-but-last cr groups, last sq group, last cr group — so each of the
    two output flushes is ONE contiguous dma and the final flush covers
    only the last-tile groups.
    """
    assert sum(sub_sizes) == n_tiles and sum(cr_sizes) == n_tiles
    assert sum(sq_sizes) == n_tiles
    assert n_rows % (P * n_tiles) == 0
    r = n_rows // (P * n_tiles)
    h = 3 * r
    f32, bf16 = mybir.dt.float32, mybir.dt.bfloat16

    def bounds(sizes):
        ends, e = [], -1
        for sz in sizes:
            e += sz
            ends.append(e)
        return [e - sz + 1 for e, sz in zip(ends, sizes)], ends

    sub_starts, sub_ends = bounds(sub_sizes)
    cr_starts, cr_ends = bounds(cr_sizes)
    sq_starts, sq_ends = bounds(sq_sizes)
    n_sq, n_cr = len(sq_sizes), len(cr_sizes)
    # fused subs and cross groups need their tiles contiguous in the rings
    for s, e in zip(sub_starts, sub_ends):
        assert (s % n_bufs) + (e - s) < n_bufs and (s % n_dbufs) + (e - s) < n_dbufs
    for s, e in zip(cr_starts, cr_ends):
        assert (s % n_dbufs) + (e - s) < n_dbufs
    for s, e in zip(sq_starts, sq_ends):
        assert (s % n_dbufs) + (e - s) < n_dbufs

    # acc columns ordered so the early flush [0, chunk_split) covers all
    # groups finishing by tile E1, and the final flush only the rest
    E1 = n_tiles - 5
    sq_early = [g for g in range(n_sq) if sq_ends[g] <= E1]
    sq_late = [g for g in range(n_sq) if sq_ends[g] > E1]
    cr_early = [g for g in range(n_cr) if cr_ends[g] <= E1]
    cr_late = [g for g in range(n_cr) if cr_ends[g] > E1]
    # groups complete in index order on each engine, so early must be a prefix
    assert sq_early == list(range(len(sq_early)))
    assert cr_early == list(range(len(cr_early)))
    sq_col, cr_col = {}, {}
    c = 0
    for g in sq_early:
        sq_col[g] = c; c += 3
    for g in cr_early:
        cr_col[g] = c; c += 3
    chunk_split = c
    for g in sq_late:
        sq_col[g] = c; c += 3
    for g in cr_late:
        cr_col[g] = c; c += 3
    n_cols = c

    nc = bacc.Bacc("TRN2", target_bir_lowering=False, debug=False)
    pt = nc.dram_tensor("pt", [n_tiles, P, 2 * h], f32, kind="ExternalInput")
    out = nc.dram_tensor("partials", [P, n_cols], f32, kind="ExternalOutput")
    pt_v = pt[:]

    ring = nc.alloc_sbuf_tensor("ring", [P, n_bufs * 2 * h], f32).ap()
    dring = nc.alloc_sbuf_tensor("dring", [P, n_dbufs * h], bf16).ap()
    acc = nc.alloc_sbuf_tensor("acc", [P, n_cols], f32).ap()
    max_cr = max(cr_sizes)
    max_sq = max(sq_sizes)
    dve_scrs = [
        nc.alloc_sbuf_tensor(f"dve_scr{k}", [P, max_cr * r], bf16).ap()
        for k in range(3)
    ]
    act_scrs = [
        nc.alloc_sbuf_tensor(f"act_scr{k}", [P, max_sq * r], bf16).ap()
        for k in range(3)
    ]

    dma_sems = [nc.alloc_semaphore(f"dma_sem{i}") for i in range(n_bufs)]
    out_sem = nc.alloc_semaphore("out_sem")
    dve_sem = nc.alloc_semaphore("dve_sem")
    act_sem = nc.alloc_semaphore("act_sem")
    gp_sem = nc.alloc_semaphore("gp_sem")

    def sub_views(b):
        s, sz = sub_starts[b], sub_sizes[b]
        rs = s % n_bufs
        w = ring[:, rs * 2 * h : (rs + sz) * 2 * h].rearrange(
            "p (t w h) -> p t w h", t=sz, w=2)
        ds = s % n_dbufs
        d = dring[:, ds * h : (ds + sz) * h].rearrange(
            "p (t h) -> p t h", t=sz)
        return w[:, :, 0, :], w[:, :, 1, :], d

    def dgroup(starts, sizes, g, i):
        s, sz = starts[g], sizes[g]
        ds = s % n_dbufs
        w = dring[:, ds * h : (ds + sz) * h]
        return w.rearrange("p (t c r) -> p t c r", t=sz, c=3)[:, :, i, :]

    # Sub batches listed in gp_batches run on the (otherwise idle) GpSimd
    # engine — DVE sub work shrinks below the DMA-arrival window so
    # schedule jitter can't cascade.  GpSimd elementwise is ~2.2 ns/elem,
    # under the 4-tile arrival period.
    n_batches = len(sub_sizes)
    gp_batches = tuple(sorted(gp_batches))
    assert all(0 <= b < n_batches for b in gp_batches)
    gp_done = {b: i + 1 for i, b in enumerate(gp_batches)}

    # DVE emission: cross group g emitted as soon as every DVE sub batch
    # overlapping it has been emitted (no stagger — a stagger delays
    # crosses a whole batch and starves the DVE when batches wait on DMA
    # arrivals; gp-run batches are covered by gp_sem waits instead)
    def overlapping_dve_batches(g):
        return [b for b in range(n_batches)
                if b not in gp_done
                and sub_ends[b] >= cr_starts[g] and sub_starts[b] <= cr_ends[g]]

    dve_order = []
    emitted_cr = set()
    for b in range(n_batches):
        if b in gp_done:
            continue
        dve_order.append(("sub", b))
        for g in range(n_cr):
            if g not in emitted_cr and all(
                bb <= b for bb in overlapping_dve_batches(g)
            ):
                emitted_cr.add(g)
                dve_order.append(("cr", g))
    assert len(emitted_cr) == n_cr
    sub_done, crg_done = {}, {}
    v = 0
    for kind, x in dve_order:
        v += 1 if kind == "sub" else 3
        (sub_done if kind == "sub" else crg_done)[x] = v

    def sub_targets(last_tile):
        """(dve_sem, gp_sem) targets ensuring every sub batch touching
        tiles <= last_tile has completed."""
        dve_t = gp_t = 0
        for b in range(n_batches):
            if sub_starts[b] <= last_tile:
                if b in gp_done:
                    gp_t = max(gp_t, gp_done[b])
                else:
                    dve_t = max(dve_t, sub_done[b])
        return dve_t, gp_t

    import contextlib

    @contextlib.contextmanager
    def _block():
        with nc.Block(no_gpsimd_drain=True) as blk:
            try:
                yield blk
            finally:
                if skip_exit_barrier:
                    nc.all_engine_barrier = lambda **kw: None
        if skip_exit_barrier:
            del nc.all_engine_barrier

    def emit_dma(eng, t):
        # split each partition's 6 KB chunk into two 3 KB descriptors:
        # >4 KB packets degrade SDMA round-robin under concurrent engine
        # load (SDMA 15 became a 20% straggler and paced every tile sem)
        eng.dma_start(
            out=ring[:, (t % n_bufs) * 2 * h : (t % n_bufs + 1) * 2 * h]
            .rearrange("p (x k) -> p x k", x=2),
            in_=pt_v[t].rearrange("p (x k) -> p x k", x=2),
        ).then_inc(dma_sems[t % n_bufs], 16)

    with _block() as block:

        if gp_batches:

            @block.gpsimd
            def _(gp):
                for b in gp_batches:
                    e = sub_ends[b]
                    gp.wait_ge(dma_sems[e % n_bufs], 16 * (e // n_bufs + 1))
                    p_v, t_v, d_v = sub_views(b)
                    gp.tensor_tensor(
                        out=d_v, in0=p_v, in1=t_v,
                        op=mybir.AluOpType.subtract,
                    ).then_inc(gp_sem, 1)

        @block.sync
        def _(sync):
            for t in range(n_scalar_dmas, n_tiles):
                if t >= n_bufs:
                    dve_t, gp_t = sub_targets(t - n_bufs)
                    if dve_t:
                        sync.wait_ge(dve_sem, dve_t)
                    if gp_t:
                        sync.wait_ge(gp_sem, gp_t)
                emit_dma(sync, t)
            n_out = 0
            for lo, hi, n_sq_done, n_cr_done in (
                (0, chunk_split, len(sq_early), len(cr_early)),
                (chunk_split, n_cols, n_sq, n_cr),
            ):
                sync.wait_ge(act_sem, 3 * n_sq_done)
                sync.wait_ge(dve_sem, crg_done[n_cr_done - 1])
                sync.dma_start(
                    out=out[:, lo:hi], in_=acc[:, lo:hi]
                ).then_inc(out_sem, 16)
                n_out += 16
            sync.wait_ge(out_sem, n_out)

        @block.scalar
        def _(scalar):
            for t in range(n_scalar_dmas):
                emit_dma(scalar, t)
            for g in range(n_sq):
                dve_t, gp_t = sub_targets(sq_ends[g])
                if dve_t:
                    scalar.wait_ge(dve_sem, dve_t)
                if gp_t:
                    scalar.wait_ge(gp_sem, gp_t)
                for i in range(3):
                    scalar.activation(
                        out=act_scrs[i][:, : sq_sizes[g] * r].rearrange(
                            "p (t r) -> p t r", t=sq_sizes[g]),
                        in_=dgroup(sq_starts, sq_sizes, g, i),
                        func=mybir.ActivationFunctionType.Square,
                        accum_out=acc[:, sq_col[g] + i : sq_col[g] + i + 1],
                    ).then_inc(act_sem, 1)

        @block.vector
        def _(vector):
            for kind, x in dve_order:
                if kind == "sub":
                    p_v, t_v, d_v = sub_views(x)
                    s, e = sub_starts[x], sub_ends[x]
                    vector.wait_ge(dma_sems[e % n_bufs], 16 * (e // n_bufs + 1))
                    if s >= n_dbufs:
                        # d-slot reuse: ACT squares over the evicted tiles
                        # must be done (DVE's own reads are program-ordered)
                        gp = next(g for g in range(n_sq)
                                  if sq_ends[g] >= e - n_dbufs)
                        vector.wait_ge(act_sem, 3 * (gp + 1))
                    vector.tensor_tensor(
                        out=d_v, in0=p_v, in1=t_v,
                        op=mybir.AluOpType.subtract,
                    ).then_inc(dve_sem, 1)
                else:
                    dve_t, gp_t = sub_targets(cr_ends[x])
                    if dve_t:
                        vector.wait_ge(dve_sem, dve_t)
                    if gp_t:
                        vector.wait_ge(gp_sem, gp_t)
                    for k, (i, j) in enumerate(_PAIRS):
                        scr = dve_scrs[k][:, : cr_sizes[x] * r].rearrange(
                            "p (t r) -> p t r", t=cr_sizes[x])
                        if use_ttr:
                            # fused multiply + sum-reduce, accumulator
                            # written directly by the instruction
                            vector.tensor_tensor_reduce(
                                out=scr,
                                in0=dgroup(cr_starts, cr_sizes, x, i),
                                in1=dgroup(cr_starts, cr_sizes, x, j),
                                scale=1.0,
                                scalar=0.0,
                                op0=mybir.AluOpType.mult,
                                op1=mybir.AluOpType.add,
                                accum_out=acc[:, cr_col[x] + k : cr_col[x] + k + 1],
                            ).then_inc(dve_sem, 1)
                        else:
                            vector.scalar_tensor_tensor(
                                out=scr,
                                in0=dgroup(cr_starts, cr_sizes, x, i),
                                scalar=1.0,
                                in1=dgroup(cr_starts, cr_sizes, x, j),
                                op0=mybir.AluOpType.mult,
                                op1=mybir.AluOpType.mult,
                                accum_out=acc[:, cr_col[x] + k : cr_col[x] + k + 1],
                            ).then_inc(dve_sem, 1)

    # record triple-row roles for the host-side unpack
    _V4_SQ_ROWS.clear()
    _V4_SQ_ROWS.extend(sq_col[g] // 3 for g in range(n_sq))
    _V4_CR_ROWS.clear()
    _V4_CR_ROWS.extend(cr_col[g] // 3 for g in range(n_cr))

    nc.compile()
    return nc


_V4_SQ_ROWS: list = []
_V4_CR_ROWS: list = []


def gram_from_partials_v4(partials: np.ndarray) -> np.ndarray:
    """v4 partials [..., 128, 3*(n_sq+n_cr)] -> 3x3 Gram (float64).

    Column triples ordered [sq_early, cr_early, sq_late, cr_late]; the
    builder records which triple rows are squares vs crosses in
    _V4_SQ_ROWS/_V4_CR_ROWS.
    """
    s = partials.astype(np.float64).reshape(-1, partials.shape[-1]).sum(axis=0)
    tri = s.reshape(-1, 3)
    sq = tri[_V4_SQ_ROWS].sum(axis=0)
    cr = tri[_V4_CR_ROWS].sum(axis=0)
    g = np.empty((3, 3), dtype=np.float64)
    g[0, 0], g[1, 1], g[2, 2] = sq
    for k, (i, j) in enumerate(_PAIRS):
        g[i, j] = g[j, i] = cr[k]
    return g


def planarize(predictions: np.ndarray, targets: np.ndarray,
              n_tiles: int = 32) -> np.ndarray:
    """[B,3] pred/targ -> per-core planar tiles [cores, n_tiles, P, 6r] f32."""
    b = predictions.shape[0]
    n_rows = b // N_CORES
    r = n_rows // (P * n_tiles)
    out = np.empty((N_CORES, n_tiles, P, 6 * r), dtype=np.float32)
    pv = out[..., : 3 * r].reshape(N_CORES, n_tiles, P, 3, r)
    tv = out[..., 3 * r :].reshape(N_CORES, n_tiles, P, 3, r)
    pv[:] = np.asarray(predictions, dtype=np.float32).reshape(
        N_CORES, n_tiles, P, r, 3).transpose(0, 1, 2, 4, 3)
    tv[:] = np.asarray(targets, dtype=np.float32).reshape(
        N_CORES, n_tiles, P, r, 3).transpose(0, 1, 2, 4, 3)
    return out


_NC_CACHE: dict[tuple, object] = {}


def _get_nc(n_rows: int, n_tiles: int, use_act: bool, raw: bool = False,
            group: int = 4, **kw):
    key = (n_rows, n_tiles, use_act, raw, group, tuple(sorted(kw.items())))
    if key not in _NC_CACHE:
        if raw:
            _NC_CACHE[key] = build_gram_kernel_v4(n_rows, n_tiles, **kw)
        else:
            _NC_CACHE[key] = build_gram_kernel(n_rows, n_tiles, use_act)
    return _NC_CACHE[key]


def gram_from_partials(partials: np.ndarray, n_tiles: int | None = None) -> np.ndarray:
    """Device partials -> full 3x3 Gram matrix (float64).

    Dispatches on column count: 51 -> v4 layout, else v2/v3 group-major
    layout (col 6g+i = sq_i, col 6g+3+k = cross pair k).
    """
    if partials.shape[-1] == 3 * (len(_V4_SQ_ROWS) + len(_V4_CR_ROWS)):
        return gram_from_partials_v4(partials)
    slots = partials.shape[-1] // 6
    s = partials.astype(np.float64).reshape(-1, slots, 6).sum(axis=0).sum(axis=0)
    g = np.empty((3, 3), dtype=np.float64)
    g[0, 0], g[1, 1], g[2, 2] = s[0:3]
    for k, (i, j) in enumerate(_PAIRS):
        g[i, j] = g[j, i] = s[3 + k]
    return g


def run_device_partials(predictions: np.ndarray, targets: np.ndarray,
                        n_tiles: int = 32, use_act: bool = True,
                        raw: bool = True, group: int = 4, **run_kwargs):
    """Shard over N_CORES, run on device, return per-core partials + results."""
    b = predictions.shape[0]
    assert b % N_CORES == 0
    n_rows = b // N_CORES
    nc = _get_nc(n_rows, n_tiles, use_act, raw, group)
    planar = planarize(predictions, targets, n_tiles)
    in_maps = [{"pt": planar[c]} for c in range(N_CORES)]
    res = run_bass_kernel_spmd(nc, in_maps, list(range(N_CORES)), **run_kwargs)
    partials = np.stack([r["partials"] for r in res.results])
    return partials, res


def kernel(predictions: np.ndarray, targets: np.ndarray, sigma: np.ndarray) -> np.ndarray:
    predictions = np.asarray(predictions, dtype=np.float32)
    targets = np.asarray(targets, dtype=np.float32)
    sigma64 = np.asarray(sigma, dtype=np.float64)

    partials, _ = run_device_partials(predictions, targets, n_tiles=32, raw=True)
    g = gram_from_partials(partials)

    sigma_inv = np.linalg.inv(sigma64)
    _, logdet = np.linalg.slogdet(sigma64)
    mean_mahal = float((sigma_inv * g).sum()) / predictions.shape[0]
    loss = abs(logdet + mean_mahal)
    return np.float32(loss)



# revision 30
# speedup vs baseline: 1.1167x; 1.0028x over previous
"""Trainium2 Bass kernel for CustomLossWithCovariance.

loss = abs(logdet(sigma) + mean_b[(p_b - t_b)^T sigma^{-1} (p_b - t_b)])

Only the 3x3 Gram matrix G = sum_b d_b d_b^T (d = pred - targ) requires
touching the [B, 3] data; the device computes per-core partial pair-sums
of G, and the host finishes with the tiny 3x3 algebra:
    mean_mahalanobis = <sigma_inv, G> / B
    loss = |logdet(sigma) + mean_mahalanobis|

Sharding: data-parallel over the batch across 8 NeuronCores (each core
streams a contiguous [B/8, 3] shard; partial sums gathered on host).

Active per-core kernel: build_gram_kernel_v4 (raw Bacc, manual
semaphores).  The host pre-lays each tile component-PLANAR — per
(tile, partition) one contiguous fp32 chunk [p0 r|p1 r|p2 r|t0 r|..] —
so every on-chip operand is unit-stride and each partition chunk maps
to two 3 KB HWDGE descriptors (~415 GB/s streaming; >4 KB descriptors
or SWDGE made SDMA engine 15 a straggler that paced every tile).
Per batch of tiles:
  - DVE: fused d = pred - targ -> bf16 d-ring (every tile has its own
    SBUF slot and d slot; no ring-reuse waits anywhere)
  - ACT: Square(d_i) with accum_out over 4-tile groups (bf16 step-1)
  - DVE: scalar_tensor_tensor(d_i * d_j) accum over 4-tile groups
Batch/group sizes taper at the edges so DVE starts at first-tile
arrival and only a 1-tile sub + one reduce group trail the last DMA.
Earlier variants kept for reference: build_gram_kernel (Tile
framework), build_gram_kernel_raw (fp32 stride-3, the 84 us baseline),
v2/v3 (scatter-write and SWDGE-cast experiments).
"""

import numpy as np

import concourse.bass as bass
import concourse.bacc as bacc
import concourse.mybir as mybir
from concourse import tile
from concourse.bass_utils import run_bass_kernel_spmd

N_CORES = 8
B_FULL = 8388608
P = 128

_PAIRS = [(0, 1), (0, 2), (1, 2)]


def build_gram_kernel(n_rows: int, n_tiles: int, use_act: bool = True):
    """Build the per-core Bass module.

    Input: pt [2, n_rows, 3] f32 (pred stacked with targ)
    Output: partials [128, 6 * n_tiles] f32
        col t*3+i            : sum over this tile/partition of d_i^2
        col 3*n_tiles + t*3+k: sum of d_i*d_j for pair k in _PAIRS
    """
    assert n_rows % (P * n_tiles) == 0
    r = n_rows // (P * n_tiles)  # rows per partition per tile
    m = 3 * r                    # flat f32 elements per partition per tile
    f32 = mybir.dt.float32

    # Bacc (not plain Bass): its compile() pass legalizes semaphore waits
    # (each TRN2 instruction holds at most one wait slot).
    nc = bacc.Bacc("TRN2", target_bir_lowering=False, debug=False)
    pt = nc.dram_tensor("pt", [2, n_rows, 3], f32, kind="ExternalInput")
    out = nc.dram_tensor("partials", [P, 6 * n_tiles], f32, kind="ExternalOutput")

    # [t][p][w(2), m] — per tile/partition: pred chunk and targ chunk, each
    # m contiguous f32 in DRAM.
    pt_v = pt[:].rearrange("w (t p r) c -> t p w (r c)", t=n_tiles, p=P)

    with tile.TileContext(nc) as tc:
        with (
            tc.tile_pool(name="io", bufs=3) as io_pool,
            tc.tile_pool(name="dve_scr", bufs=2) as dve_scr,
            tc.tile_pool(name="act_scr", bufs=2) as act_scr,
            tc.tile_pool(name="acc", bufs=1) as acc_pool,
        ):
            acc_sq = acc_pool.tile([P, 3 * n_tiles], f32)
            acc_cr = acc_pool.tile([P, 3 * n_tiles], f32)

            for t in range(n_tiles):
                buf = io_pool.tile([P, 2 * m], f32, tag="buf")
                nc.sync.dma_start(
                    out=buf[:].rearrange("p (w m) -> p w m", w=2),
                    in_=pt_v[t],
                )

                # In-place: d = pred - targ, overwriting the pred half.
                nc.vector.tensor_tensor(
                    out=buf[:, 0:m],
                    in0=buf[:, 0:m],
                    in1=buf[:, m : 2 * m],
                    op=mybir.AluOpType.subtract,
                )
                d3 = buf[:, 0:m].rearrange("p (r c) -> p c r", c=3)

                # Diagonal sums on the scalar engine (Square + accum_out),
                # overlapping with the DVE cross-products.
                if use_act:
                    for i in range(3):
                        sq = act_scr.tile([P, r], f32, tag="sq")
                        nc.scalar.activation(
                            out=sq[:],
                            in_=d3[:, i, :],
                            func=mybir.ActivationFunctionType.Square,
                            accum_out=acc_sq[:, t * 3 + i : t * 3 + i + 1],
                        )
                else:
                    for i in range(3):
                        sq = dve_scr.tile([P, r], f32, tag="pr")
                        nc.vector.scalar_tensor_tensor(
                            out=sq[:],
                            in0=d3[:, i, :],
                            scalar=1.0,
                            in1=d3[:, i, :],
                            op0=mybir.AluOpType.mult,
                            op1=mybir.AluOpType.mult,
                            accum_out=acc_sq[:, t * 3 + i : t * 3 + i + 1],
                        )
                # Cross sums: fused multiply+reduce on DVE
                # (scalar_tensor_tensor: out = (in0 * 1.0) * in1, accum = sum).
                for k, (i, j) in enumerate(_PAIRS):
                    pr = dve_scr.tile([P, r], f32, tag="pr")
                    nc.vector.scalar_tensor_tensor(
                        out=pr[:],
                        in0=d3[:, i, :],
                        scalar=1.0,
                        in1=d3[:, j, :],
                        op0=mybir.AluOpType.mult,
                        op1=mybir.AluOpType.mult,
                        accum_out=acc_cr[:, t * 3 + k : t * 3 + k + 1],
                    )

            nc.sync.dma_start(out=out[:, 0 : 3 * n_tiles], in_=acc_sq[:])
            nc.sync.dma_start(out=out[:, 3 * n_tiles : 6 * n_tiles], in_=acc_cr[:])

    nc.compile()
    return nc


def build_gram_kernel_raw(n_rows: int, n_tiles: int = 32, n_bufs: int = 24,
                          group: int = 4, skip_exit_barrier: bool = True):
    """Raw-Bacc variant: manual semaphores, no TileContext.

    Skips Tile's prologue/epilogue (drain + two all-engine EVSEM
    barriers, ~16 us) — the only sync needed is a three-semaphore chain:
    DMA loads (one HWDGE ring) -> DVE -> ACT.

    The ring of tile buffers lives in ONE SBUF tensor so the fused
    multiply-reduces can span `group` consecutive tiles with a single
    instruction (free-dim AP [group, r]) — amortizing the per-op fixed
    cost and the accumulator-drain, which keeps both compute engines
    well under the DMA pace.

    Input: pt [2, n_rows, 3] f32. Output: partials [128, 6 * n_groups]
    (same slot layout as build_gram_kernel, with n_groups slots).
    """
    assert n_tiles % group == 0 and n_bufs % group == 0
    assert n_rows % (P * n_tiles) == 0
    n_groups = n_tiles // group
    r = n_rows // (P * n_tiles)
    m = 3 * r
    f32 = mybir.dt.float32

    nc = bacc.Bacc("TRN2", target_bir_lowering=False, debug=False)
    pt = nc.dram_tensor("pt", [2, n_rows, 3], f32, kind="ExternalInput")
    out = nc.dram_tensor("partials", [P, 6 * n_groups], f32, kind="ExternalOutput")
    pt_v = pt[:].rearrange("w (t p r) c -> t p w (r c)", t=n_tiles, p=P)

    ring = nc.alloc_sbuf_tensor("ring", [P, n_bufs * 2 * m], f32).ap()

    def buf(t):
        s = t % n_bufs
        return ring[:, s * 2 * m : (s + 1) * 2 * m]

    def dgroup(g, i):
        # component i of the diff halves of tiles 4g..4g+3: [128, group, r]
        s0 = (g * group) % n_bufs
        w = ring[:, s0 * 2 * m : (s0 + group) * 2 * m]
        return w.rearrange("p (t w r c) -> p t w c r", t=group, w=2, c=3)[:, :, 0, i, :]

    acc_sq = nc.alloc_sbuf_tensor("acc_sq", [P, 3 * n_groups], f32).ap()
    acc_cr = nc.alloc_sbuf_tensor("acc_cr", [P, 3 * n_groups], f32).ap()
    # Rotated scratch (dead stores of the fused ops), 2 groups deep so each
    # group's single stale semaphore wait also covers the scratch WAW from
    # two groups back.
    pr_scrs = [
        nc.alloc_sbuf_tensor(f"pr_scr{k}", [P, group * r], f32).ap() for k in range(6)
    ]
    sq_scrs = [
        nc.alloc_sbuf_tensor(f"sq_scr{k}", [P, group * r], f32).ap() for k in range(6)
    ]

    # One DMA-completion semaphore per ring buffer: a single shared sem
    # would be unsound — each dma_start is split across 16 SDMA engines
    # whose sub-completions interleave across in-flight DMAs.
    dma_sems = [nc.alloc_semaphore(f"dma_sem{i}") for i in range(n_bufs)]
    out_sem = nc.alloc_semaphore("out_sem")
    dve_sem = nc.alloc_semaphore("dve_sem")
    act_sem = nc.alloc_semaphore("act_sem")

    # DVE emission order: subs run ahead; the grouped multiply-reduces for
    # group g are emitted after sub(4g+4) so their drain-wait on the last
    # sub of the group is already satisfied when it executes (DVE writes
    # drain asynchronously). Only the last group trails the final sub.
    dve_order = []
    for t in range(n_tiles):
        dve_order.append(("sub", t))
        if t % group == 0 and t >= group:
            # one sub of stagger after the group's last sub
            dve_order.append(("stt", t // group - 1))
    dve_order.append(("stt", n_groups - 1))
    sub_done, sttg_done = {}, {}
    v = 0
    for kind, x in dve_order:
        if kind == "sub":
            v += 1
            sub_done[x] = v
        else:
            v += 3
            sttg_done[x] = v

    # Output chunks: flush finished accumulator columns while later tiles
    # still stream, so the tail only waits on the last small chunk.
    chunk = max(1, n_groups // 2)
    chunks = [(c, min(c + chunk, n_groups)) for c in range(0, n_groups, chunk)]

    import contextlib

    @contextlib.contextmanager
    def _block():
        # no_gpsimd_drain=True emits per-engine drains explicitly and then a
        # sem-only all-engine butterfly. The butterfly only delays NEFF end
        # (outputs are already fenced by the sequencer's out_sem wait), so
        # optionally no-op it during Block.__exit__.
        with nc.Block(no_gpsimd_drain=True) as blk:
            try:
                yield blk
            finally:
                if skip_exit_barrier:
                    nc.all_engine_barrier = lambda **kw: None
        if skip_exit_barrier:
            del nc.all_engine_barrier  # restore class method

    with _block() as block:

        @block.sync
        def _(sync):
            for t in range(n_tiles):
                if t >= n_bufs:
                    # ring reuse: all consumers of the buffer's previous
                    # occupant (tile t - n_bufs) must be done
                    prev = t - n_bufs
                    sync.wait_ge(dve_sem, sttg_done[prev // group])
                    sync.wait_ge(act_sem, 3 * (prev // group + 1))
                sync.dma_start(
                    out=buf(t).rearrange("p (w m) -> p w m", w=2),
                    in_=pt_v[t],
                ).then_inc(dma_sems[t % n_bufs], 16)
            n_out = 0
            for lo, hi in chunks:
                sync.wait_ge(act_sem, 3 * hi)
                sync.dma_start(
                    out=out[:, 3 * lo : 3 * hi], in_=acc_sq[:, 3 * lo : 3 * hi]
                ).then_inc(out_sem, 16)
                sync.wait_ge(dve_sem, sttg_done[hi - 1])
                sync.dma_start(
                    out=out[:, 3 * (n_groups + lo) : 3 * (n_groups + hi)],
                    in_=acc_cr[:, 3 * lo : 3 * hi],
                ).then_inc(out_sem, 16)
                n_out += 32
            sync.wait_ge(out_sem, n_out)

        @block.vector
        def _(vector):
            for kind, x in dve_order:
                if kind == "sub":
                    b = buf(x)
                    vector.wait_ge(dma_sems[x % n_bufs], 16 * (x // n_bufs + 1))
                    vector.tensor_tensor(
                        out=b[:, 0:m],
                        in0=b[:, 0:m],
                        in1=b[:, m : 2 * m],
                        op=mybir.AluOpType.subtract,
                    ).then_inc(dve_sem, 1)
                else:
                    vector.wait_ge(dve_sem, sub_done[(x + 1) * group - 1])
                    for k, (i, j) in enumerate(_PAIRS):
                        vector.scalar_tensor_tensor(
                            out=pr_scrs[(x % 2) * 3 + k][:].rearrange(
                                "p (t r) -> p t r", t=group
                            ),
                            in0=dgroup(x, i),
                            scalar=1.0,
                            in1=dgroup(x, j),
                            op0=mybir.AluOpType.mult,
                            op1=mybir.AluOpType.mult,
                            accum_out=acc_cr[:, x * 3 + k : x * 3 + k + 1],
                        ).then_inc(dve_sem, 1)

        @block.scalar
        def _(scalar):
            for g in range(n_groups):
                scalar.wait_ge(dve_sem, sub_done[(g + 1) * group - 1])
                if g >= 2:
                    # scratch slot reuse from two groups back
                    scalar.wait_ge(act_sem, 3 * (g - 1))
                for i in range(3):
                    scalar.activation(
                        out=sq_scrs[(g % 2) * 3 + i][:].rearrange(
                            "p (t r) -> p t r", t=group
                        ),
                        in_=dgroup(g, i),
                        func=mybir.ActivationFunctionType.Square,
                        accum_out=acc_sq[:, g * 3 + i : g * 3 + i + 1],
                    ).then_inc(act_sem, 1)

    nc.compile()
    return nc

def build_gram_kernel_v2(n_rows: int, n_tiles: int = 32, n_bufs: int = 24,
                         group: int = 4, n_dbufs: int = 12,
                         n_scalar_dmas: int = 8, act_squares: int = 3,
                         skip_exit_barrier: bool = True):
    """bf16-deinterleaved variant of build_gram_kernel_raw.

    The fp32 per-tile subtract writes d = pred - targ as bf16 with the
    three vector components DEINTERLEAVED (each component a unit-stride
    block) into a small d-ring.  The multiply-reduces then read bf16 at
    step 1, which unlocks the DVE 2x packed perf mode (fp32/stride-3 in
    the baseline capped DVE at ~85-103 elem/ns and made compute lag the
    402 GB/s DMA stream by ~8 us).

    Other deltas vs the baseline:
      - input-ring reuse only waits on the SUB of the evicted tile (the
        crosses read the d-ring, not the input ring), so the DMA queue
        never stalls on the reduce tail;
      - the first `n_scalar_dmas` tile loads are issued from the scalar
        engine's HWDGE ring in parallel with the sync engine's, halving
        the issue-rate-limited ramp;
      - the accumulator is laid out group-major ([sq0..2 cr0..2] per
        group) so each output flush is ONE dma, and the final flush
        covers only the last group's 6 columns.

    Output: partials [128, 6 * n_groups], col 6g+i = sum d_i^2 of group
    g for i<3, col 6g+3+k = sum d_i*d_j for pair k.
    """
    assert n_tiles % group == 0 and n_bufs % group == 0
    assert n_dbufs % group == 0 and n_dbufs >= 2 * group
    assert n_rows % (P * n_tiles) == 0
    assert 0 <= act_squares <= 3
    n_groups = n_tiles // group
    r = n_rows // (P * n_tiles)
    m = 3 * r
    f32, bf16 = mybir.dt.float32, mybir.dt.bfloat16

    nc = bacc.Bacc("TRN2", target_bir_lowering=False, debug=False)
    pt = nc.dram_tensor("pt", [2, n_rows, 3], f32, kind="ExternalInput")
    out = nc.dram_tensor("partials", [P, 6 * n_groups], f32, kind="ExternalOutput")
    pt_v = pt[:].rearrange("w (t p r) c -> t p w (r c)", t=n_tiles, p=P)

    ring = nc.alloc_sbuf_tensor("ring", [P, n_bufs * 2 * m], f32).ap()
    dring = nc.alloc_sbuf_tensor("dring", [P, n_dbufs * m], bf16).ap()
    acc = nc.alloc_sbuf_tensor("acc", [P, 6 * n_groups], f32).ap()

    def buf(t):
        s = t % n_bufs
        return ring[:, s * 2 * m : (s + 1) * 2 * m]

    def sub_views(t):
        b = buf(t)
        p_v = b[:, 0:m].rearrange("p (r c) -> p r c", c=3)
        t_v = b[:, m : 2 * m].rearrange("p (r c) -> p r c", c=3)
        s = t % n_dbufs
        d_v = dring[:, s * m : (s + 1) * m].rearrange("p (c r) -> p r c", c=3)
        return p_v, t_v, d_v

    def dgroup(g, i):
        # component i of groups' d tiles: [128, group, r], unit inner stride
        s0 = (g * group) % n_dbufs
        w = dring[:, s0 * m : (s0 + group) * m]
        return w.rearrange("p (t c r) -> p t c r", t=group, c=3)[:, :, i, :]

    n_dve_sq = 3 - act_squares
    dve_ops_per_group = 3 + n_dve_sq
    # rotated dead-store scratch (2 groups deep per engine)
    dve_scrs = [
        nc.alloc_sbuf_tensor(f"dve_scr{k}", [P, group * r], bf16).ap()
        for k in range(2 * dve_ops_per_group)
    ]
    act_scrs = [
        nc.alloc_sbuf_tensor(f"act_scr{k}", [P, group * r], bf16).ap()
        for k in range(2 * act_squares)
    ]

    dma_sems = [nc.alloc_semaphore(f"dma_sem{i}") for i in range(n_bufs)]
    out_sem = nc.alloc_semaphore("out_sem")
    dve_sem = nc.alloc_semaphore("dve_sem")
    act_sem = nc.alloc_semaphore("act_sem") if act_squares else None

    # DVE emission order (as baseline): subs run ahead; group g's reduces
    # emitted after sub(4g+4) so their wait is stale when reached.
    dve_order = []
    for t in range(n_tiles):
        dve_order.append(("sub", t))
        if t % group == 0 and t >= group:
            dve_order.append(("red", t // group - 1))
    dve_order.append(("red", n_groups - 1))
    sub_done, redg_done = {}, {}
    v = 0
    for kind, x in dve_order:
        if kind == "sub":
            v += 1
            sub_done[x] = v
        else:
            v += dve_ops_per_group
            redg_done[x] = v

    # output flushes: all-but-last groups early, last group alone at the end
    chunks = [(0, n_groups - 1), (n_groups - 1, n_groups)]

    import contextlib

    @contextlib.contextmanager
    def _block():
        with nc.Block(no_gpsimd_drain=True) as blk:
            try:
                yield blk
            finally:
                if skip_exit_barrier:
                    nc.all_engine_barrier = lambda **kw: None
        if skip_exit_barrier:
            del nc.all_engine_barrier

    def emit_dma(eng, t):
        eng.dma_start(
            out=buf(t).rearrange("p (w m) -> p w m", w=2),
            in_=pt_v[t],
        ).then_inc(dma_sems[t % n_bufs], 16)

    with _block() as block:

        @block.sync
        def _(sync):
            for t in range(n_scalar_dmas, n_tiles):
                if t >= n_bufs:
                    # ring reuse: only the evicted tile's sub must be done
                    sync.wait_ge(dve_sem, sub_done[t - n_bufs])
                emit_dma(sync, t)
            n_out = 0
            for lo, hi in chunks:
                if act_squares:
                    sync.wait_ge(act_sem, act_squares * hi)
                sync.wait_ge(dve_sem, redg_done[hi - 1])
                sync.dma_start(
                    out=out[:, 6 * lo : 6 * hi], in_=acc[:, 6 * lo : 6 * hi]
                ).then_inc(out_sem, 16)
                n_out += 16
            sync.wait_ge(out_sem, n_out)

        @block.scalar
        def _(scalar):
            # early tile loads on the second HWDGE ring (ramp)
            for t in range(n_scalar_dmas):
                emit_dma(scalar, t)
            for g in range(n_groups):
                scalar.wait_ge(dve_sem, sub_done[(g + 1) * group - 1])
                if g >= 2:
                    scalar.wait_ge(act_sem, act_squares * (g - 1))
                for i in range(act_squares):
                    scalar.activation(
                        out=act_scrs[(g % 2) * act_squares + i][:].rearrange(
                            "p (t r) -> p t r", t=group
                        ),
                        in_=dgroup(g, i),
                        func=mybir.ActivationFunctionType.Square,
                        accum_out=acc[:, g * 6 + i : g * 6 + i + 1],
                    ).then_inc(act_sem, 1)

        @block.vector
        def _(vector):
            for kind, x in dve_order:
                if kind == "sub":
                    p_v, t_v, d_v = sub_views(x)
                    vector.wait_ge(dma_sems[x % n_bufs], 16 * (x // n_bufs + 1))
                    if x >= n_dbufs and act_squares:
                        # d-slot reuse: ACT squares of the evicted tile's
                        # group must be done (DVE's own reads are ordered
                        # by program order)
                        gp = (x - n_dbufs) // group
                        vector.wait_ge(act_sem, act_squares * (gp + 1))
                    vector.tensor_tensor(
                        out=d_v,
                        in0=p_v,
                        in1=t_v,
                        op=mybir.AluOpType.subtract,
                    ).then_inc(dve_sem, 1)
                else:
                    vector.wait_ge(dve_sem, sub_done[(x + 1) * group - 1])
                    ops = [(i, j, 3 + k) for k, (i, j) in enumerate(_PAIRS)]
                    ops += [(i, i, i) for i in range(act_squares, 3)]
                    for n, (i, j, col) in enumerate(ops):
                        vector.scalar_tensor_tensor(
                            out=dve_scrs[(x % 2) * dve_ops_per_group + n][:]
                            .rearrange("p (t r) -> p t r", t=group),
                            in0=dgroup(x, i),
                            scalar=1.0,
                            in1=dgroup(x, j),
                            op0=mybir.AluOpType.mult,
                            op1=mybir.AluOpType.mult,
                            accum_out=acc[:, x * 6 + col : x * 6 + col + 1],
                        ).then_inc(dve_sem, 1)

    nc.compile()
    return nc


def build_gram_kernel_v3(n_rows: int, n_tiles: int = 32,
                         group_sizes: tuple = (4, 4, 4, 4, 4, 4, 4, 2, 1, 1),
                         skip_exit_barrier: bool = True):
    """Planar bf16 variant: host supplies component-planar tiles, the DMA
    casts fp32->bf16 in flight (SWDGE), and every on-chip operand is
    unit-stride bf16.

    Input pt [n_tiles, 128, 6r] f32, per (tile, partition) one contiguous
    chunk [p0 r | p1 r | p2 r | t0 r | t1 r | t2 r] (6 KB for r=256 — the
    descriptor sweet spot, ~413 GB/s vs 402 for the baseline's split
    chunks).  SWDGE (gpsimd-issued) DMA casts to bf16 on the fly — probe-
    measured at full read rate.  Per tile the DVE subtract then runs in
    2x packed mode (bf16, step 1): d = pred - targ IN-PLACE over the pred
    half.  Cross/square reduces read d unit-stride (no stride-3 penalty).

    Every tile has its own SBUF slot (32 x 3 KB bf16) and semaphore — no
    ring reuse, so the DMA stream never waits on compute.  Group sizes
    taper at the end so the after-last-DMA tail is only the final tile's
    sub + three N=256 reduces.

    Output: partials [128, 6 * n_groups]; col 6g+i = sum d_i^2, col
    6g+3+k = sum d_i d_j over group g's tiles.
    """
    assert sum(group_sizes) == n_tiles
    assert n_rows % (P * n_tiles) == 0
    n_groups = len(group_sizes)
    r = n_rows // (P * n_tiles)
    h = 3 * r  # bf16 elems per half-tile per partition
    f32, bf16 = mybir.dt.float32, mybir.dt.bfloat16
    max_g = max(group_sizes)

    ends = []
    e = -1
    for sz in group_sizes:
        e += sz
        ends.append(e)
    starts = [e - sz + 1 for e, sz in zip(ends, group_sizes)]

    nc = bacc.Bacc("TRN2", target_bir_lowering=False, debug=False)
    pt = nc.dram_tensor("pt", [n_tiles, P, 2 * h], f32, kind="ExternalInput")
    out = nc.dram_tensor("partials", [P, 6 * n_groups], f32, kind="ExternalOutput")
    pt_v = pt[:]

    ring = nc.alloc_sbuf_tensor("ring", [P, n_tiles * 2 * h], bf16).ap()
    acc = nc.alloc_sbuf_tensor("acc", [P, 6 * n_groups], f32).ap()

    def half(t, w):
        return ring[:, (2 * t + w) * h : (2 * t + w + 1) * h]

    def dgroup(g, i):
        # component i of group g's d (pred) halves: [128, size, r] step-1
        s = starts[g]
        w = ring[:, 2 * s * h : 2 * (s + group_sizes[g]) * h]
        return w.rearrange(
            "p (t w c r) -> p t w c r", t=group_sizes[g], w=2, c=3
        )[:, :, 0, i, :]

    dve_scrs = [
        nc.alloc_sbuf_tensor(f"dve_scr{k}", [P, max_g * r], bf16).ap()
        for k in range(6)
    ]
    act_scrs = [
        nc.alloc_sbuf_tensor(f"act_scr{k}", [P, max_g * r], bf16).ap()
        for k in range(6)
    ]

    dma_sems = [nc.alloc_semaphore(f"dma_sem{t}") for t in range(n_tiles)]
    out_sem = nc.alloc_semaphore("out_sem")
    dve_sem = nc.alloc_semaphore("dve_sem")
    act_sem = nc.alloc_semaphore("act_sem")

    # DVE order: subs run ahead, group reduces staggered one sub late.
    dve_order = []
    for t in range(n_tiles):
        dve_order.append(("sub", t))
        dve_order.extend(("red", g) for g in range(n_groups) if ends[g] == t - 1)
    dve_order.extend(("red", g) for g in range(n_groups) if ends[g] >= n_tiles - 1)
    sub_done, redg_done = {}, {}
    v = 0
    for kind, x in dve_order:
        if kind == "sub":
            v += 1
            sub_done[x] = v
        else:
            v += 3
            redg_done[x] = v

    chunks = [(0, n_groups - 1), (n_groups - 1, n_groups)]

    import contextlib

    @contextlib.contextmanager
    def _block():
        with nc.Block() as blk:
            try:
                yield blk
            finally:
                if skip_exit_barrier:
                    nc.all_engine_barrier = lambda **kw: None
        if skip_exit_barrier:
            del nc.all_engine_barrier

    with _block() as block:

        @block.gpsimd
        def _(gp):
            for t in range(n_tiles):
                gp.dma_start(
                    out=ring[:, 2 * t * h : 2 * (t + 1) * h],
                    in_=pt_v[t],
                ).then_inc(dma_sems[t], 16)

        @block.sync
        def _(sync):
            n_out = 0
            for lo, hi in chunks:
                sync.wait_ge(act_sem, 3 * hi)
                sync.wait_ge(dve_sem, redg_done[hi - 1])
                sync.dma_start(
                    out=out[:, 6 * lo : 6 * hi], in_=acc[:, 6 * lo : 6 * hi]
                ).then_inc(out_sem, 16)
                n_out += 16
            sync.wait_ge(out_sem, n_out)

        @block.scalar
        def _(scalar):
            for g in range(n_groups):
                scalar.wait_ge(dve_sem, sub_done[ends[g]])
                if g >= 2:
                    scalar.wait_ge(act_sem, 3 * (g - 1))
                for i in range(3):
                    scalar.activation(
                        out=act_scrs[(g % 2) * 3 + i][
                            :, : group_sizes[g] * r
                        ].rearrange("p (t r) -> p t r", t=group_sizes[g]),
                        in_=dgroup(g, i),
                        func=mybir.ActivationFunctionType.Square,
                        accum_out=acc[:, g * 6 + i : g * 6 + i + 1],
                    ).then_inc(act_sem, 1)

        @block.vector
        def _(vector):
            for kind, x in dve_order:
                if kind == "sub":
                    vector.wait_ge(dma_sems[x], 16)
                    vector.tensor_tensor(
                        out=half(x, 0),
                        in0=half(x, 0),
                        in1=half(x, 1),
                        op=mybir.AluOpType.subtract,
                    ).then_inc(dve_sem, 1)
                else:
                    vector.wait_ge(dve_sem, sub_done[ends[x]])
                    for k, (i, j) in enumerate(_PAIRS):
                        vector.scalar_tensor_tensor(
                            out=dve_scrs[(x % 2) * 3 + k][
                                :, : group_sizes[x] * r
                            ].rearrange("p (t r) -> p t r", t=group_sizes[x]),
                            in0=dgroup(x, i),
                            scalar=1.0,
                            in1=dgroup(x, j),
                            op0=mybir.AluOpType.mult,
                            op1=mybir.AluOpType.mult,
                            accum_out=acc[:, x * 6 + 3 + k : x * 6 + 4 + k],
                        ).then_inc(dve_sem, 1)

    nc.compile()
    return nc


def build_gram_kernel_v4(n_rows: int, n_tiles: int = 32, n_bufs: int = 16,
                         n_dbufs: int = 32,
                         sub_sizes: tuple = (1, 1, 1, 1) + (4,) * 6 + (2, 1, 1),
                         cr_sizes: tuple = (4,) * 8,
                         sq_sizes: tuple = (4,) * 8,
                         n_scalar_dmas: int = 0, use_ttr: bool = False,
                         gp_batches: tuple = (),
                         skip_exit_barrier: bool = True):
    """Planar fp32 over HWDGE + bf16 d-ring; fused sub batches.

    Input pt [n_tiles, 128, 6r] f32 planar (one contiguous 6 KB chunk
    per tile/partition -> ~413 GB/s vs 402 for split chunks; HWDGE so no
    SWDGE descriptor-ring contention, which made SDMA engine 15 a 17%
    straggler that paced the whole SWDGE-cast variant).

    DVE subtracts in fused batches (one op across a batch's tiles, 3D AP
    [t, h] over the fp32 ring) writing unit-stride bf16 into the d-ring;
    cross/square reduces then run on step-1 bf16 (no stride-3 penalty).
    Crosses (DVE) use 8-tile groups, squares (ACT) 4-tile groups — the
    tilings are independent since the host sums all partial columns.
    All sizes taper to 1 tile at the end so the post-stream tail is just
    the last tile's sub + three N=r reduces.

    acc column layout (out [128, 3*(n_sq+n_cr)]): all-but-last sq groups,
    all-but-last cr groups, last sq group, last cr group — so each of the
    two output flushes is ONE contiguous dma and the final flush covers
    only the last-tile groups.
    """
    assert sum(sub_sizes) == n_tiles and sum(cr_sizes) == n_tiles
    assert sum(sq_sizes) == n_tiles
    assert n_rows % (P * n_tiles) == 0
    r = n_rows // (P * n_tiles)
    h = 3 * r
    f32, bf16 = mybir.dt.float32, mybir.dt.bfloat16

    def bounds(sizes):
        ends, e = [], -1
        for sz in sizes:
            e += sz
            ends.append(e)
        return [e - sz + 1 for e, sz in zip(ends, sizes)], ends

    sub_starts, sub_ends = bounds(sub_sizes)
    cr_starts, cr_ends = bounds(cr_sizes)
    sq_starts, sq_ends = bounds(sq_sizes)
    n_sq, n_cr = len(sq_sizes), len(cr_sizes)
    # fused subs and cross groups need their tiles contiguous in the rings
    for s, e in zip(sub_starts, sub_ends):
        assert (s % n_bufs) + (e - s) < n_bufs and (s % n_dbufs) + (e - s) < n_dbufs
    for s, e in zip(cr_starts, cr_ends):
        assert (s % n_dbufs) + (e - s) < n_dbufs
    for s, e in zip(sq_starts, sq_ends):
        assert (s % n_dbufs) + (e - s) < n_dbufs

    # acc columns ordered so the early flush [0, chunk_split) covers all
    # groups finishing by tile E1, and the final flush only the rest
    E1 = n_tiles - 5
    sq_early = [g for g in range(n_sq) if sq_ends[g] <= E1]
    sq_late = [g for g in range(n_sq) if sq_ends[g] > E1]
    cr_early = [g for g in range(n_cr) if cr_ends[g] <= E1]
    cr_late = [g for g in range(n_cr) if cr_ends[g] > E1]
    # groups complete in index order on each engine, so early must be a prefix
    assert sq_early == list(range(len(sq_early)))
    assert cr_early == list(range(len(cr_early)))
    sq_col, cr_col = {}, {}
    c = 0
    for g in sq_early:
        sq_col[g] = c; c += 3
    for g in cr_early:
        cr_col[g] = c; c += 3
    chunk_split = c
    for g in sq_late:
        sq_col[g] = c; c += 3
    for g in cr_late:
        cr_col[g] = c; c += 3
    n_cols = c

    nc = bacc.Bacc("TRN2", target_bir_lowering=False, debug=False)
    pt = nc.dram_tensor("pt", [n_tiles, P, 2 * h], f32, kind="ExternalInput")
    out = nc.dram_tensor("partials", [P, n_cols], f32, kind="ExternalOutput")
    pt_v = pt[:]

    ring = nc.alloc_sbuf_tensor("ring", [P, n_bufs * 2 * h], f32).ap()
    dring = nc.alloc_sbuf_tensor("dring", [P, n_dbufs * h], bf16).ap()
    acc = nc.alloc_sbuf_tensor("acc", [P, n_cols], f32).ap()
    max_cr = max(cr_sizes)
    max_sq = max(sq_sizes)
    dve_scrs = [
        nc.alloc_sbuf_tensor(f"dve_scr{k}", [P, max_cr * r], bf16).ap()
        for k in range(3)
    ]
    act_scrs = [
        nc.alloc_sbuf_tensor(f"act_scr{k}", [P, max_sq * r], bf16).ap()
        for k in range(3)
    ]

    dma_sems = [nc.alloc_semaphore(f"dma_sem{i}") for i in range(n_bufs)]
    out_sem = nc.alloc_semaphore("out_sem")
    dve_sem = nc.alloc_semaphore("dve_sem")
    act_sem = nc.alloc_semaphore("act_sem")
    gp_sem = nc.alloc_semaphore("gp_sem")

    def sub_views(b):
        s, sz = sub_starts[b], sub_sizes[b]
        rs = s % n_bufs
        w = ring[:, rs * 2 * h : (rs + sz) * 2 * h].rearrange(
            "p (t w h) -> p t w h", t=sz, w=2)
        ds = s % n_dbufs
        d = dring[:, ds * h : (ds + sz) * h].rearrange(
            "p (t h) -> p t h", t=sz)
        return w[:, :, 0, :], w[:, :, 1, :], d

    def dgroup(starts, sizes, g, i):
        s, sz = starts[g], sizes[g]
        ds = s % n_dbufs
        w = dring[:, ds * h : (ds + sz) * h]
        return w.rearrange("p (t c r) -> p t c r", t=sz, c=3)[:, :, i, :]

    # Sub batches listed in gp_batches run on the (otherwise idle) GpSimd
    # engine — DVE sub work shrinks below the DMA-arrival window so
    # schedule jitter can't cascade.  GpSimd elementwise is ~2.2 ns/elem,
    # under the 4-tile arrival period.
    n_batches = len(sub_sizes)
    gp_batches = tuple(sorted(gp_batches))
    assert all(0 <= b < n_batches for b in gp_batches)
    gp_done = {b: i + 1 for i, b in enumerate(gp_batches)}

    # DVE emission: cross group g emitted as soon as every DVE sub batch
    # overlapping it has been emitted (no stagger — a stagger delays
    # crosses a whole batch and starves the DVE when batches wait on DMA
    # arrivals; gp-run batches are covered by gp_sem waits instead)
    def overlapping_dve_batches(g):
        return [b for b in range(n_batches)
                if b not in gp_done
                and sub_ends[b] >= cr_starts[g] and sub_starts[b] <= cr_ends[g]]

    dve_order = []
    emitted_cr = set()
    for b in range(n_batches):
        if b in gp_done:
            continue
        dve_order.append(("sub", b))
        for g in range(n_cr):
            if g not in emitted_cr and all(
                bb <= b for bb in overlapping_dve_batches(g)
            ):
                emitted_cr.add(g)
                dve_order.append(("cr", g))
    assert len(emitted_cr) == n_cr
    sub_done, crg_done = {}, {}
    v = 0
    for kind, x in dve_order:
        v += 1 if kind == "sub" else 3
        (sub_done if kind == "sub" else crg_done)[x] = v

    def sub_targets(last_tile):
        """(dve_sem, gp_sem) targets ensuring every sub batch touching
        tiles <= last_tile has completed."""
        dve_t = gp_t = 0
        for b in range(n_batches):
            if sub_starts[b] <= last_tile:
                if b in gp_done:
                    gp_t = max(gp_t, gp_done[b])
                else:
                    dve_t = max(dve_t, sub_done[b])
        return dve_t, gp_t

    import contextlib

    @contextlib.contextmanager
    def _block():
        with nc.Block(no_gpsimd_drain=True) as blk:
            try:
                yield blk
            finally:
                if skip_exit_barrier:
                    nc.all_engine_barrier = lambda **kw: None
        if skip_exit_barrier:
            del nc.all_engine_barrier

    def emit_dma(eng, t):
        # split each partition's 6 KB chunk into two 3 KB descriptors:
        # >4 KB packets degrade SDMA round-robin under concurrent engine
        # load (SDMA 15 became a 20% straggler and paced every tile sem)
        eng.dma_start(
            out=ring[:, (t % n_bufs) * 2 * h : (t % n_bufs + 1) * 2 * h]
            .rearrange("p (x k) -> p x k", x=2),
            in_=pt_v[t].rearrange("p (x k) -> p x k", x=2),
        ).then_inc(dma_sems[t % n_bufs], 16)

    with _block() as block:

        if gp_batches:

            @block.gpsimd
            def _(gp):
                for b in gp_batches:
                    e = sub_ends[b]
                    gp.wait_ge(dma_sems[e % n_bufs], 16 * (e // n_bufs + 1))
                    p_v, t_v, d_v = sub_views(b)
                    gp.tensor_tensor(
                        out=d_v, in0=p_v, in1=t_v,
                        op=mybir.AluOpType.subtract,
                    ).then_inc(gp_sem, 1)

        @block.sync
        def _(sync):
            for t in range(n_scalar_dmas, n_tiles):
                if t >= n_bufs:
                    dve_t, gp_t = sub_targets(t - n_bufs)
                    if dve_t:
                        sync.wait_ge(dve_sem, dve_t)
                    if gp_t:
                        sync.wait_ge(gp_sem, gp_t)
                emit_dma(sync, t)
            n_out = 0
            for lo, hi, n_sq_done, n_cr_done in (
                (0, chunk_split, len(sq_early), len(cr_early)),
                (chunk_split, n_cols, n_sq, n_cr),
            ):
                sync.wait_ge(act_sem, 3 * n_sq_done)
                sync.wait_ge(dve_sem, crg_done[n_cr_done - 1])
                sync.dma_start(
                    out=out[:, lo:hi], in_=acc[:, lo:hi]
                ).then_inc(out_sem, 16)
                n_out += 16
            sync.wait_ge(out_sem, n_out)

        @block.scalar
        def _(scalar):
            for t in range(n_scalar_dmas):
                emit_dma(scalar, t)
            for g in range(n_sq):
                dve_t, gp_t = sub_targets(sq_ends[g])
                if dve_t:
                    scalar.wait_ge(dve_sem, dve_t)
                if gp_t:
                    scalar.wait_ge(gp_sem, gp_t)
                for i in range(3):
                    scalar.activation(
                        out=act_scrs[i][:, : sq_sizes[g] * r].rearrange(
                            "p (t r) -> p t r", t=sq_sizes[g]),
                        in_=dgroup(sq_starts, sq_sizes, g, i),
                        func=mybir.ActivationFunctionType.Square,
                        accum_out=acc[:, sq_col[g] + i : sq_col[g] + i + 1],
                    ).then_inc(act_sem, 1)

        @block.vector
        def _(vector):
            for kind, x in dve_order:
                if kind == "sub":
                    p_v, t_v, d_v = sub_views(x)
                    s, e = sub_starts[x], sub_ends[x]
                    vector.wait_ge(dma_sems[e % n_bufs], 16 * (e // n_bufs + 1))
                    if s >= n_dbufs:
                        # d-slot reuse: ACT squares over the evicted tiles
                        # must be done (DVE's own reads are program-ordered)
                        gp = next(g for g in range(n_sq)
                                  if sq_ends[g] >= e - n_dbufs)
                        vector.wait_ge(act_sem, 3 * (gp + 1))
                    vector.tensor_tensor(
                        out=d_v, in0=p_v, in1=t_v,
                        op=mybir.AluOpType.subtract,
                    ).then_inc(dve_sem, 1)
                else:
                    dve_t, gp_t = sub_targets(cr_ends[x])
                    if dve_t:
                        vector.wait_ge(dve_sem, dve_t)
                    if gp_t:
                        vector.wait_ge(gp_sem, gp_t)
                    for k, (i, j) in enumerate(_PAIRS):
                        scr = dve_scrs[k][:, : cr_sizes[x] * r].rearrange(
                            "p (t r) -> p t r", t=cr_sizes[x])
                        if use_ttr:
                            # fused multiply + sum-reduce, accumulator
                            # written directly by the instruction
                            vector.tensor_tensor_reduce(
                                out=scr,
                                in0=dgroup(cr_starts, cr_sizes, x, i),
                                in1=dgroup(cr_starts, cr_sizes, x, j),
                                scale=1.0,
                                scalar=0.0,
                                op0=mybir.AluOpType.mult,
                                op1=mybir.AluOpType.add,
                                accum_out=acc[:, cr_col[x] + k : cr_col[x] + k + 1],
                            ).then_inc(dve_sem, 1)
                        else:
                            vector.scalar_tensor_tensor(
                                out=scr,
                                in0=dgroup(cr_starts, cr_sizes, x, i),
                                scalar=1.0,
                                in1=dgroup(cr_starts, cr_sizes, x, j),
                                op0=mybir.AluOpType.mult,
                                op1=mybir.AluOpType.mult,
                                accum_out=acc[:, cr_col[x] + k : cr_col[x] + k + 1],
                            ).then_inc(dve_sem, 1)

    # record triple-row roles for the host-side unpack
    _V4_SQ_ROWS.clear()
    _V4_SQ_ROWS.extend(sq_col[g] // 3 for g in range(n_sq))
    _V4_CR_ROWS.clear()
    _V4_CR_ROWS.extend(cr_col[g] // 3 for g in range(n_cr))

    nc.compile()
    return nc


_V4_SQ_ROWS: list = []
_V4_CR_ROWS: list = []


def gram_from_partials_v4(partials: np.ndarray) -> np.ndarray:
    """v4 partials [..., 128, 3*(n_sq+n_cr)] -> 3x3 Gram (float64).

    Column triples ordered [sq_early, cr_early, sq_late, cr_late]; the
    builder records which triple rows are squares vs crosses in
    _V4_SQ_ROWS/_V4_CR_ROWS.
    """
    s = partials.astype(np.float64).reshape(-1, partials.shape[-1]).sum(axis=0)
    tri = s.reshape(-1, 3)
    sq = tri[_V4_SQ_ROWS].sum(axis=0)
    cr = tri[_V4_CR_ROWS].sum(axis=0)
    g = np.empty((3, 3), dtype=np.float64)
    g[0, 0], g[1, 1], g[2, 2] = sq
    for k, (i, j) in enumerate(_PAIRS):
        g[i, j] = g[j, i] = cr[k]
    return g


def planarize(predictions: np.ndarray, targets: np.ndarray,
              n_tiles: int = 32) -> np.ndarray:
    """[B,3] pred/targ -> per-core planar tiles [cores, n_tiles, P, 6r] f32."""
    b = predictions.shape[0]
    n_rows = b // N_CORES
    r = n_rows // (P * n_tiles)
    out = np.empty((N_CORES, n_tiles, P, 6 * r), dtype=np.float32)
    pv = out[..., : 3 * r].reshape(N_CORES, n_tiles, P, 3, r)
    tv = out[..., 3 * r :].reshape(N_CORES, n_tiles, P, 3, r)
    pv[:] = np.asarray(predictions, dtype=np.float32).reshape(
        N_CORES, n_tiles, P, r, 3).transpose(0, 1, 2, 4, 3)
    tv[:] = np.asarray(targets, dtype=np.float32).reshape(
        N_CORES, n_tiles, P, r, 3).transpose(0, 1, 2, 4, 3)
    return out


_NC_CACHE: dict[tuple, object] = {}


def _get_nc(n_rows: int, n_tiles: int, use_act: bool, raw: bool = False,
            group: int = 4, **kw):
    key = (n_rows, n_tiles, use_act, raw, group, tuple(sorted(kw.items())))
    if key not in _NC_CACHE:
        if raw:
            _NC_CACHE[key] = build_gram_kernel_v4(n_rows, n_tiles, **kw)
        else:
            _NC_CACHE[key] = build_gram_kernel(n_rows, n_tiles, use_act)
    return _NC_CACHE[key]


def gram_from_partials(partials: np.ndarray, n_tiles: int | None = None) -> np.ndarray:
    """Device partials -> full 3x3 Gram matrix (float64).

    Dispatches on column count: 51 -> v4 layout, else v2/v3 group-major
    layout (col 6g+i = sq_i, col 6g+3+k = cross pair k).
    """
    if partials.shape[-1] == 3 * (len(_V4_SQ_ROWS) + len(_V4_CR_ROWS)):
        return gram_from_partials_v4(partials)
    slots = partials.shape[-1] // 6
    s = partials.astype(np.float64).reshape(-1, slots, 6).sum(axis=0).sum(axis=0)
    g = np.empty((3, 3), dtype=np.float64)
    g[0, 0], g[1, 1], g[2, 2] = s[0:3]
    for k, (i, j) in enumerate(_PAIRS):
        g[i, j] = g[j, i] = s[3 + k]
    return g


def run_device_partials(predictions: np.ndarray, targets: np.ndarray,
                        n_tiles: int = 32, use_act: bool = True,
                        raw: bool = True, group: int = 4, **run_kwargs):
    """Shard over N_CORES, run on device, return per-core partials + results."""
    b = predictions.shape[0]
    assert b % N_CORES == 0
    n_rows = b // N_CORES
    nc = _get_nc(n_rows, n_tiles, use_act, raw, group)
    planar = planarize(predictions, targets, n_tiles)
    in_maps = [{"pt": planar[c]} for c in range(N_CORES)]
    res = run_bass_kernel_spmd(nc, in_maps, list(range(N_CORES)), **run_kwargs)
    partials = np.stack([r["partials"] for r in res.results])
    return partials, res


def kernel(predictions: np.ndarray, targets: np.ndarray, sigma: np.ndarray) -> np.ndarray:
    predictions = np.asarray(predictions, dtype=np.float32)
    targets = np.asarray(targets, dtype=np.float32)
    sigma64 = np.asarray(sigma, dtype=np.float64)

    partials, _ = run_device_partials(predictions, targets, n_tiles=32, raw=True)
    g = gram_from_partials(partials)

    sigma_inv = np.linalg.inv(sigma64)
    _, logdet = np.linalg.slogdet(sigma64)
    mean_mahal = float((sigma_inv * g).sum()) / predictions.shape[0]
    loss = abs(logdet + mean_mahal)
    return np.float32(loss)



# revision 33
# speedup vs baseline: 1.1177x; 1.0009x over previous
"""Trainium2 Bass kernel for CustomLossWithCovariance.

loss = abs(logdet(sigma) + mean_b[(p_b - t_b)^T sigma^{-1} (p_b - t_b)])

Only the 3x3 Gram matrix G = sum_b d_b d_b^T (d = pred - targ) requires
touching the [B, 3] data; the device computes per-core partial pair-sums
of G, and the host finishes with the tiny 3x3 algebra:
    mean_mahalanobis = <sigma_inv, G> / B
    loss = |logdet(sigma) + mean_mahalanobis|

Sharding: data-parallel over the batch across 8 NeuronCores (each core
streams a contiguous [B/8, 3] shard; partial sums gathered on host).

Active per-core kernel: build_gram_kernel_v4 (raw Bacc, manual
semaphores).  The host pre-lays each tile component-PLANAR — per
(tile, partition) one contiguous fp32 chunk [p0 r|p1 r|p2 r|t0 r|..] —
so every on-chip operand is unit-stride and each partition chunk maps
to two 3 KB HWDGE descriptors (~415 GB/s streaming; >4 KB descriptors
or SWDGE made SDMA engine 15 a straggler that paced every tile).
Per batch of tiles:
  - DVE: fused d = pred - targ -> bf16 d-ring (every tile has its own
    SBUF slot and d slot; no ring-reuse waits anywhere)
  - ACT: Square(d_i) with accum_out over 4-tile groups (bf16 step-1)
  - DVE: scalar_tensor_tensor(d_i * d_j) accum over 4-tile groups
Batch/group sizes taper at the edges so DVE starts at first-tile
arrival and only a 1-tile sub + one reduce group trail the last DMA.
Earlier variants kept for reference: build_gram_kernel (Tile
framework), build_gram_kernel_raw (fp32 stride-3, the 84 us baseline),
v2/v3 (scatter-write and SWDGE-cast experiments).
"""

import numpy as np

import concourse.bass as bass
import concourse.bacc as bacc
import concourse.mybir as mybir
from concourse import tile
from concourse.bass_utils import run_bass_kernel_spmd

N_CORES = 8
B_FULL = 8388608
P = 128

_PAIRS = [(0, 1), (0, 2), (1, 2)]


def build_gram_kernel(n_rows: int, n_tiles: int, use_act: bool = True):
    """Build the per-core Bass module.

    Input: pt [2, n_rows, 3] f32 (pred stacked with targ)
    Output: partials [128, 6 * n_tiles] f32
        col t*3+i            : sum over this tile/partition of d_i^2
        col 3*n_tiles + t*3+k: sum of d_i*d_j for pair k in _PAIRS
    """
    assert n_rows % (P * n_tiles) == 0
    r = n_rows // (P * n_tiles)  # rows per partition per tile
    m = 3 * r                    # flat f32 elements per partition per tile
    f32 = mybir.dt.float32

    # Bacc (not plain Bass): its compile() pass legalizes semaphore waits
    # (each TRN2 instruction holds at most one wait slot).
    nc = bacc.Bacc("TRN2", target_bir_lowering=False, debug=False)
    pt = nc.dram_tensor("pt", [2, n_rows, 3], f32, kind="ExternalInput")
    out = nc.dram_tensor("partials", [P, 6 * n_tiles], f32, kind="ExternalOutput")

    # [t][p][w(2), m] — per tile/partition: pred chunk and targ chunk, each
    # m contiguous f32 in DRAM.
    pt_v = pt[:].rearrange("w (t p r) c -> t p w (r c)", t=n_tiles, p=P)

    with tile.TileContext(nc) as tc:
        with (
            tc.tile_pool(name="io", bufs=3) as io_pool,
            tc.tile_pool(name="dve_scr", bufs=2) as dve_scr,
            tc.tile_pool(name="act_scr", bufs=2) as act_scr,
            tc.tile_pool(name="acc", bufs=1) as acc_pool,
        ):
            acc_sq = acc_pool.tile([P, 3 * n_tiles], f32)
            acc_cr = acc_pool.tile([P, 3 * n_tiles], f32)

            for t in range(n_tiles):
                buf = io_pool.tile([P, 2 * m], f32, tag="buf")
                nc.sync.dma_start(
                    out=buf[:].rearrange("p (w m) -> p w m", w=2),
                    in_=pt_v[t],
                )

                # In-place: d = pred - targ, overwriting the pred half.
                nc.vector.tensor_tensor(
                    out=buf[:, 0:m],
                    in0=buf[:, 0:m],
                    in1=buf[:, m : 2 * m],
                    op=mybir.AluOpType.subtract,
                )
                d3 = buf[:, 0:m].rearrange("p (r c) -> p c r", c=3)

                # Diagonal sums on the scalar engine (Square + accum_out),
                # overlapping with the DVE cross-products.
                if use_act:
                    for i in range(3):
                        sq = act_scr.tile([P, r], f32, tag="sq")
                        nc.scalar.activation(
                            out=sq[:],
                            in_=d3[:, i, :],
                            func=mybir.ActivationFunctionType.Square,
                            accum_out=acc_sq[:, t * 3 + i : t * 3 + i + 1],
                        )
                else:
                    for i in range(3):
                        sq = dve_scr.tile([P, r], f32, tag="pr")
                        nc.vector.scalar_tensor_tensor(
                            out=sq[:],
                            in0=d3[:, i, :],
                            scalar=1.0,
                            in1=d3[:, i, :],
                            op0=mybir.AluOpType.mult,
                            op1=mybir.AluOpType.mult,
                            accum_out=acc_sq[:, t * 3 + i : t * 3 + i + 1],
                        )
                # Cross sums: fused multiply+reduce on DVE
                # (scalar_tensor_tensor: out = (in0 * 1.0) * in1, accum = sum).
                for k, (i, j) in enumerate(_PAIRS):
                    pr = dve_scr.tile([P, r], f32, tag="pr")
                    nc.vector.scalar_tensor_tensor(
                        out=pr[:],
                        in0=d3[:, i, :],
                        scalar=1.0,
                        in1=d3[:, j, :],
                        op0=mybir.AluOpType.mult,
                        op1=mybir.AluOpType.mult,
                        accum_out=acc_cr[:, t * 3 + k : t * 3 + k + 1],
                    )

            nc.sync.dma_start(out=out[:, 0 : 3 * n_tiles], in_=acc_sq[:])
            nc.sync.dma_start(out=out[:, 3 * n_tiles : 6 * n_tiles], in_=acc_cr[:])

    nc.compile()
    return nc


def build_gram_kernel_raw(n_rows: int, n_tiles: int = 32, n_bufs: int = 24,
                          group: int = 4, skip_exit_barrier: bool = True):
    """Raw-Bacc variant: manual semaphores, no TileContext.

    Skips Tile's prologue/epilogue (drain + two all-engine EVSEM
    barriers, ~16 us) — the only sync needed is a three-semaphore chain:
    DMA loads (one HWDGE ring) -> DVE -> ACT.

    The ring of tile buffers lives in ONE SBUF tensor so the fused
    multiply-reduces can span `group` consecutive tiles with a single
    instruction (free-dim AP [group, r]) — amortizing the per-op fixed
    cost and the accumulator-drain, which keeps both compute engines
    well under the DMA pace.

    Input: pt [2, n_rows, 3] f32. Output: partials [128, 6 * n_groups]
    (same slot layout as build_gram_kernel, with n_groups slots).
    """
    assert n_tiles % group == 0 and n_bufs % group == 0
    assert n_rows % (P * n_tiles) == 0
    n_groups = n_tiles // group
    r = n_rows // (P * n_tiles)
    m = 3 * r
    f32 = mybir.dt.float32

    nc = bacc.Bacc("TRN2", target_bir_lowering=False, debug=False)
    pt = nc.dram_tensor("pt", [2, n_rows, 3], f32, kind="ExternalInput")
    out = nc.dram_tensor("partials", [P, 6 * n_groups], f32, kind="ExternalOutput")
    pt_v = pt[:].rearrange("w (t p r) c -> t p w (r c)", t=n_tiles, p=P)

    ring = nc.alloc_sbuf_tensor("ring", [P, n_bufs * 2 * m], f32).ap()

    def buf(t):
        s = t % n_bufs
        return ring[:, s * 2 * m : (s + 1) * 2 * m]

    def dgroup(g, i):
        # component i of the diff halves of tiles 4g..4g+3: [128, group, r]
        s0 = (g * group) % n_bufs
        w = ring[:, s0 * 2 * m : (s0 + group) * 2 * m]
        return w.rearrange("p (t w r c) -> p t w c r", t=group, w=2, c=3)[:, :, 0, i, :]

    acc_sq = nc.alloc_sbuf_tensor("acc_sq", [P, 3 * n_groups], f32).ap()
    acc_cr = nc.alloc_sbuf_tensor("acc_cr", [P, 3 * n_groups], f32).ap()
    # Rotated scratch (dead stores of the fused ops), 2 groups deep so each
    # group's single stale semaphore wait also covers the scratch WAW from
    # two groups back.
    pr_scrs = [
        nc.alloc_sbuf_tensor(f"pr_scr{k}", [P, group * r], f32).ap() for k in range(6)
    ]
    sq_scrs = [
        nc.alloc_sbuf_tensor(f"sq_scr{k}", [P, group * r], f32).ap() for k in range(6)
    ]

    # One DMA-completion semaphore per ring buffer: a single shared sem
    # would be unsound — each dma_start is split across 16 SDMA engines
    # whose sub-completions interleave across in-flight DMAs.
    dma_sems = [nc.alloc_semaphore(f"dma_sem{i}") for i in range(n_bufs)]
    out_sem = nc.alloc_semaphore("out_sem")
    dve_sem = nc.alloc_semaphore("dve_sem")
    act_sem = nc.alloc_semaphore("act_sem")

    # DVE emission order: subs run ahead; the grouped multiply-reduces for
    # group g are emitted after sub(4g+4) so their drain-wait on the last
    # sub of the group is already satisfied when it executes (DVE writes
    # drain asynchronously). Only the last group trails the final sub.
    dve_order = []
    for t in range(n_tiles):
        dve_order.append(("sub", t))
        if t % group == 0 and t >= group:
            # one sub of stagger after the group's last sub
            dve_order.append(("stt", t // group - 1))
    dve_order.append(("stt", n_groups - 1))
    sub_done, sttg_done = {}, {}
    v = 0
    for kind, x in dve_order:
        if kind == "sub":
            v += 1
            sub_done[x] = v
        else:
            v += 3
            sttg_done[x] = v

    # Output chunks: flush finished accumulator columns while later tiles
    # still stream, so the tail only waits on the last small chunk.
    chunk = max(1, n_groups // 2)
    chunks = [(c, min(c + chunk, n_groups)) for c in range(0, n_groups, chunk)]

    import contextlib

    @contextlib.contextmanager
    def _block():
        # no_gpsimd_drain=True emits per-engine drains explicitly and then a
        # sem-only all-engine butterfly. The butterfly only delays NEFF end
        # (outputs are already fenced by the sequencer's out_sem wait), so
        # optionally no-op it during Block.__exit__.
        with nc.Block(no_gpsimd_drain=True) as blk:
            try:
                yield blk
            finally:
                if skip_exit_barrier:
                    nc.all_engine_barrier = lambda **kw: None
        if skip_exit_barrier:
            del nc.all_engine_barrier  # restore class method

    with _block() as block:

        @block.sync
        def _(sync):
            for t in range(n_tiles):
                if t >= n_bufs:
                    # ring reuse: all consumers of the buffer's previous
                    # occupant (tile t - n_bufs) must be done
                    prev = t - n_bufs
                    sync.wait_ge(dve_sem, sttg_done[prev // group])
                    sync.wait_ge(act_sem, 3 * (prev // group + 1))
                sync.dma_start(
                    out=buf(t).rearrange("p (w m) -> p w m", w=2),
                    in_=pt_v[t],
                ).then_inc(dma_sems[t % n_bufs], 16)
            n_out = 0
            for lo, hi in chunks:
                sync.wait_ge(act_sem, 3 * hi)
                sync.dma_start(
                    out=out[:, 3 * lo : 3 * hi], in_=acc_sq[:, 3 * lo : 3 * hi]
                ).then_inc(out_sem, 16)
                sync.wait_ge(dve_sem, sttg_done[hi - 1])
                sync.dma_start(
                    out=out[:, 3 * (n_groups + lo) : 3 * (n_groups + hi)],
                    in_=acc_cr[:, 3 * lo : 3 * hi],
                ).then_inc(out_sem, 16)
                n_out += 32
            sync.wait_ge(out_sem, n_out)

        @block.vector
        def _(vector):
            for kind, x in dve_order:
                if kind == "sub":
                    b = buf(x)
                    vector.wait_ge(dma_sems[x % n_bufs], 16 * (x // n_bufs + 1))
                    vector.tensor_tensor(
                        out=b[:, 0:m],
                        in0=b[:, 0:m],
                        in1=b[:, m : 2 * m],
                        op=mybir.AluOpType.subtract,
                    ).then_inc(dve_sem, 1)
                else:
                    vector.wait_ge(dve_sem, sub_done[(x + 1) * group - 1])
                    for k, (i, j) in enumerate(_PAIRS):
                        vector.scalar_tensor_tensor(
                            out=pr_scrs[(x % 2) * 3 + k][:].rearrange(
                                "p (t r) -> p t r", t=group
                            ),
                            in0=dgroup(x, i),
                            scalar=1.0,
                            in1=dgroup(x, j),
                            op0=mybir.AluOpType.mult,
                            op1=mybir.AluOpType.mult,
                            accum_out=acc_cr[:, x * 3 + k : x * 3 + k + 1],
                        ).then_inc(dve_sem, 1)

        @block.scalar
        def _(scalar):
            for g in range(n_groups):
                scalar.wait_ge(dve_sem, sub_done[(g + 1) * group - 1])
                if g >= 2:
                    # scratch slot reuse from two groups back
                    scalar.wait_ge(act_sem, 3 * (g - 1))
                for i in range(3):
                    scalar.activation(
                        out=sq_scrs[(g % 2) * 3 + i][:].rearrange(
                            "p (t r) -> p t r", t=group
                        ),
                        in_=dgroup(g, i),
                        func=mybir.ActivationFunctionType.Square,
                        accum_out=acc_sq[:, g * 3 + i : g * 3 + i + 1],
                    ).then_inc(act_sem, 1)

    nc.compile()
    return nc

def build_gram_kernel_v2(n_rows: int, n_tiles: int = 32, n_bufs: int = 24,
                         group: int = 4, n_dbufs: int = 12,
                         n_scalar_dmas: int = 8, act_squares: int = 3,
                         skip_exit_barrier: bool = True):
    """bf16-deinterleaved variant of build_gram_kernel_raw.

    The fp32 per-tile subtract writes d = pred - targ as bf16 with the
    three vector components DEINTERLEAVED (each component a unit-stride
    block) into a small d-ring.  The multiply-reduces then read bf16 at
    step 1, which unlocks the DVE 2x packed perf mode (fp32/stride-3 in
    the baseline capped DVE at ~85-103 elem/ns and made compute lag the
    402 GB/s DMA stream by ~8 us).

    Other deltas vs the baseline:
      - input-ring reuse only waits on the SUB of the evicted tile (the
        crosses read the d-ring, not the input ring), so the DMA queue
        never stalls on the reduce tail;
      - the first `n_scalar_dmas` tile loads are issued from the scalar
        engine's HWDGE ring in parallel with the sync engine's, halving
        the issue-rate-limited ramp;
      - the accumulator is laid out group-major ([sq0..2 cr0..2] per
        group) so each output flush is ONE dma, and the final flush
        covers only the last group's 6 columns.

    Output: partials [128, 6 * n_groups], col 6g+i = sum d_i^2 of group
    g for i<3, col 6g+3+k = sum d_i*d_j for pair k.
    """
    assert n_tiles % group == 0 and n_bufs % group == 0
    assert n_dbufs % group == 0 and n_dbufs >= 2 * group
    assert n_rows % (P * n_tiles) == 0
    assert 0 <= act_squares <= 3
    n_groups = n_tiles // group
    r = n_rows // (P * n_tiles)
    m = 3 * r
    f32, bf16 = mybir.dt.float32, mybir.dt.bfloat16

    nc = bacc.Bacc("TRN2", target_bir_lowering=False, debug=False)
    pt = nc.dram_tensor("pt", [2, n_rows, 3], f32, kind="ExternalInput")
    out = nc.dram_tensor("partials", [P, 6 * n_groups], f32, kind="ExternalOutput")
    pt_v = pt[:].rearrange("w (t p r) c -> t p w (r c)", t=n_tiles, p=P)

    ring = nc.alloc_sbuf_tensor("ring", [P, n_bufs * 2 * m], f32).ap()
    dring = nc.alloc_sbuf_tensor("dring", [P, n_dbufs * m], bf16).ap()
    acc = nc.alloc_sbuf_tensor("acc", [P, 6 * n_groups], f32).ap()

    def buf(t):
        s = t % n_bufs
        return ring[:, s * 2 * m : (s + 1) * 2 * m]

    def sub_views(t):
        b = buf(t)
        p_v = b[:, 0:m].rearrange("p (r c) -> p r c", c=3)
        t_v = b[:, m : 2 * m].rearrange("p (r c) -> p r c", c=3)
        s = t % n_dbufs
        d_v = dring[:, s * m : (s + 1) * m].rearrange("p (c r) -> p r c", c=3)
        return p_v, t_v, d_v

    def dgroup(g, i):
        # component i of groups' d tiles: [128, group, r], unit inner stride
        s0 = (g * group) % n_dbufs
        w = dring[:, s0 * m : (s0 + group) * m]
        return w.rearrange("p (t c r) -> p t c r", t=group, c=3)[:, :, i, :]

    n_dve_sq = 3 - act_squares
    dve_ops_per_group = 3 + n_dve_sq
    # rotated dead-store scratch (2 groups deep per engine)
    dve_scrs = [
        nc.alloc_sbuf_tensor(f"dve_scr{k}", [P, group * r], bf16).ap()
        for k in range(2 * dve_ops_per_group)
    ]
    act_scrs = [
        nc.alloc_sbuf_tensor(f"act_scr{k}", [P, group * r], bf16).ap()
        for k in range(2 * act_squares)
    ]

    dma_sems = [nc.alloc_semaphore(f"dma_sem{i}") for i in range(n_bufs)]
    out_sem = nc.alloc_semaphore("out_sem")
    dve_sem = nc.alloc_semaphore("dve_sem")
    act_sem = nc.alloc_semaphore("act_sem") if act_squares else None

    # DVE emission order (as baseline): subs run ahead; group g's reduces
    # emitted after sub(4g+4) so their wait is stale when reached.
    dve_order = []
    for t in range(n_tiles):
        dve_order.append(("sub", t))
        if t % group == 0 and t >= group:
            dve_order.append(("red", t // group - 1))
    dve_order.append(("red", n_groups - 1))
    sub_done, redg_done = {}, {}
    v = 0
    for kind, x in dve_order:
        if kind == "sub":
            v += 1
            sub_done[x] = v
        else:
            v += dve_ops_per_group
            redg_done[x] = v

    # output flushes: all-but-last groups early, last group alone at the end
    chunks = [(0, n_groups - 1), (n_groups - 1, n_groups)]

    import contextlib

    @contextlib.contextmanager
    def _block():
        with nc.Block(no_gpsimd_drain=True) as blk:
            try:
                yield blk
            finally:
                if skip_exit_barrier:
                    nc.all_engine_barrier = lambda **kw: None
        if skip_exit_barrier:
            del nc.all_engine_barrier

    def emit_dma(eng, t):
        eng.dma_start(
            out=buf(t).rearrange("p (w m) -> p w m", w=2),
            in_=pt_v[t],
        ).then_inc(dma_sems[t % n_bufs], 16)

    with _block() as block:

        @block.sync
        def _(sync):
            for t in range(n_scalar_dmas, n_tiles):
                if t >= n_bufs:
                    # ring reuse: only the evicted tile's sub must be done
                    sync.wait_ge(dve_sem, sub_done[t - n_bufs])
                emit_dma(sync, t)
            n_out = 0
            for lo, hi in chunks:
                if act_squares:
                    sync.wait_ge(act_sem, act_squares * hi)
                sync.wait_ge(dve_sem, redg_done[hi - 1])
                sync.dma_start(
                    out=out[:, 6 * lo : 6 * hi], in_=acc[:, 6 * lo : 6 * hi]
                ).then_inc(out_sem, 16)
                n_out += 16
            sync.wait_ge(out_sem, n_out)

        @block.scalar
        def _(scalar):
            # early tile loads on the second HWDGE ring (ramp)
            for t in range(n_scalar_dmas):
                emit_dma(scalar, t)
            for g in range(n_groups):
                scalar.wait_ge(dve_sem, sub_done[(g + 1) * group - 1])
                if g >= 2:
                    scalar.wait_ge(act_sem, act_squares * (g - 1))
                for i in range(act_squares):
                    scalar.activation(
                        out=act_scrs[(g % 2) * act_squares + i][:].rearrange(
                            "p (t r) -> p t r", t=group
                        ),
                        in_=dgroup(g, i),
                        func=mybir.ActivationFunctionType.Square,
                        accum_out=acc[:, g * 6 + i : g * 6 + i + 1],
                    ).then_inc(act_sem, 1)

        @block.vector
        def _(vector):
            for kind, x in dve_order:
                if kind == "sub":
                    p_v, t_v, d_v = sub_views(x)
                    vector.wait_ge(dma_sems[x % n_bufs], 16 * (x // n_bufs + 1))
                    if x >= n_dbufs and act_squares:
                        # d-slot reuse: ACT squares of the evicted tile's
                        # group must be done (DVE's own reads are ordered
                        # by program order)
                        gp = (x - n_dbufs) // group
                        vector.wait_ge(act_sem, act_squares * (gp + 1))
                    vector.tensor_tensor(
                        out=d_v,
                        in0=p_v,
                        in1=t_v,
                        op=mybir.AluOpType.subtract,
                    ).then_inc(dve_sem, 1)
                else:
                    vector.wait_ge(dve_sem, sub_done[(x + 1) * group - 1])
                    ops = [(i, j, 3 + k) for k, (i, j) in enumerate(_PAIRS)]
                    ops += [(i, i, i) for i in range(act_squares, 3)]
                    for n, (i, j, col) in enumerate(ops):
                        vector.scalar_tensor_tensor(
                            out=dve_scrs[(x % 2) * dve_ops_per_group + n][:]
                            .rearrange("p (t r) -> p t r", t=group),
                            in0=dgroup(x, i),
                            scalar=1.0,
                            in1=dgroup(x, j),
                            op0=mybir.AluOpType.mult,
                            op1=mybir.AluOpType.mult,
                            accum_out=acc[:, x * 6 + col : x * 6 + col + 1],
                        ).then_inc(dve_sem, 1)

    nc.compile()
    return nc


def build_gram_kernel_v3(n_rows: int, n_tiles: int = 32,
                         group_sizes: tuple = (4, 4, 4, 4, 4, 4, 4, 2, 1, 1),
                         skip_exit_barrier: bool = True):
    """Planar bf16 variant: host supplies component-planar tiles, the DMA
    casts fp32->bf16 in flight (SWDGE), and every on-chip operand is
    unit-stride bf16.

    Input pt [n_tiles, 128, 6r] f32, per (tile, partition) one contiguous
    chunk [p0 r | p1 r | p2 r | t0 r | t1 r | t2 r] (6 KB for r=256 — the
    descriptor sweet spot, ~413 GB/s vs 402 for the baseline's split
    chunks).  SWDGE (gpsimd-issued) DMA casts to bf16 on the fly — probe-
    measured at full read rate.  Per tile the DVE subtract then runs in
    2x packed mode (bf16, step 1): d = pred - targ IN-PLACE over the pred
    half.  Cross/square reduces read d unit-stride (no stride-3 penalty).

    Every tile has its own SBUF slot (32 x 3 KB bf16) and semaphore — no
    ring reuse, so the DMA stream never waits on compute.  Group sizes
    taper at the end so the after-last-DMA tail is only the final tile's
    sub + three N=256 reduces.

    Output: partials [128, 6 * n_groups]; col 6g+i = sum d_i^2, col
    6g+3+k = sum d_i d_j over group g's tiles.
    """
    assert sum(group_sizes) == n_tiles
    assert n_rows % (P * n_tiles) == 0
    n_groups = len(group_sizes)
    r = n_rows // (P * n_tiles)
    h = 3 * r  # bf16 elems per half-tile per partition
    f32, bf16 = mybir.dt.float32, mybir.dt.bfloat16
    max_g = max(group_sizes)

    ends = []
    e = -1
    for sz in group_sizes:
        e += sz
        ends.append(e)
    starts = [e - sz + 1 for e, sz in zip(ends, group_sizes)]

    nc = bacc.Bacc("TRN2", target_bir_lowering=False, debug=False)
    pt = nc.dram_tensor("pt", [n_tiles, P, 2 * h], f32, kind="ExternalInput")
    out = nc.dram_tensor("partials", [P, 6 * n_groups], f32, kind="ExternalOutput")
    pt_v = pt[:]

    ring = nc.alloc_sbuf_tensor("ring", [P, n_tiles * 2 * h], bf16).ap()
    acc = nc.alloc_sbuf_tensor("acc", [P, 6 * n_groups], f32).ap()

    def half(t, w):
        return ring[:, (2 * t + w) * h : (2 * t + w + 1) * h]

    def dgroup(g, i):
        # component i of group g's d (pred) halves: [128, size, r] step-1
        s = starts[g]
        w = ring[:, 2 * s * h : 2 * (s + group_sizes[g]) * h]
        return w.rearrange(
            "p (t w c r) -> p t w c r", t=group_sizes[g], w=2, c=3
        )[:, :, 0, i, :]

    dve_scrs = [
        nc.alloc_sbuf_tensor(f"dve_scr{k}", [P, max_g * r], bf16).ap()
        for k in range(6)
    ]
    act_scrs = [
        nc.alloc_sbuf_tensor(f"act_scr{k}", [P, max_g * r], bf16).ap()
        for k in range(6)
    ]

    dma_sems = [nc.alloc_semaphore(f"dma_sem{t}") for t in range(n_tiles)]
    out_sem = nc.alloc_semaphore("out_sem")
    dve_sem = nc.alloc_semaphore("dve_sem")
    act_sem = nc.alloc_semaphore("act_sem")

    # DVE order: subs run ahead, group reduces staggered one sub late.
    dve_order = []
    for t in range(n_tiles):
        dve_order.append(("sub", t))
        dve_order.extend(("red", g) for g in range(n_groups) if ends[g] == t - 1)
    dve_order.extend(("red", g) for g in range(n_groups) if ends[g] >= n_tiles - 1)
    sub_done, redg_done = {}, {}
    v = 0
    for kind, x in dve_order:
        if kind == "sub":
            v += 1
            sub_done[x] = v
        else:
            v += 3
            redg_done[x] = v

    chunks = [(0, n_groups - 1), (n_groups - 1, n_groups)]

    import contextlib

    @contextlib.contextmanager
    def _block():
        with nc.Block() as blk:
            try:
                yield blk
            finally:
                if skip_exit_barrier:
                    nc.all_engine_barrier = lambda **kw: None
        if skip_exit_barrier:
            del nc.all_engine_barrier

    with _block() as block:

        @block.gpsimd
        def _(gp):
            for t in range(n_tiles):
                gp.dma_start(
                    out=ring[:, 2 * t * h : 2 * (t + 1) * h],
                    in_=pt_v[t],
                ).then_inc(dma_sems[t], 16)

        @block.sync
        def _(sync):
            n_out = 0
            for lo, hi in chunks:
                sync.wait_ge(act_sem, 3 * hi)
                sync.wait_ge(dve_sem, redg_done[hi - 1])
                sync.dma_start(
                    out=out[:, 6 * lo : 6 * hi], in_=acc[:, 6 * lo : 6 * hi]
                ).then_inc(out_sem, 16)
                n_out += 16
            sync.wait_ge(out_sem, n_out)

        @block.scalar
        def _(scalar):
            for g in range(n_groups):
                scalar.wait_ge(dve_sem, sub_done[ends[g]])
                if g >= 2:
                    scalar.wait_ge(act_sem, 3 * (g - 1))
                for i in range(3):
                    scalar.activation(
                        out=act_scrs[(g % 2) * 3 + i][
                            :, : group_sizes[g] * r
                        ].rearrange("p (t r) -> p t r", t=group_sizes[g]),
                        in_=dgroup(g, i),
                        func=mybir.ActivationFunctionType.Square,
                        accum_out=acc[:, g * 6 + i : g * 6 + i + 1],
                    ).then_inc(act_sem, 1)

        @block.vector
        def _(vector):
            for kind, x in dve_order:
                if kind == "sub":
                    vector.wait_ge(dma_sems[x], 16)
                    vector.tensor_tensor(
                        out=half(x, 0),
                        in0=half(x, 0),
                        in1=half(x, 1),
                        op=mybir.AluOpType.subtract,
                    ).then_inc(dve_sem, 1)
                else:
                    vector.wait_ge(dve_sem, sub_done[ends[x]])
                    for k, (i, j) in enumerate(_PAIRS):
                        vector.scalar_tensor_tensor(
                            out=dve_scrs[(x % 2) * 3 + k][
                                :, : group_sizes[x] * r
                            ].rearrange("p (t r) -> p t r", t=group_sizes[x]),
                            in0=dgroup(x, i),
                            scalar=1.0,
                            in1=dgroup(x, j),
                            op0=mybir.AluOpType.mult,
                            op1=mybir.AluOpType.mult,
                            accum_out=acc[:, x * 6 + 3 + k : x * 6 + 4 + k],
                        ).then_inc(dve_sem, 1)

    nc.compile()
    return nc


def build_gram_kernel_v4(n_rows: int, n_tiles: int = 32, n_bufs: int = 16,
                         n_dbufs: int = 32,
                         sub_sizes: tuple = (1, 1, 1, 1) + (4,) * 6 + (2, 1, 1),
                         cr_sizes: tuple = (4,) * 8,
                         sq_sizes: tuple = (4,) * 8,
                         n_scalar_dmas: int = 0, use_ttr: bool = False,
                         gp_batches: tuple = (),
                         skip_exit_barrier: bool = True):
    """Planar fp32 over HWDGE + bf16 d-ring; fused sub batches.

    Input pt [n_tiles, 128, 6r] f32 planar (one contiguous 6 KB chunk
    per tile/partition -> ~413 GB/s vs 402 for split chunks; HWDGE so no
    SWDGE descriptor-ring contention, which made SDMA engine 15 a 17%
    straggler that paced the whole SWDGE-cast variant).

    DVE subtracts in fused batches (one op across a batch's tiles, 3D AP
    [t, h] over the fp32 ring) writing unit-stride bf16 into the d-ring;
    cross/square reduces then run on step-1 bf16 (no stride-3 penalty).
    Crosses (DVE) use 8-tile groups, squares (ACT) 4-tile groups — the
    tilings are independent since the host sums all partial columns.
    All sizes taper to 1 tile at the end so the post-stream tail is just
    the last tile's sub + three N=r reduces.

    acc column layout (out [128, 3*(n_sq+n_cr)]): all-but-last sq groups,
    all-but-last cr groups, last sq group, last cr group — so each of the
    two output flushes is ONE contiguous dma and the final flush covers
    only the last-tile groups.
    """
    assert sum(sub_sizes) == n_tiles and sum(cr_sizes) == n_tiles
    assert sum(sq_sizes) == n_tiles
    assert n_rows % (P * n_tiles) == 0
    r = n_rows // (P * n_tiles)
    h = 3 * r
    f32, bf16 = mybir.dt.float32, mybir.dt.bfloat16

    def bounds(sizes):
        ends, e = [], -1
        for sz in sizes:
            e += sz
            ends.append(e)
        return [e - sz + 1 for e, sz in zip(ends, sizes)], ends

    sub_starts, sub_ends = bounds(sub_sizes)
    cr_starts, cr_ends = bounds(cr_sizes)
    sq_starts, sq_ends = bounds(sq_sizes)
    n_sq, n_cr = len(sq_sizes), len(cr_sizes)
    # fused subs and cross groups need their tiles contiguous in the rings
    for s, e in zip(sub_starts, sub_ends):
        assert (s % n_bufs) + (e - s) < n_bufs and (s % n_dbufs) + (e - s) < n_dbufs
    for s, e in zip(cr_starts, cr_ends):
        assert (s % n_dbufs) + (e - s) < n_dbufs
    for s, e in zip(sq_starts, sq_ends):
        assert (s % n_dbufs) + (e - s) < n_dbufs

    # acc columns ordered so the early flush [0, chunk_split) covers all
    # groups finishing by tile E1, and the final flush only the rest
    E1 = n_tiles - 5
    sq_early = [g for g in range(n_sq) if sq_ends[g] <= E1]
    sq_late = [g for g in range(n_sq) if sq_ends[g] > E1]
    cr_early = [g for g in range(n_cr) if cr_ends[g] <= E1]
    cr_late = [g for g in range(n_cr) if cr_ends[g] > E1]
    # groups complete in index order on each engine, so early must be a prefix
    assert sq_early == list(range(len(sq_early)))
    assert cr_early == list(range(len(cr_early)))
    sq_col, cr_col = {}, {}
    c = 0
    for g in sq_early:
        sq_col[g] = c; c += 3
    for g in cr_early:
        cr_col[g] = c; c += 3
    chunk_split = c
    for g in sq_late:
        sq_col[g] = c; c += 3
    for g in cr_late:
        cr_col[g] = c; c += 3
    n_cols = c

    # DRAM partition stride padded 6 KB -> 7 KB: gcd(7 KB, 16 KB HBM
    # channel period) = 1 KB so every partition/engine cycles through all
    # channel phases; at the natural 6 KB stride each SDMA engine was
    # pinned to a channel subset and intermittently straggled ~20%.
    pad = 256
    nc = bacc.Bacc("TRN2", target_bir_lowering=False, debug=False)
    pt = nc.dram_tensor("pt", [n_tiles, P, 2 * h + pad], f32,
                        kind="ExternalInput")
    out = nc.dram_tensor("partials", [P, n_cols], f32, kind="ExternalOutput")
    pt_v = pt[:]

    ring = nc.alloc_sbuf_tensor("ring", [P, n_bufs * 2 * h], f32).ap()
    dring = nc.alloc_sbuf_tensor("dring", [P, n_dbufs * h], bf16).ap()
    acc = nc.alloc_sbuf_tensor("acc", [P, n_cols], f32).ap()
    max_cr = max(cr_sizes)
    max_sq = max(sq_sizes)
    dve_scrs = [
        nc.alloc_sbuf_tensor(f"dve_scr{k}", [P, max_cr * r], bf16).ap()
        for k in range(3)
    ]
    act_scrs = [
        nc.alloc_sbuf_tensor(f"act_scr{k}", [P, max_sq * r], bf16).ap()
        for k in range(3)
    ]

    dma_sems = [nc.alloc_semaphore(f"dma_sem{i}") for i in range(n_bufs)]
    out_sem = nc.alloc_semaphore("out_sem")
    dve_sem = nc.alloc_semaphore("dve_sem")
    act_sem = nc.alloc_semaphore("act_sem")
    gp_sem = nc.alloc_semaphore("gp_sem")

    def sub_views(b):
        s, sz = sub_starts[b], sub_sizes[b]
        rs = s % n_bufs
        w = ring[:, rs * 2 * h : (rs + sz) * 2 * h].rearrange(
            "p (t w h) -> p t w h", t=sz, w=2)
        ds = s % n_dbufs
        d = dring[:, ds * h : (ds + sz) * h].rearrange(
            "p (t h) -> p t h", t=sz)
        return w[:, :, 0, :], w[:, :, 1, :], d

    def dgroup(starts, sizes, g, i):
        s, sz = starts[g], sizes[g]
        ds = s % n_dbufs
        w = dring[:, ds * h : (ds + sz) * h]
        return w.rearrange("p (t c r) -> p t c r", t=sz, c=3)[:, :, i, :]

    # Sub batches listed in gp_batches run on the (otherwise idle) GpSimd
    # engine — DVE sub work shrinks below the DMA-arrival window so
    # schedule jitter can't cascade.  GpSimd elementwise is ~2.2 ns/elem,
    # under the 4-tile arrival period.
    n_batches = len(sub_sizes)
    gp_batches = tuple(sorted(gp_batches))
    assert all(0 <= b < n_batches for b in gp_batches)
    gp_done = {b: i + 1 for i, b in enumerate(gp_batches)}

    # DVE emission: cross group g emitted as soon as every DVE sub batch
    # overlapping it has been emitted (no stagger — a stagger delays
    # crosses a whole batch and starves the DVE when batches wait on DMA
    # arrivals; gp-run batches are covered by gp_sem waits instead)
    def overlapping_dve_batches(g):
        return [b for b in range(n_batches)
                if b not in gp_done
                and sub_ends[b] >= cr_starts[g] and sub_starts[b] <= cr_ends[g]]

    dve_order = []
    emitted_cr = set()
    for b in range(n_batches):
        if b in gp_done:
            continue
        dve_order.append(("sub", b))
        for g in range(n_cr):
            if g not in emitted_cr and all(
                bb <= b for bb in overlapping_dve_batches(g)
            ):
                emitted_cr.add(g)
                dve_order.append(("cr", g))
    assert len(emitted_cr) == n_cr
    sub_done, crg_done = {}, {}
    v = 0
    for kind, x in dve_order:
        v += 1 if kind == "sub" else 3
        (sub_done if kind == "sub" else crg_done)[x] = v

    def sub_targets(last_tile):
        """(dve_sem, gp_sem) targets ensuring every sub batch touching
        tiles <= last_tile has completed."""
        dve_t = gp_t = 0
        for b in range(n_batches):
            if sub_starts[b] <= last_tile:
                if b in gp_done:
                    gp_t = max(gp_t, gp_done[b])
                else:
                    dve_t = max(dve_t, sub_done[b])
        return dve_t, gp_t

    import contextlib

    @contextlib.contextmanager
    def _block():
        with nc.Block(no_gpsimd_drain=True) as blk:
            try:
                yield blk
            finally:
                if skip_exit_barrier:
                    nc.all_engine_barrier = lambda **kw: None
        if skip_exit_barrier:
            del nc.all_engine_barrier

    def emit_dma(eng, t):
        # split each partition's 6 KB chunk into two 3 KB descriptors:
        # >4 KB packets degrade SDMA round-robin under concurrent engine
        # load (SDMA 15 became a 20% straggler and paced every tile sem)
        eng.dma_start(
            out=ring[:, (t % n_bufs) * 2 * h : (t % n_bufs + 1) * 2 * h]
            .rearrange("p (x k) -> p x k", x=2),
            in_=pt_v[t][:, : 2 * h].rearrange("p (x k) -> p x k", x=2),
        ).then_inc(dma_sems[t % n_bufs], 16)

    with _block() as block:

        if gp_batches:

            @block.gpsimd
            def _(gp):
                for b in gp_batches:
                    e = sub_ends[b]
                    gp.wait_ge(dma_sems[e % n_bufs], 16 * (e // n_bufs + 1))
                    p_v, t_v, d_v = sub_views(b)
                    gp.tensor_tensor(
                        out=d_v, in0=p_v, in1=t_v,
                        op=mybir.AluOpType.subtract,
                    ).then_inc(gp_sem, 1)

        @block.sync
        def _(sync):
            for t in range(n_scalar_dmas, n_tiles):
                if t >= n_bufs:
                    dve_t, gp_t = sub_targets(t - n_bufs)
                    if dve_t:
                        sync.wait_ge(dve_sem, dve_t)
                    if gp_t:
                        sync.wait_ge(gp_sem, gp_t)
                emit_dma(sync, t)
            n_out = 0
            for lo, hi, n_sq_done, n_cr_done in (
                (0, chunk_split, len(sq_early), len(cr_early)),
                (chunk_split, n_cols, n_sq, n_cr),
            ):
                sync.wait_ge(act_sem, 3 * n_sq_done)
                sync.wait_ge(dve_sem, crg_done[n_cr_done - 1])
                sync.dma_start(
                    out=out[:, lo:hi], in_=acc[:, lo:hi]
                ).then_inc(out_sem, 16)
                n_out += 16
            sync.wait_ge(out_sem, n_out)

        @block.scalar
        def _(scalar):
            for t in range(n_scalar_dmas):
                emit_dma(scalar, t)
            for g in range(n_sq):
                dve_t, gp_t = sub_targets(sq_ends[g])
                if dve_t:
                    scalar.wait_ge(dve_sem, dve_t)
                if gp_t:
                    scalar.wait_ge(gp_sem, gp_t)
                for i in range(3):
                    scalar.activation(
                        out=act_scrs[i][:, : sq_sizes[g] * r].rearrange(
                            "p (t r) -> p t r", t=sq_sizes[g]),
                        in_=dgroup(sq_starts, sq_sizes, g, i),
                        func=mybir.ActivationFunctionType.Square,
                        accum_out=acc[:, sq_col[g] + i : sq_col[g] + i + 1],
                    ).then_inc(act_sem, 1)

        @block.vector
        def _(vector):
            for kind, x in dve_order:
                if kind == "sub":
                    p_v, t_v, d_v = sub_views(x)
                    s, e = sub_starts[x], sub_ends[x]
                    vector.wait_ge(dma_sems[e % n_bufs], 16 * (e // n_bufs + 1))
                    if s >= n_dbufs:
                        # d-slot reuse: ACT squares over the evicted tiles
                        # must be done (DVE's own reads are program-ordered)
                        gp = next(g for g in range(n_sq)
                                  if sq_ends[g] >= e - n_dbufs)
                        vector.wait_ge(act_sem, 3 * (gp + 1))
                    vector.tensor_tensor(
                        out=d_v, in0=p_v, in1=t_v,
                        op=mybir.AluOpType.subtract,
                    ).then_inc(dve_sem, 1)
                else:
                    dve_t, gp_t = sub_targets(cr_ends[x])
                    if dve_t:
                        vector.wait_ge(dve_sem, dve_t)
                    if gp_t:
                        vector.wait_ge(gp_sem, gp_t)
                    for k, (i, j) in enumerate(_PAIRS):
                        scr = dve_scrs[k][:, : cr_sizes[x] * r].rearrange(
                            "p (t r) -> p t r", t=cr_sizes[x])
                        if use_ttr:
                            # fused multiply + sum-reduce, accumulator
                            # written directly by the instruction
                            vector.tensor_tensor_reduce(
                                out=scr,
                                in0=dgroup(cr_starts, cr_sizes, x, i),
                                in1=dgroup(cr_starts, cr_sizes, x, j),
                                scale=1.0,
                                scalar=0.0,
                                op0=mybir.AluOpType.mult,
                                op1=mybir.AluOpType.add,
                                accum_out=acc[:, cr_col[x] + k : cr_col[x] + k + 1],
                            ).then_inc(dve_sem, 1)
                        else:
                            vector.scalar_tensor_tensor(
                                out=scr,
                                in0=dgroup(cr_starts, cr_sizes, x, i),
                                scalar=1.0,
                                in1=dgroup(cr_starts, cr_sizes, x, j),
                                op0=mybir.AluOpType.mult,
                                op1=mybir.AluOpType.mult,
                                accum_out=acc[:, cr_col[x] + k : cr_col[x] + k + 1],
                            ).then_inc(dve_sem, 1)

    # record triple-row roles for the host-side unpack
    _V4_SQ_ROWS.clear()
    _V4_SQ_ROWS.extend(sq_col[g] // 3 for g in range(n_sq))
    _V4_CR_ROWS.clear()
    _V4_CR_ROWS.extend(cr_col[g] // 3 for g in range(n_cr))

    nc.compile()
    return nc


_V4_SQ_ROWS: list = []
_V4_CR_ROWS: list = []


def gram_from_partials_v4(partials: np.ndarray) -> np.ndarray:
    """v4 partials [..., 128, 3*(n_sq+n_cr)] -> 3x3 Gram (float64).

    Column triples ordered [sq_early, cr_early, sq_late, cr_late]; the
    builder records which triple rows are squares vs crosses in
    _V4_SQ_ROWS/_V4_CR_ROWS.
    """
    s = partials.astype(np.float64).reshape(-1, partials.shape[-1]).sum(axis=0)
    tri = s.reshape(-1, 3)
    sq = tri[_V4_SQ_ROWS].sum(axis=0)
    cr = tri[_V4_CR_ROWS].sum(axis=0)
    g = np.empty((3, 3), dtype=np.float64)
    g[0, 0], g[1, 1], g[2, 2] = sq
    for k, (i, j) in enumerate(_PAIRS):
        g[i, j] = g[j, i] = cr[k]
    return g


def planarize(predictions: np.ndarray, targets: np.ndarray,
              n_tiles: int = 32, pad: int = 256) -> np.ndarray:
    """[B,3] pred/targ -> per-core planar tiles [cores, n_tiles, P, 6r+pad].

    Per (tile, partition): [p0 r | p1 r | p2 r | t0 r | t1 r | t2 r | pad].
    """
    b = predictions.shape[0]
    n_rows = b // N_CORES
    r = n_rows // (P * n_tiles)
    out = np.empty((N_CORES, n_tiles, P, 6 * r + pad), dtype=np.float32)
    pv = out[..., : 3 * r].reshape(N_CORES, n_tiles, P, 3, r)
    tv = out[..., 3 * r : 6 * r].reshape(N_CORES, n_tiles, P, 3, r)
    pv[:] = np.asarray(predictions, dtype=np.float32).reshape(
        N_CORES, n_tiles, P, r, 3).transpose(0, 1, 2, 4, 3)
    tv[:] = np.asarray(targets, dtype=np.float32).reshape(
        N_CORES, n_tiles, P, r, 3).transpose(0, 1, 2, 4, 3)
    return out


_NC_CACHE: dict[tuple, object] = {}


def _get_nc(n_rows: int, n_tiles: int, use_act: bool, raw: bool = False,
            group: int = 4, **kw):
    key = (n_rows, n_tiles, use_act, raw, group, tuple(sorted(kw.items())))
    if key not in _NC_CACHE:
        if raw:
            _NC_CACHE[key] = build_gram_kernel_v4(n_rows, n_tiles, **kw)
        else:
            _NC_CACHE[key] = build_gram_kernel(n_rows, n_tiles, use_act)
    return _NC_CACHE[key]


def gram_from_partials(partials: np.ndarray, n_tiles: int | None = None) -> np.ndarray:
    """Device partials -> full 3x3 Gram matrix (float64).

    Dispatches on column count: 51 -> v4 layout, else v2/v3 group-major
    layout (col 6g+i = sq_i, col 6g+3+k = cross pair k).
    """
    if partials.shape[-1] == 3 * (len(_V4_SQ_ROWS) + len(_V4_CR_ROWS)):
        return gram_from_partials_v4(partials)
    slots = partials.shape[-1] // 6
    s = partials.astype(np.float64).reshape(-1, slots, 6).sum(axis=0).sum(axis=0)
    g = np.empty((3, 3), dtype=np.float64)
    g[0, 0], g[1, 1], g[2, 2] = s[0:3]
    for k, (i, j) in enumerate(_PAIRS):
        g[i, j] = g[j, i] = s[3 + k]
    return g


def run_device_partials(predictions: np.ndarray, targets: np.ndarray,
                        n_tiles: int = 32, use_act: bool = True,
                        raw: bool = True, group: int = 4, **run_kwargs):
    """Shard over N_CORES, run on device, return per-core partials + results."""
    b = predictions.shape[0]
    assert b % N_CORES == 0
    n_rows = b // N_CORES
    nc = _get_nc(n_rows, n_tiles, use_act, raw, group)
    planar = planarize(predictions, targets, n_tiles)
    in_maps = [{"pt": planar[c]} for c in range(N_CORES)]
    res = run_bass_kernel_spmd(nc, in_maps, list(range(N_CORES)), **run_kwargs)
    partials = np.stack([r["partials"] for r in res.results])
    return partials, res


def kernel(predictions: np.ndarray, targets: np.ndarray, sigma: np.ndarray) -> np.ndarray:
    predictions = np.asarray(predictions, dtype=np.float32)
    targets = np.asarray(targets, dtype=np.float32)
    sigma64 = np.asarray(sigma, dtype=np.float64)

    partials, _ = run_device_partials(predictions, targets, n_tiles=32, raw=True)
    g = gram_from_partials(partials)

    sigma_inv = np.linalg.inv(sigma64)
    _, logdet = np.linalg.slogdet(sigma64)
    mean_mahal = float((sigma_inv * g).sum()) / predictions.shape[0]
    loss = abs(logdet + mean_mahal)
    return np.float32(loss)

